# revision 9
# baseline (speedup 1.0000x reference)
"""Trainium2 Bass kernel for nn_CompatStatefulSelfModMixerModel.

Fully on-device: input projection, 2x (token SRWM scan + token mixer +
channel SRWM scan + channel mixer), final LN + patch-mean, output SRWM,
linear head - one Bass program per core. Data-parallel over batch B=8
across 8 NeuronCores (1 sample/core, weights replicated, no collectives).

Scan fast-math: fast-weight state kept in bf16 (DVE 2x_1p mode for all
big tensor_tensor ops), state split into y-rows (gpsimd-updated) and
q/k/beta-rows (vector-updated), softmax without max-subtraction, single
activation table set (rsqrt via exp(-0.5*ln(v)), sigmoid via exp).
"""
import sys

sys.path.insert(0, "/opt/trn_rl_repo")

import numpy as np

import concourse.bacc as bacc
import concourse.tile as tile
from concourse import mybir

F32 = mybir.dt.float32
BF = mybir.dt.bfloat16
AF = mybir.ActivationFunctionType
ALU = mybir.AluOpType
AX = mybir.AxisListType

S, B, NCLS = 16, 8, 5
D, H, DH = 256, 16, 16
PS, IMG = 7, 28
P = 16
L = 2
PD = 49
QIN = PD + NCLS  # 54
DFT = 128
EPS = 1e-5


def _srwm_scan(nc, wk, npart, C, sty_f, sty_b, stq, x_of_step, ys_all,
               use_gp=True):
    """S steps of the SRWM recurrence.

    sty_f: f32 [npart, C*256] - Wy rows master (gpsimd-updated)
    sty_b: bf16 mirror of sty_f (vector-read for the vy path)
    stq: bf16 [npart, C*576] - Wq/Wk/wb rows, viewed [p, c, g, 16]
         (0:16 Wq, 16:32 Wk, 32:36 wb)
    x_of_step(s) -> bf16 AP [npart, C, 16]
    ys_all: fp32 [npart, S*C*16]; y_t lands at [:, s, c, :].
    """
    styf_v = sty_f[:, :].rearrange("p (c i j) -> p c i j", c=C, j=16)
    sty_v = sty_b[:, :].rearrange("p (c i j) -> p c i j", c=C, j=16)
    stq_v = stq[:, :].rearrange("p (c g j) -> p c g j", c=C, j=16)
    GE = nc.gpsimd if use_gp else nc.vector

    for s in range(S):
        xt = x_of_step(s)  # [p, C, 16] bf16
        # y path: y_t = Wy . x in f32 (mul on gpsimd, reduce on vector)
        zy0 = wk.tile([npart, C * 256], F32, tag="sc_zy0")
        zy0_v = zy0[:, :].rearrange("p (c i j) -> p c i j", c=C, j=16)
        GE.tensor_mul(zy0_v, styf_v,
                      xt.unsqueeze(2).broadcast_to([npart, C, 16, 16]))
        y_out = ys_all[:, :].rearrange("p (s c j) -> p s c j", s=S, c=C)[:, s]
        nc.vector.tensor_reduce(y_out, zy0_v, axis=AX.X, op=ALU.add)

        # seg = [q; k; b-logits] = stq . x
        zq = wk.tile([npart, C * 576], BF, tag="sc_zq")
        zq_v = zq[:, :].rearrange("p (c g j) -> p c g j", c=C, j=16)
        nc.vector.tensor_mul(zq_v, stq_v,
                             xt.unsqueeze(2).broadcast_to([npart, C, 36, 16]))
        seg = wk.tile([npart, C * 36], F32, tag="sc_seg")
        seg_v = seg[:, :].rearrange("p (c g) -> p c g", c=C)
        nc.vector.tensor_reduce(seg_v, zq_v, axis=AX.X, op=ALU.add)

        # merged softmax(q), softmax(k) - no max subtraction
        eqk = wk.tile([npart, C * 32], F32, tag="sc_eqk")
        eqk_v = eqk[:, :].rearrange("p (c t j) -> p c t j", c=C, t=2)
        nc.scalar.activation(
            out=eqk_v,
            in_=seg_v[:, :, 0:32].rearrange("p c (t j) -> p c t j", t=2),
            func=AF.Exp)
        sums = wk.tile([npart, C * 2], F32, tag="sc_sums")
        sums_v = sums[:, :].rearrange("p (c t) -> p c t", c=C)
        nc.vector.tensor_reduce(sums_v, eqk_v, axis=AX.X, op=ALU.add)
        rec = wk.tile([npart, C * 2], F32, tag="sc_rec")
        nc.vector.reciprocal(rec[:, :], sums[:, :])
        rec_v = rec[:, :].rearrange("p (c t) -> p c t", c=C)
        kq = wk.tile([npart, C * 32], BF, tag="sc_kq")
        kq_v = kq[:, :].rearrange("p (c t j) -> p c t j", c=C, t=2)
        nc.vector.tensor_mul(kq_v, eqk_v,
                             rec_v.unsqueeze(3).broadcast_to([npart, C, 2, 16]))
        qs = kq_v[:, :, 0]
        ks = kq_v[:, :, 1]
        e = wk.tile([npart, C * 16], BF, tag="sc_e")
        e_v = e[:, :].rearrange("p (c j) -> p c j", c=C)
        nc.vector.tensor_sub(e_v, qs, ks)

        # beta = sigmoid(b-logits) via exp
        bta = wk.tile([npart, C * 4], F32, tag="sc_beta")
        bta_v = bta[:, :].rearrange("p (c w) -> p c w", c=C)
        nc.scalar.activation(out=bta_v, in_=seg_v[:, :, 32:36], func=AF.Exp,
                             scale=-1.0)
        nc.vector.tensor_scalar(bta[:, :], bta[:, :], 1.0, None, ALU.add)
        nc.vector.reciprocal(bta[:, :], bta[:, :])

        # d rows 16:52 = stq . (qs - ks)
        z2 = wk.tile([npart, C * 576], BF, tag="sc_z2")
        z2_v = z2[:, :].rearrange("p (c g j) -> p c g j", c=C, j=16)
        nc.vector.tensor_mul(z2_v, stq_v,
                             e_v.unsqueeze(2).broadcast_to([npart, C, 36, 16]))
        d = wk.tile([npart, C * 52], F32, tag="sc_d")
        d_v = d[:, :].rearrange("p (c g) -> p c g", c=C)
        nc.vector.tensor_reduce(d_v[:, :, 16:52], z2_v, axis=AX.X, op=ALU.add)

        # vy over both qs and ks: vykq[c,t,i] = sum_j Wy[c,i,j]*kq[c,t,j]
        zy = wk.tile([npart, C * 512], BF, tag="sc_zy")
        zy_v = zy[:, :].rearrange("p (c t i j) -> p c t i j", c=C, t=2, j=16)
        for t in range(2):
            nc.vector.tensor_mul(
                zy_v[:, :, t], sty_v,
                kq_v[:, :, t].unsqueeze(2).broadcast_to([npart, C, 16, 16]))
        vykq = wk.tile([npart, C * 32], F32, tag="sc_vykq")
        vykq_v = vykq[:, :].rearrange("p (c t i) -> p c t i", c=C, t=2)
        nc.vector.tensor_reduce(
            vykq[:, :],
            zy[:, :].rearrange("p (a j) -> p a j", j=16),
            axis=AX.X, op=ALU.add)

        # v-softmax on vy_q; d rows 0:16 = softmax(vy_q) - vy_k
        ev = wk.tile([npart, C * 16], F32, tag="sc_ev")
        ev_v = ev[:, :].rearrange("p (c i) -> p c i", c=C)
        nc.scalar.activation(out=ev_v, in_=vykq_v[:, :, 0], func=AF.Exp)
        vs = wk.tile([npart, C], F32, tag="sc_vs")
        nc.vector.tensor_reduce(vs[:, :], ev_v, axis=AX.X, op=ALU.add)
        nc.vector.reciprocal(vs[:, :], vs[:, :])
        for c in range(C):
            nc.vector.scalar_tensor_tensor(
                out=d_v[:, c, 0:16], in0=ev_v[:, c], scalar=vs[:, c:c + 1],
                in1=vykq_v[:, c, 1], op0=ALU.mult, op1=ALU.subtract)

        # expand beta to per-row b52, then d2x = d * b52 as paired bf16
        b52 = wk.tile([npart, C * 52], F32, tag="sc_b52")
        b52_v = b52[:, :].rearrange("p (c g) -> p c g", c=C)
        nc.vector.tensor_scalar(
            b52_v[:, :, 0:48].rearrange("p c (w g) -> p c w g", g=16),
            bta_v[:, :, 0:3].unsqueeze(3).broadcast_to([npart, C, 3, 16]),
            1.0, None, ALU.mult)
        nc.vector.tensor_scalar(
            b52_v[:, :, 48:52],
            bta_v[:, :, 3:4].broadcast_to([npart, C, 4]),
            1.0, None, ALU.mult)
        d2x = wk.tile([npart, C * 104], BF, tag="sc_d2x")
        d2x_v = d2x[:, :].rearrange("p (c g t) -> p c g t", c=C, t=2)
        nc.vector.tensor_mul(
            d2x_v,
            d_v.unsqueeze(3).broadcast_to([npart, C, 52, 2]),
            b52_v.unsqueeze(3).broadcast_to([npart, C, 52, 2]))

        # state update: W += d (x) ks  (paired views keep 2x mode)
        kspq = ks.rearrange("p c (j2 t) -> p c j2 t", t=2)  # [p, C, 8, 2]
        zu = wk.tile([npart, C * 576], BF, tag="sc_zu")
        zu_p = zu[:, :].rearrange("p (c g j2 t) -> p c g j2 t", c=C, j2=8, t=2)
        for c in range(C):
            nc.vector.tensor_mul(
                zu_p[:, c],
                d2x_v[:, c, 16:52].unsqueeze(2)
                .broadcast_to([npart, 36, 8, 2]),
                kspq[:, c].unsqueeze(1).broadcast_to([npart, 36, 8, 2]))
        nc.vector.tensor_add(stq[:, :], stq[:, :], zu[:, :])
        zuy = wk.tile([npart, C * 256], F32, tag="sc_zuy")
        zuy_p = zuy[:, :].rearrange("p (c g j2 t) -> p c g j2 t",
                                    c=C, j2=8, t=2)
        for c in range(C):
            GE.tensor_mul(
                zuy_p[:, c],
                d2x_v[:, c, 0:16].unsqueeze(2)
                .broadcast_to([npart, 16, 8, 2]),
                kspq[:, c].unsqueeze(1).broadcast_to([npart, 16, 8, 2]))
        GE.tensor_add(sty_f[:, :], sty_f[:, :], zuy[:, :])
        if s + 1 < S:
            GE.tensor_copy(sty_b[:, :], sty_f[:, :])


def _ln(nc, wk, tag, npart, A1, A2, J, out_v, in_v, g_v, b_v):
    """LayerNorm over innermost J of 4-dim views [npart, A1, A2, J].

    rstd computed as exp(-0.5*ln(v+eps)) to stay in one act table set.
    """
    A = A1 * A2
    m = wk.tile([npart, A], F32, tag=f"ln_m_{tag}")
    v = wk.tile([npart, A], F32, tag=f"ln_v_{tag}")
    sq = wk.tile([npart, A * J], F32, tag="ln_sq")
    m4 = m[:, :].rearrange("p (a b) -> p a b", a=A1)
    v4 = v[:, :].rearrange("p (a b) -> p a b", a=A1)
    nc.vector.tensor_reduce(m4, in_v, axis=AX.X, op=ALU.add)
    nc.vector.tensor_scalar(m[:, :], m[:, :], -1.0 / J, None, ALU.mult)
    m_b = m4.unsqueeze(3).broadcast_to([npart, A1, A2, J])
    nc.vector.tensor_add(out_v, in_v, m_b)
    sq_v = sq[:, :].rearrange("p (a b j) -> p a b j", a=A1, b=A2)
    nc.vector.tensor_mul(sq_v, out_v, out_v)
    nc.vector.tensor_reduce(v4, sq_v, axis=AX.X, op=ALU.add)
    nc.vector.tensor_scalar(v[:, :], v[:, :], 1.0 / J, EPS, ALU.mult, ALU.add)
    nc.scalar.activation(out=v[:, :], in_=v[:, :], func=AF.Ln)
    nc.scalar.activation(out=v[:, :], in_=v[:, :], func=AF.Exp, scale=-0.5)
    v_b = v4.unsqueeze(3).broadcast_to([npart, A1, A2, J])
    nc.vector.tensor_mul(out_v, out_v, v_b)
    nc.vector.tensor_mul(out_v, out_v, g_v)
    nc.vector.tensor_add(out_v, out_v, b_v)


GELU_C = 1.5957691216057308  # 2*sqrt(2/pi)
GELU_A = 0.044715


def _gelu(nc, wk, out_v, pt_v, bias_ap, npart, F):
    """out = gelu_tanh(pt + bias); pt may be PSUM. [npart, F] views.

    sigmoid computed via exp to stay in one act table set.
    """
    xb = wk.tile([npart, F], F32, tag="gelu_xb")
    x2 = wk.tile([npart, F], F32, tag="gelu_x2")
    nc.scalar.activation(out=xb[:, :], in_=pt_v, func=AF.Identity,
                         bias=bias_ap, scale=1.0)
    nc.scalar.activation(out=x2[:, :], in_=pt_v, func=AF.Square,
                         bias=bias_ap, scale=1.0)
    nc.vector.tensor_scalar(x2[:, :], x2[:, :], GELU_A, 1.0, ALU.mult, ALU.add)
    nc.vector.tensor_mul(x2[:, :], x2[:, :], xb[:, :])
    nc.scalar.activation(out=x2[:, :], in_=x2[:, :], func=AF.Exp,
                         scale=-GELU_C)
    nc.vector.tensor_scalar(x2[:, :], x2[:, :], 1.0, None, ALU.add)
    nc.vector.reciprocal(x2[:, :], x2[:, :])
    nc.vector.tensor_mul(out_v, xb[:, :], x2[:, :])


def build_nc(debug=()):
    nc = bacc.Bacc(None, target_bir_lowering=False)
    dp = lambda nm, shp: nc.declare_dram_parameter(nm, shp, F32, isOutput=False)

    xinT = dp("xinT", [QIN, S * P])
    inW = dp("inW", [QIN, D])
    inb = dp("inb", [128, 2])
    I128 = dp("I128", [128, 128])
    MEAN = dp("MEAN", [128, 32])
    tkSy = dp("tkSy", [L * 128, 512])
    tkSq = dp("tkSq", [L * 128, 1152])
    chSy = dp("chSy", [L * 128, 512])
    chSq = dp("chSq", [L * 128, 1152])
    oSy = dp("oSy", [16, 256])
    oSq = dp("oSq", [16, 576])
    tkg = dp("tkg", [128, L * 16])
    tkb = dp("tkb", [128, L * 16])
    tkmg = dp("tkmg", [128, L * 256])
    tkmb = dp("tkmb", [128, L * 256])
    chg = dp("chg", [128, L * 256])
    chb = dp("chb", [128, L * 256])
    chmg = dp("chmg", [128, L * 256])
    chmb = dp("chmb", [128, L * 256])
    flng = dp("flng", [128, 256])
    flnb = dp("flnb", [128, 256])
    og = dp("og", [16, 256])
    ob = dp("ob", [16, 256])
    tkmW1 = dp("tkmW1", [16, L * 64])
    tkmB1 = dp("tkmB1", [64, L])
    tkmW2 = dp("tkmW2", [64, L * 16])
    tkmB2 = dp("tkmB2", [16, L])
    chmW1 = dp("chmW1", [128, L * 256])
    chmB1 = dp("chmB1", [128, L])
    chmW2 = dp("chmW2", [128, L * 256])
    chmB2 = dp("chmB2", [128, L * 2])
    outW = dp("outW", [128, 10])
    outB = dp("outB", [16, 5])
    out = nc.declare_dram_parameter("out", [S, NCLS], F32, isOutput=True)
    dbg = {}
    for nm, shp in debug:
        dbg[nm] = nc.declare_dram_parameter(nm, shp, F32, isOutput=True)

    with tile.TileContext(nc) as tc:
        with tc.tile_pool(name="cst", bufs=1) as cst, \
             tc.tile_pool(name="hb", bufs=1) as hb, \
             tc.tile_pool(name="wk", bufs=2) as wk, \
             tc.tile_pool(name="pst", bufs=4, space="PSUM") as pst, \
             tc.tile_pool(name="psm", bufs=2, space="PSUM") as psm, \
             tc.tile_pool(name="dr", bufs=1, space="DRAM") as dr:

            def load(tensor, shape, tag, sl=None):
                t = cst.tile(shape, F32, tag=tag)
                nc.gpsimd.dma_start(out=t, in_=tensor[:] if sl is None else sl)
                return t

            i_t = load(I128, [128, 128], "I128")
            mean_t = load(MEAN, [128, 32], "MEAN")
            tkg_t = load(tkg, [128, L * 16], "tkg")
            tkb_t = load(tkb, [128, L * 16], "tkb")
            tkmg_t = load(tkmg, [128, L * 256], "tkmg")
            tkmb_t = load(tkmb, [128, L * 256], "tkmb")
            chg_t = load(chg, [128, L * 256], "chg")
            chb_t = load(chb, [128, L * 256], "chb")
            chmg_t = load(chmg, [128, L * 256], "chmg")
            chmb_t = load(chmb, [128, L * 256], "chmb")
            flng_t = load(flng, [128, 256], "flng")
            flnb_t = load(flnb, [128, 256], "flnb")
            og_t = load(og, [16, 256], "og")
            ob_t = load(ob, [16, 256], "ob")
            tkmW1_t = load(tkmW1, [16, L * 64], "tkmW1")
            tkmB1_t = load(tkmB1, [64, L], "tkmB1")
            tkmW2_t = load(tkmW2, [64, L * 16], "tkmW2")
            tkmB2_t = load(tkmB2, [16, L], "tkmB2")
            chmW1_t = load(chmW1, [128, L * 256], "chmW1")
            chmB1_t = load(chmB1, [128, L], "chmB1")
            chmW2_t = load(chmW2, [128, L * 256], "chmW2")
            chmB2_t = load(chmB2, [128, L * 2], "chmB2")
            outW_t = load(outW, [128, 10], "outW")
            outB_t = load(outB, [16, 5], "outB")

            # ---- A. input projection -> hT (bf16) [128, (c, s*16+p)] ----
            xin_t = cst.tile([QIN, S * P], F32, tag="xin")
            w_t = cst.tile([QIN, D], F32, tag="inW")
            b_t = cst.tile([128, 2], F32, tag="inb")
            nc.gpsimd.dma_start(out=xin_t, in_=xinT[:])
            nc.gpsimd.dma_start(out=w_t, in_=inW[:])
            nc.gpsimd.dma_start(out=b_t, in_=inb[:])
            hT = hb.tile([128, 512], BF, tag="hT0")
            for c in range(2):
                pt = psm.tile([128, S * P], F32, tag="psm")
                nc.tensor.matmul(pt, w_t[:, c * 128:(c + 1) * 128], xin_t,
                                 start=True, stop=True)
                nc.scalar.activation(out=hT[:, c * 256:(c + 1) * 256], in_=pt,
                                     func=AF.Identity, bias=b_t[:, c:c + 1],
                                     scale=1.0)

            def t_to_sp(src, dst):
                """src [128, (c,sp)] hT-layout -> dst [128, (r,d)] SP-layout."""
                for c in range(2):
                    for r in range(2):
                        pt = pst.tile([128, 128], F32, tag="pst")
                        nc.tensor.transpose(
                            pt, src[:, c * 256 + r * 128:c * 256 + (r + 1) * 128],
                            i_t)
                        nc.scalar.copy(
                            out=dst[:, r * 256 + c * 128:r * 256 + (c + 1) * 128],
                            in_=pt)

            def t_to_ht(src, dst):
                for r in range(2):
                    for c in range(2):
                        pt = pst.tile([128, 128], F32, tag="pst")
                        nc.tensor.transpose(
                            pt, src[:, r * 256 + c * 128:r * 256 + (c + 1) * 128],
                            i_t)
                        nc.scalar.copy(
                            out=dst[:, c * 256 + r * 128:c * 256 + (r + 1) * 128],
                            in_=pt)

            def tap(nm, t):
                if nm in dbg:
                    nc.gpsimd.dma_start(out=dbg[nm][:], in_=t)

            def load_state(dram_y, dram_q, row0, npart, C):
                sty_f = hb.tile([npart, C * 256], F32, tag="sty_f")
                sty_b = hb.tile([npart, C * 256], BF, tag="sty_b")
                stq = hb.tile([npart, C * 576], BF, tag="stq")
                sgq = wk.tile([npart, C * 576], F32, tag="stg_q")
                nc.gpsimd.dma_start(out=sty_f,
                                    in_=dram_y[row0:row0 + npart, :])
                nc.gpsimd.dma_start(out=sgq, in_=dram_q[row0:row0 + npart, :])
                nc.vector.tensor_copy(sty_b[:, :], sty_f[:, :])
                nc.vector.tensor_copy(stq[:, :], sgq[:, :])
                return sty_f, sty_b, stq

            h_sp = None
            for i in range(L):
                # ---- B1. token SRWM ----
                sty_f, sty_b, stq = load_state(tkSy, tkSq, i * 128, 128, 2)
                ys_tk = hb.tile([128, S * 2 * 16], F32, tag="ys")
                ht_cur = hT

                def x_tk(s, ht_cur=ht_cur):
                    return ht_cur[:, :].rearrange("p (c sp) -> p c sp", c=2)[
                        :, :, s * 16:(s + 1) * 16]

                _srwm_scan(nc, wk, 128, 2, sty_f, sty_b, stq, x_tk, ys_tk)

                hT2 = hb.tile([128, 512], F32, tag="hT2")
                ys_v = ys_tk[:, :].rearrange("p (s c j) -> p c s j", s=S, c=2)
                out_v = hT2[:, :].rearrange("p (c s j) -> p c s j", c=2, j=16)
                g_v = tkg_t[:, i * 16:(i + 1) * 16].unsqueeze(1).unsqueeze(1) \
                    .broadcast_to([128, 2, 16, 16])
                b_v = tkb_t[:, i * 16:(i + 1) * 16].unsqueeze(1).unsqueeze(1) \
                    .broadcast_to([128, 2, 16, 16])
                _ln(nc, wk, "a", 128, 2, 16, 16, out_v, ys_v, g_v, b_v)
                tap(f"d_hT2_{i}", hT2)

                # ---- B2. token mixer ----
                h2 = hb.tile([128, 512], F32, tag="h2")
                t_to_sp(hT2, h2)
                hn = hb.tile([128, 512], F32, tag="hn")
                hn_v = hn[:, :].rearrange("p (a r d) -> p a r d", a=1, r=2)
                h2_v = h2[:, :].rearrange("p (a r d) -> p a r d", a=1, r=2)
                g_v = tkmg_t[:, i * 256:(i + 1) * 256].unsqueeze(1).unsqueeze(1) \
                    .broadcast_to([128, 1, 2, 256])
                b_v = tkmb_t[:, i * 256:(i + 1) * 256].unsqueeze(1).unsqueeze(1) \
                    .broadcast_to([128, 1, 2, 256])
                _ln(nc, wk, "b", 128, 1, 2, 256, hn_v, h2_v, g_v, b_v)
                drn = dr.tile([S, P, D], F32, tag="drn")
                for r in range(2):
                    nc.gpsimd.dma_start(
                        out=drn[r * 8:(r + 1) * 8, :, :],
                        in_=hn[:, r * 256:(r + 1) * 256])
                pmj = hb.tile([16, S * D], F32, tag="pmj")
                nc.gpsimd.dma_start(
                    out=pmj[:, :].rearrange("p (s d) -> p s d", s=S),
                    in_=drn[:, :, :].rearrange("s p d -> p s d"))
                gl = hb.tile([64, S * D], F32, tag="gl")
                for k in range(8):
                    pt1 = psm.tile([64, 512], F32, tag="psm")
                    nc.tensor.matmul(pt1, tkmW1_t[:, i * 64:(i + 1) * 64],
                                     pmj[:, k * 512:(k + 1) * 512],
                                     start=True, stop=True)
                    _gelu(nc, wk, gl[:, k * 512:(k + 1) * 512], pt1[:, :],
                          tkmB1_t[:, i:i + 1], 64, 512)
                mxo = hb.tile([16, S * D], F32, tag="mxo")
                for k in range(8):
                    pt2 = psm.tile([16, 512], F32, tag="psm")
                    nc.tensor.matmul(pt2, tkmW2_t[:, i * 16:(i + 1) * 16],
                                     gl[:, k * 512:(k + 1) * 512],
                                     start=True, stop=True)
                    nc.scalar.activation(out=mxo[:, k * 512:(k + 1) * 512],
                                         in_=pt2, func=AF.Identity,
                                         bias=tkmB2_t[:, i:i + 1], scale=1.0)
                dr3 = dr.tile([S, P, D], F32, tag="dr3")
                nc.gpsimd.dma_start(
                    out=dr3[:, :, :].rearrange("s p d -> p s d"),
                    in_=mxo[:, :].rearrange("p (s d) -> p s d", s=S))
                mxsp = hb.tile([128, 512], F32, tag="mxsp")
                for r in range(2):
                    nc.gpsimd.dma_start(
                        out=mxsp[:, r * 256:(r + 1) * 256],
                        in_=dr3[r * 8:(r + 1) * 8, :, :])
                h3 = hb.tile([128, 512], F32, tag="h3")
                nc.vector.tensor_add(h3[:, :], h2[:, :], mxsp[:, :])
                tap(f"d_h3_{i}", h3)

                # ---- B3. channel SRWM ----
                dr1 = dr.tile([S, P, H, DH], F32, tag="dr1")
                for r in range(2):
                    nc.gpsimd.dma_start(
                        out=dr1[r * 8:(r + 1) * 8, :, :, :].rearrange(
                            "s p hh j -> s p (hh j)"),
                        in_=h3[:, r * 256:(r + 1) * 256])
                xc = hb.tile([128, 512], F32, tag="xc")
                for c in range(2):
                    nc.gpsimd.dma_start(
                        out=xc[:, :].rearrange("q (c s j) -> q c s j",
                                               c=2, s=S)[:, c],
                        in_=dr1[:, c * 8:(c + 1) * 8, :, :].rearrange(
                            "s ph hh j -> (ph hh) s j"))
                xcb = hb.tile([128, 512], BF, tag="xcb")
                nc.vector.tensor_copy(xcb[:, :], xc[:, :])
                sty_f, sty_b, stq = load_state(chSy, chSq, i * 128, 128, 2)
                ys_ch = hb.tile([128, S * 2 * 16], F32, tag="ys")

                def x_ch(s, xcb=xcb):
                    return xcb[:, :].rearrange("p (c s j) -> p c s j",
                                               c=2, s=S)[:, :, s, :]

                _srwm_scan(nc, wk, 128, 2, sty_f, sty_b, stq, x_ch, ys_ch)

                dr2 = dr.tile([S, P, H, DH], F32, tag="dr2")
                for c in range(2):
                    nc.gpsimd.dma_start(
                        out=dr2[:, c * 8:(c + 1) * 8, :, :].rearrange(
                            "s ph hh i -> (ph hh) s i"),
                        in_=ys_ch[:, :].rearrange(
                            "q (s c j) -> q s c j", s=S, c=2)[:, :, c, :])
                ysp = hb.tile([128, 512], F32, tag="ysp")
                for r in range(2):
                    nc.gpsimd.dma_start(
                        out=ysp[:, r * 256:(r + 1) * 256],
                        in_=dr2[r * 8:(r + 1) * 8, :, :, :].rearrange(
                            "s p hh i -> s p (hh i)"))
                h4 = hb.tile([128, 512], F32, tag="h4")
                h4_v = h4[:, :].rearrange("p (a r d) -> p a r d", a=1, r=2)
                ysp_v = ysp[:, :].rearrange("p (a r d) -> p a r d", a=1, r=2)
                g_v = chg_t[:, i * 256:(i + 1) * 256].unsqueeze(1).unsqueeze(1) \
                    .broadcast_to([128, 1, 2, 256])
                b_v = chb_t[:, i * 256:(i + 1) * 256].unsqueeze(1).unsqueeze(1) \
                    .broadcast_to([128, 1, 2, 256])
                _ln(nc, wk, "c", 128, 1, 2, 256, h4_v, ysp_v, g_v, b_v)
                tap(f"d_h4_{i}", h4)

                # ---- B4. channel mixer ----
                hn2 = hb.tile([128, 512], F32, tag="hn2")
                hn2_v = hn2[:, :].rearrange("p (a r d) -> p a r d", a=1, r=2)
                g_v = chmg_t[:, i * 256:(i + 1) * 256].unsqueeze(1).unsqueeze(1) \
                    .broadcast_to([128, 1, 2, 256])
                b_v = chmb_t[:, i * 256:(i + 1) * 256].unsqueeze(1).unsqueeze(1) \
                    .broadcast_to([128, 1, 2, 256])
                _ln(nc, wk, "d", 128, 1, 2, 256, hn2_v, h4_v, g_v, b_v)
                hn2T = hb.tile([128, 512], F32, tag="hn2T")
                t_to_ht(hn2, hn2T)
                pt1 = psm.tile([128, 256], F32, tag="psm")
                for c in range(2):
                    nc.tensor.matmul(
                        pt1, chmW1_t[:, i * 256 + c * 128:i * 256 + (c + 1) * 128],
                        hn2T[:, c * 256:(c + 1) * 256],
                        start=(c == 0), stop=(c == 1))
                gl2 = hb.tile([128, 256], F32, tag="gl2")
                _gelu(nc, wk, gl2[:, :], pt1[:, :], chmB1_t[:, i:i + 1],
                      128, 256)
                moT = hb.tile([128, 512], F32, tag="moT")
                for c in range(2):
                    pt2 = psm.tile([128, 256], F32, tag="psm")
                    nc.tensor.matmul(
                        pt2, chmW2_t[:, i * 256 + c * 128:i * 256 + (c + 1) * 128],
                        gl2, start=True, stop=True)
                    nc.scalar.activation(out=moT[:, c * 256:(c + 1) * 256],
                                         in_=pt2, func=AF.Identity,
                                         bias=chmB2_t[:, i * 2 + c:i * 2 + c + 1],
                                         scale=1.0)
                mosp = hb.tile([128, 512], F32, tag="mosp")
                t_to_sp(moT, mosp)
                h5 = hb.tile([128, 512], F32, tag="h5")
                nc.vector.tensor_add(h5[:, :], h4[:, :], mosp[:, :])
                h_sp = h5
                tap(f"d_h5_{i}", h5)

                if i + 1 < L:
                    hT = hb.tile([128, 512], BF, tag="hT0")
                    t_to_ht(h_sp, hT)

            # ---- C. final ----
            hf = hb.tile([128, 512], F32, tag="hf")
            hf_v = hf[:, :].rearrange("p (a r d) -> p a r d", a=1, r=2)
            hsp_v = h_sp[:, :].rearrange("p (a r d) -> p a r d", a=1, r=2)
            g_v = flng_t[:, :].unsqueeze(1).unsqueeze(1) \
                .broadcast_to([128, 1, 2, 256])
            b_v = flnb_t[:, :].unsqueeze(1).unsqueeze(1) \
                .broadcast_to([128, 1, 2, 256])
            _ln(nc, wk, "f", 128, 1, 2, 256, hf_v, hsp_v, g_v, b_v)
            pm = psm.tile([16, 256], F32, tag="psm")
            for r in range(2):
                nc.tensor.matmul(pm, mean_t[:, r * 16:(r + 1) * 16],
                                 hf[:, r * 256:(r + 1) * 256],
                                 start=(r == 0), stop=(r == 1))
            ho = hb.tile([16, 256], F32, tag="ho")
            nc.scalar.copy(out=ho, in_=pm)
            tap("d_ho", ho)
            dr4 = dr.tile([S, H, DH], F32, tag="dr4")
            nc.gpsimd.dma_start(
                out=dr4[:, :, :],
                in_=ho[:, :].rearrange("s (hh j) -> s hh j", hh=H))
            xo = hb.tile([16, S * DH], F32, tag="xo")
            nc.gpsimd.dma_start(
                out=xo[:, :].rearrange("p (s j) -> p s j", s=S),
                in_=dr4[:, :, :].rearrange("s hh j -> hh s j"))
            xob = hb.tile([16, S * DH], BF, tag="xob")
            nc.vector.tensor_copy(xob[:, :], xo[:, :])
            sty_f, sty_b, stq = load_state(oSy, oSq, 0, 16, 1)
            ys_o = hb.tile([16, S * 16], F32, tag="ys_o")

            def x_o(s, xob=xob):
                return xob[:, :].rearrange("p (c s j) -> p c s j", c=1, s=S)[
                    :, :, s, :]

            _srwm_scan(nc, wk, 16, 1, sty_f, sty_b, stq, x_o, ys_o,
                       use_gp=False)

            dr5 = dr.tile([H, S, DH], F32, tag="dr5")
            nc.gpsimd.dma_start(
                out=dr5[:, :, :],
                in_=ys_o[:, :].rearrange("p (s j) -> p s j", s=S))
            ho2 = hb.tile([16, 256], F32, tag="ho2")
            nc.gpsimd.dma_start(
                out=ho2[:, :].rearrange("s (hh i) -> s hh i", hh=H),
                in_=dr5[:, :, :].rearrange("hh s i -> s hh i"))
            hon = hb.tile([16, 256], F32, tag="hon")
            _ln(nc, wk, "o", 16, 1, 1, 256,
                hon[:, :].unsqueeze(1).unsqueeze(1),
                ho2[:, :].unsqueeze(1).unsqueeze(1),
                og_t[:, :].unsqueeze(1).unsqueeze(1),
                ob_t[:, :].unsqueeze(1).unsqueeze(1))
            hoT = hb.tile([128, 32], F32, tag="hoT")
            for c in range(2):
                pt = pst.tile([128, 128], F32, tag="pst")
                nc.tensor.transpose(pt[:, 0:16], hon[:, c * 128:(c + 1) * 128],
                                    i_t[0:16, 0:16])
                nc.scalar.copy(out=hoT[:, c * 16:(c + 1) * 16], in_=pt[:, 0:16])
            po = psm.tile([16, 5], F32, tag="psm")
            for c in range(2):
                nc.tensor.matmul(po, hoT[:, c * 16:(c + 1) * 16],
                                 outW_t[:, c * 5:(c + 1) * 5],
                                 start=(c == 0), stop=(c == 1))
            fin = hb.tile([16, 5], F32, tag="fin")
            nc.vector.tensor_add(fin[:, :], po[:, :], outB_t[:, :])
            nc.gpsimd.dma_start(out=out[:], in_=fin)

    nc.finalize()
    return nc


# ------------------------- host marshaling -------------------------

def _patchify(x):
    s, bb, c, hh, ww = x.shape
    h, w = hh // PS, ww // PS
    x = x.reshape(s, bb, c, h, PS, w, PS)
    x = x.transpose(0, 1, 3, 5, 4, 6, 2)
    return x.reshape(s, bb, h * w, PS * PS * c)


def _state_init(Wy, Wq, Wk, wb, pair_of, npart=128, C=2):
    sy = np.zeros((npart, C, 16, 16), np.float32)
    sq = np.zeros((npart, C, 36, 16), np.float32)
    for c in range(C):
        for q in range(npart):
            h = pair_of(c, q)
            sy[q, c] = Wy[h]
            sq[q, c, 0:16] = Wq[h]
            sq[q, c, 16:32] = Wk[h]
            sq[q, c, 32:36] = wb[h]
    return sy.reshape(npart, C * 256), sq.reshape(npart, C * 576)


def marshal(ins):
    """Returns (xin (S,B,P,QIN), shared input dict)."""
    x, fb = ins["x"].astype(np.float32), ins["fb"]
    xp = _patchify(x)
    emb = np.zeros((S, B, NCLS), np.float32)
    emb[np.arange(S)[:, None], np.arange(B)[None, :], fb] = 1.0
    emb = np.broadcast_to(emb[:, :, None, :], (S, B, P, NCLS))
    xin = np.concatenate([xp, emb], -1)

    f32 = lambda k: np.asarray(ins[k], np.float32)
    sh = {}
    sh["inW"] = np.ascontiguousarray(f32("in_W"))
    sh["inb"] = f32("in_b").reshape(2, 128).T.copy()
    sh["I128"] = np.eye(128, dtype=np.float32)
    mean = np.zeros((128, 32), np.float32)
    for r in range(2):
        for sp in range(128):
            s = r * 8 + sp // 16
            mean[sp, r * 16 + s] = 1.0 / 16.0
    sh["MEAN"] = mean
    tky, tkq = zip(*[
        _state_init(f32("tk_Wy")[i], f32("tk_Wq")[i], f32("tk_Wk")[i],
                    f32("tk_wb")[i], lambda c, q: 0) for i in range(L)])
    sh["tkSy"] = np.concatenate(tky, 0)
    sh["tkSq"] = np.concatenate(tkq, 0)
    chy, chq = zip(*[
        _state_init(f32("ch_Wy")[i], f32("ch_Wq")[i], f32("ch_Wk")[i],
                    f32("ch_wb")[i], lambda c, q: q % 16) for i in range(L)])
    sh["chSy"] = np.concatenate(chy, 0)
    sh["chSq"] = np.concatenate(chq, 0)
    oy, oq = _state_init(f32("o_Wy"), f32("o_Wq"), f32("o_Wk"), f32("o_wb"),
                         lambda c, q: q, npart=16, C=1)
    sh["oSy"] = oy
    sh["oSq"] = oq
    rep = lambda a, n=128: np.broadcast_to(
        np.asarray(a, np.float32).reshape(1, -1),
        (n, np.asarray(a).size)).copy()
    sh["tkg"] = rep(f32("tk_lng"))
    sh["tkb"] = rep(f32("tk_lnb"))
    sh["tkmg"] = rep(f32("tkm_g"))
    sh["tkmb"] = rep(f32("tkm_b"))
    sh["chg"] = rep(f32("ch_lng"))
    sh["chb"] = rep(f32("ch_lnb"))
    sh["chmg"] = rep(f32("chm_g"))
    sh["chmb"] = rep(f32("chm_b"))
    sh["flng"] = rep(f32("fln_g"))
    sh["flnb"] = rep(f32("fln_b"))
    sh["og"] = rep(f32("o_lng"), 16)
    sh["ob"] = rep(f32("o_lnb"), 16)
    sh["tkmW1"] = np.concatenate([f32("tkm_W1")[i] for i in range(L)], 1)
    sh["tkmB1"] = np.stack([f32("tkm_b1")[i] for i in range(L)], 1)
    sh["tkmW2"] = np.concatenate([f32("tkm_W2")[i] for i in range(L)], 1)
    sh["tkmB2"] = np.stack([f32("tkm_b2")[i] for i in range(L)], 1)
    # chmW1[i] is (D=256, DFT=128); lhsT chunk c = chm_W1[i][c*128:(c+1)*128, :]
    sh["chmW1"] = np.concatenate(
        [f32("chm_W1")[i][c * 128:(c + 1) * 128, :]
         for i in range(L) for c in range(2)], 1)
    sh["chmB1"] = np.stack([f32("chm_b1")[i] for i in range(L)], 1)
    # chmW2[i] is (DFT=128, D=256); lhsT chunk c = chm_W2[i][:, c*128:(c+1)*128]
    sh["chmW2"] = np.concatenate(
        [f32("chm_W2")[i][:, c * 128:(c + 1) * 128]
         for i in range(L) for c in range(2)], 1)
    # chmB2: bias per d; chunk c column holds b2[c*128:(c+1)*128]
    sh["chmB2"] = np.stack(
        [f32("chm_b2")[i][c * 128:(c + 1) * 128]
         for i in range(L) for c in range(2)], 1)
    sh["outW"] = np.concatenate(
        [f32("out_W")[c * 128:(c + 1) * 128, :] for c in range(2)], 1)
    sh["outB"] = rep(f32("out_b"), 16)
    return xin, sh


def in_maps_for(xin, sh):
    maps = []
    for b in range(B):
        m = dict(sh)
        m["xinT"] = np.ascontiguousarray(
            xin[:, b].reshape(S * P, QIN).T)
        maps.append(m)
    return maps


from concourse.bass_utils import run_bass_kernel_spmd

_CACHE = {}


def kernel(**inputs):
    ins = {k: np.ascontiguousarray(np.asarray(v)) for k, v in inputs.items()}
    if "nc" not in _CACHE:
        _CACHE["nc"] = build_nc()
    nc = _CACHE["nc"]
    xin, sh = marshal(ins)
    maps = in_maps_for(xin, sh)
    res = run_bass_kernel_spmd(nc, maps, core_ids=list(range(8)))
    out = np.stack([res.results[c]["out"] for c in range(B)], axis=1)
    return out.astype(np.float32)


# revision 10
# speedup vs baseline: 1.3061x; 1.3061x over previous
"""Trainium2 Bass kernel for nn_CompatStatefulSelfModMixerModel.

Fully on-device: input projection, 2x (token SRWM scan + token mixer +
channel SRWM scan + channel mixer), final LN + patch-mean, output SRWM,
linear head - one Bass program per core. Data-parallel over batch B=8
across 8 NeuronCores (1 sample/core, weights replicated, no collectives).

Scan fast-math: fast-weight state kept in bf16 (DVE 2x_1p mode for all
big tensor_tensor ops), state split into y-rows (gpsimd-updated) and
q/k/beta-rows (vector-updated), softmax without max-subtraction, single
activation table set (rsqrt via exp(-0.5*ln(v)), sigmoid via exp).
"""
import sys

sys.path.insert(0, "/opt/trn_rl_repo")

import numpy as np

import concourse.bacc as bacc
import concourse.tile as tile
from concourse import mybir

F32 = mybir.dt.float32
BF = mybir.dt.bfloat16
AF = mybir.ActivationFunctionType
ALU = mybir.AluOpType
AX = mybir.AxisListType

S, B, NCLS = 16, 8, 5
D, H, DH = 256, 16, 16
PS, IMG = 7, 28
P = 16
L = 2
PD = 49
QIN = PD + NCLS  # 54
DFT = 128
EPS = 1e-5


def _srwm_scan(nc, wk, npart, C, sty, stq, x_of_step, ys_all,
               use_gp=True):
    """S steps of the SRWM recurrence.

    sty: f32 [npart, C*256] - Wy rows (muls on gpsimd, add on vector)
    stq: bf16 [npart, C*576] - Wq/Wk/wb rows, viewed [p, c, g, 16]
         (0:16 Wq, 16:32 Wk, 32:36 wb)
    x_of_step(s) -> bf16 AP [npart, C, 16]
    ys_all: fp32 [npart, S*C*16]; y_t lands at [:, s, c, :].
    """
    styf_v = sty[:, :].rearrange("p (c i j) -> p c i j", c=C, j=16)
    sty_v = styf_v
    stq_v = stq[:, :].rearrange("p (c g j) -> p c g j", c=C, j=16)
    GE = nc.gpsimd if use_gp else nc.vector

    for s in range(S):
        xt = x_of_step(s)  # [p, C, 16] bf16
        # y path: y_t = Wy . x in f32 (mul on gpsimd, reduce on vector)
        zy0 = wk.tile([npart, C * 256], F32, tag="sc_zy0")
        zy0_v = zy0[:, :].rearrange("p (c i j) -> p c i j", c=C, j=16)
        GE.tensor_mul(zy0_v, styf_v,
                      xt.unsqueeze(2).broadcast_to([npart, C, 16, 16]))
        y_out = ys_all[:, :].rearrange("p (s c j) -> p s c j", s=S, c=C)[:, s]
        nc.vector.tensor_reduce(y_out, zy0_v, axis=AX.X, op=ALU.add)

        # seg = [q; k; b-logits] = stq . x
        zq = wk.tile([npart, C * 576], BF, tag="sc_zq")
        zq_v = zq[:, :].rearrange("p (c g j) -> p c g j", c=C, j=16)
        nc.vector.tensor_mul(zq_v, stq_v,
                             xt.unsqueeze(2).broadcast_to([npart, C, 36, 16]))
        seg = wk.tile([npart, C * 36], F32, tag="sc_seg")
        seg_v = seg[:, :].rearrange("p (c g) -> p c g", c=C)
        nc.vector.tensor_reduce(seg_v, zq_v, axis=AX.X, op=ALU.add)

        # merged softmax(q), softmax(k) - no max subtraction
        eqk = wk.tile([npart, C * 32], F32, tag="sc_eqk")
        eqk_v = eqk[:, :].rearrange("p (c t j) -> p c t j", c=C, t=2)
        nc.scalar.activation(
            out=eqk_v,
            in_=seg_v[:, :, 0:32].rearrange("p c (t j) -> p c t j", t=2),
            func=AF.Exp)
        sums = wk.tile([npart, C * 2], F32, tag="sc_sums")
        sums_v = sums[:, :].rearrange("p (c t) -> p c t", c=C)
        nc.vector.tensor_reduce(sums_v, eqk_v, axis=AX.X, op=ALU.add)
        rec = wk.tile([npart, C * 2], F32, tag="sc_rec")
        nc.vector.reciprocal_approx_fast(rec[:, :], sums[:, :])
        rec_v = rec[:, :].rearrange("p (c t) -> p c t", c=C)
        kq = wk.tile([npart, C * 32], BF, tag="sc_kq")
        kq_v = kq[:, :].rearrange("p (c t j) -> p c t j", c=C, t=2)
        nc.vector.tensor_mul(kq_v, eqk_v,
                             rec_v.unsqueeze(3).broadcast_to([npart, C, 2, 16]))
        qs = kq_v[:, :, 0]
        ks = kq_v[:, :, 1]
        e = wk.tile([npart, C * 16], BF, tag="sc_e")
        e_v = e[:, :].rearrange("p (c j) -> p c j", c=C)
        nc.vector.tensor_sub(e_v, qs, ks)

        # beta = sigmoid(b-logits) via exp
        bta = wk.tile([npart, C * 4], F32, tag="sc_beta")
        bta_v = bta[:, :].rearrange("p (c w) -> p c w", c=C)
        nc.scalar.activation(out=bta_v, in_=seg_v[:, :, 32:36], func=AF.Exp,
                             scale=-1.0)
        nc.vector.tensor_scalar(bta[:, :], bta[:, :], 1.0, None, ALU.add)
        nc.vector.reciprocal_approx_fast(bta[:, :], bta[:, :])

        # d rows 16:52 = stq . (qs - ks)
        z2 = wk.tile([npart, C * 576], BF, tag="sc_z2")
        z2_v = z2[:, :].rearrange("p (c g j) -> p c g j", c=C, j=16)
        nc.vector.tensor_mul(z2_v, stq_v,
                             e_v.unsqueeze(2).broadcast_to([npart, C, 36, 16]))
        d = wk.tile([npart, C * 52], F32, tag="sc_d")
        d_v = d[:, :].rearrange("p (c g) -> p c g", c=C)
        nc.vector.tensor_reduce(d_v[:, :, 16:52], z2_v, axis=AX.X, op=ALU.add)

        # vy over both qs and ks: vykq[c,t,i] = sum_j Wy[c,i,j]*kq[c,t,j]
        zy = wk.tile([npart, C * 512], BF, tag="sc_zy")
        zy_v = zy[:, :].rearrange("p (c t i j) -> p c t i j", c=C, t=2, j=16)
        for t in range(2):
            nc.vector.tensor_mul(
                zy_v[:, :, t], sty_v,
                kq_v[:, :, t].unsqueeze(2).broadcast_to([npart, C, 16, 16]))
        vykq = wk.tile([npart, C * 32], F32, tag="sc_vykq")
        vykq_v = vykq[:, :].rearrange("p (c t i) -> p c t i", c=C, t=2)
        nc.vector.tensor_reduce(
            vykq[:, :],
            zy[:, :].rearrange("p (a j) -> p a j", j=16),
            axis=AX.X, op=ALU.add)

        # v-softmax on vy_q; d rows 0:16 = softmax(vy_q) - vy_k
        ev = wk.tile([npart, C * 16], F32, tag="sc_ev")
        ev_v = ev[:, :].rearrange("p (c i) -> p c i", c=C)
        nc.scalar.activation(out=ev_v, in_=vykq_v[:, :, 0], func=AF.Exp)
        vs = wk.tile([npart, C], F32, tag="sc_vs")
        nc.vector.tensor_reduce(vs[:, :], ev_v, axis=AX.X, op=ALU.add)
        nc.vector.reciprocal_approx_fast(vs[:, :], vs[:, :])
        for c in range(C):
            nc.vector.scalar_tensor_tensor(
                out=d_v[:, c, 0:16], in0=ev_v[:, c], scalar=vs[:, c:c + 1],
                in1=vykq_v[:, c, 1], op0=ALU.mult, op1=ALU.subtract)

        # expand beta to per-row b52, then d2x = d * b52 as paired bf16
        b52 = wk.tile([npart, C * 52], F32, tag="sc_b52")
        b52_v = b52[:, :].rearrange("p (c g) -> p c g", c=C)
        nc.vector.tensor_scalar(
            b52_v[:, :, 0:48].rearrange("p c (w g) -> p c w g", g=16),
            bta_v[:, :, 0:3].unsqueeze(3).broadcast_to([npart, C, 3, 16]),
            1.0, None, ALU.mult)
        nc.vector.tensor_scalar(
            b52_v[:, :, 48:52],
            bta_v[:, :, 3:4].broadcast_to([npart, C, 4]),
            1.0, None, ALU.mult)
        d2x = wk.tile([npart, C * 104], BF, tag="sc_d2x")
        d2x_v = d2x[:, :].rearrange("p (c g t) -> p c g t", c=C, t=2)
        nc.vector.tensor_mul(
            d2x_v,
            d_v.unsqueeze(3).broadcast_to([npart, C, 52, 2]),
            b52_v.unsqueeze(3).broadcast_to([npart, C, 52, 2]))

        # state update: W += d (x) ks  (paired views keep 2x mode)
        kspq = ks.rearrange("p c (j2 t) -> p c j2 t", t=2)  # [p, C, 8, 2]
        zu = wk.tile([npart, C * 576], BF, tag="sc_zu")
        zu_p = zu[:, :].rearrange("p (c g j2 t) -> p c g j2 t", c=C, j2=8, t=2)
        for c in range(C):
            nc.vector.tensor_mul(
                zu_p[:, c],
                d2x_v[:, c, 16:52].unsqueeze(2)
                .broadcast_to([npart, 36, 8, 2]),
                kspq[:, c].unsqueeze(1).broadcast_to([npart, 36, 8, 2]))
        nc.vector.tensor_add(stq[:, :], stq[:, :], zu[:, :])
        zuy = wk.tile([npart, C * 256], F32, tag="sc_zuy")
        zuy_p = zuy[:, :].rearrange("p (c g j2 t) -> p c g j2 t",
                                    c=C, j2=8, t=2)
        for c in range(C):
            GE.tensor_mul(
                zuy_p[:, c],
                d2x_v[:, c, 0:16].unsqueeze(2)
                .broadcast_to([npart, 16, 8, 2]),
                kspq[:, c].unsqueeze(1).broadcast_to([npart, 16, 8, 2]))
        nc.vector.tensor_add(sty[:, :], sty[:, :], zuy[:, :])


def _ln(nc, wk, tag, npart, A1, A2, J, out_v, in_v, g_v, b_v):
    """LayerNorm over innermost J of 4-dim views [npart, A1, A2, J].

    rstd computed as exp(-0.5*ln(v+eps)) to stay in one act table set.
    """
    A = A1 * A2
    m = wk.tile([npart, A], F32, tag=f"ln_m_{tag}")
    v = wk.tile([npart, A], F32, tag=f"ln_v_{tag}")
    sq = wk.tile([npart, A * J], F32, tag="ln_sq")
    m4 = m[:, :].rearrange("p (a b) -> p a b", a=A1)
    v4 = v[:, :].rearrange("p (a b) -> p a b", a=A1)
    nc.vector.tensor_reduce(m4, in_v, axis=AX.X, op=ALU.add)
    nc.vector.tensor_scalar(m[:, :], m[:, :], -1.0 / J, None, ALU.mult)
    m_b = m4.unsqueeze(3).broadcast_to([npart, A1, A2, J])
    nc.vector.tensor_add(out_v, in_v, m_b)
    sq_v = sq[:, :].rearrange("p (a b j) -> p a b j", a=A1, b=A2)
    nc.vector.tensor_mul(sq_v, out_v, out_v)
    nc.vector.tensor_reduce(v4, sq_v, axis=AX.X, op=ALU.add)
    nc.vector.tensor_scalar(v[:, :], v[:, :], 1.0 / J, EPS, ALU.mult, ALU.add)
    nc.scalar.activation(out=v[:, :], in_=v[:, :], func=AF.Ln)
    nc.scalar.activation(out=v[:, :], in_=v[:, :], func=AF.Exp, scale=-0.5)
    v_b = v4.unsqueeze(3).broadcast_to([npart, A1, A2, J])
    nc.vector.tensor_mul(out_v, out_v, v_b)
    nc.vector.tensor_mul(out_v, out_v, g_v)
    nc.vector.tensor_add(out_v, out_v, b_v)


GELU_C = 1.5957691216057308  # 2*sqrt(2/pi)
GELU_A = 0.044715


def _gelu(nc, wk, out_v, pt_v, bias_ap, npart, F):
    """out = gelu_tanh(pt + bias); pt may be PSUM. [npart, F] views.

    sigmoid computed via exp to stay in one act table set.
    """
    xb = wk.tile([npart, F], F32, tag="gelu_xb")
    x2 = wk.tile([npart, F], F32, tag="gelu_x2")
    nc.scalar.activation(out=xb[:, :], in_=pt_v, func=AF.Identity,
                         bias=bias_ap, scale=1.0)
    nc.scalar.activation(out=x2[:, :], in_=pt_v, func=AF.Square,
                         bias=bias_ap, scale=1.0)
    nc.vector.tensor_scalar(x2[:, :], x2[:, :], GELU_A, 1.0, ALU.mult, ALU.add)
    nc.vector.tensor_mul(x2[:, :], x2[:, :], xb[:, :])
    nc.scalar.activation(out=x2[:, :], in_=x2[:, :], func=AF.Exp,
                         scale=-GELU_C)
    nc.vector.tensor_scalar(x2[:, :], x2[:, :], 1.0, None, ALU.add)
    nc.vector.reciprocal_approx_fast(x2[:, :], x2[:, :])
    nc.vector.tensor_mul(out_v, xb[:, :], x2[:, :])


def build_nc(debug=()):
    nc = bacc.Bacc(None, target_bir_lowering=False)
    dp = lambda nm, shp: nc.declare_dram_parameter(nm, shp, F32, isOutput=False)

    xinT = dp("xinT", [QIN, S * P])
    inW = dp("inW", [QIN, D])
    inb = dp("inb", [128, 2])
    I128 = dp("I128", [128, 128])
    MEAN = dp("MEAN", [128, 32])
    tkSy = dp("tkSy", [L * 128, 512])
    tkSq = dp("tkSq", [L * 128, 1152])
    chSy = dp("chSy", [L * 128, 512])
    chSq = dp("chSq", [L * 128, 1152])
    oSy = dp("oSy", [16, 256])
    oSq = dp("oSq", [16, 576])
    tkg = dp("tkg", [128, L * 16])
    tkb = dp("tkb", [128, L * 16])
    tkmg = dp("tkmg", [128, L * 256])
    tkmb = dp("tkmb", [128, L * 256])
    chg = dp("chg", [128, L * 256])
    chb = dp("chb", [128, L * 256])
    chmg = dp("chmg", [128, L * 256])
    chmb = dp("chmb", [128, L * 256])
    flng = dp("flng", [128, 256])
    flnb = dp("flnb", [128, 256])
    og = dp("og", [16, 256])
    ob = dp("ob", [16, 256])
    tkmW1 = dp("tkmW1", [16, L * 64])
    tkmB1 = dp("tkmB1", [64, L])
    tkmW2 = dp("tkmW2", [64, L * 16])
    tkmB2 = dp("tkmB2", [16, L])
    chmW1 = dp("chmW1", [128, L * 256])
    chmB1 = dp("chmB1", [128, L])
    chmW2 = dp("chmW2", [128, L * 256])
    chmB2 = dp("chmB2", [128, L * 2])
    outW = dp("outW", [128, 10])
    outB = dp("outB", [16, 5])
    out = nc.declare_dram_parameter("out", [S, NCLS], F32, isOutput=True)
    dbg = {}
    for nm, shp in debug:
        dbg[nm] = nc.declare_dram_parameter(nm, shp, F32, isOutput=True)

    with tile.TileContext(nc) as tc:
        with tc.tile_pool(name="cst", bufs=1) as cst, \
             tc.tile_pool(name="hb", bufs=1) as hb, \
             tc.tile_pool(name="wk", bufs=2) as wk, \
             tc.tile_pool(name="pst", bufs=4, space="PSUM") as pst, \
             tc.tile_pool(name="psm", bufs=2, space="PSUM") as psm, \
             tc.tile_pool(name="dr", bufs=1, space="DRAM") as dr:

            def load(tensor, shape, tag, sl=None):
                t = cst.tile(shape, F32, tag=tag)
                nc.gpsimd.dma_start(out=t, in_=tensor[:] if sl is None else sl)
                return t

            i_t = load(I128, [128, 128], "I128")
            mean_t = load(MEAN, [128, 32], "MEAN")
            tkg_t = load(tkg, [128, L * 16], "tkg")
            tkb_t = load(tkb, [128, L * 16], "tkb")
            tkmg_t = load(tkmg, [128, L * 256], "tkmg")
            tkmb_t = load(tkmb, [128, L * 256], "tkmb")
            chg_t = load(chg, [128, L * 256], "chg")
            chb_t = load(chb, [128, L * 256], "chb")
            chmg_t = load(chmg, [128, L * 256], "chmg")
            chmb_t = load(chmb, [128, L * 256], "chmb")
            flng_t = load(flng, [128, 256], "flng")
            flnb_t = load(flnb, [128, 256], "flnb")
            og_t = load(og, [16, 256], "og")
            ob_t = load(ob, [16, 256], "ob")
            tkmW1_t = load(tkmW1, [16, L * 64], "tkmW1")
            tkmB1_t = load(tkmB1, [64, L], "tkmB1")
            tkmW2_t = load(tkmW2, [64, L * 16], "tkmW2")
            tkmB2_t = load(tkmB2, [16, L], "tkmB2")
            chmW1_t = load(chmW1, [128, L * 256], "chmW1")
            chmB1_t = load(chmB1, [128, L], "chmB1")
            chmW2_t = load(chmW2, [128, L * 256], "chmW2")
            chmB2_t = load(chmB2, [128, L * 2], "chmB2")
            outW_t = load(outW, [128, 10], "outW")
            outB_t = load(outB, [16, 5], "outB")

            # ---- A. input projection -> hT (bf16) [128, (c, s*16+p)] ----
            xin_t = cst.tile([QIN, S * P], F32, tag="xin")
            w_t = cst.tile([QIN, D], F32, tag="inW")
            b_t = cst.tile([128, 2], F32, tag="inb")
            nc.gpsimd.dma_start(out=xin_t, in_=xinT[:])
            nc.gpsimd.dma_start(out=w_t, in_=inW[:])
            nc.gpsimd.dma_start(out=b_t, in_=inb[:])
            hT = hb.tile([128, 512], BF, tag="hT0")
            for c in range(2):
                pt = psm.tile([128, S * P], F32, tag="psm")
                nc.tensor.matmul(pt, w_t[:, c * 128:(c + 1) * 128], xin_t,
                                 start=True, stop=True)
                nc.scalar.activation(out=hT[:, c * 256:(c + 1) * 256], in_=pt,
                                     func=AF.Identity, bias=b_t[:, c:c + 1],
                                     scale=1.0)

            def t_to_sp(src, dst):
                """src [128, (c,sp)] hT-layout -> dst [128, (r,d)] SP-layout."""
                for c in range(2):
                    for r in range(2):
                        pt = pst.tile([128, 128], F32, tag="pst")
                        nc.tensor.transpose(
                            pt, src[:, c * 256 + r * 128:c * 256 + (r + 1) * 128],
                            i_t)
                        nc.scalar.copy(
                            out=dst[:, r * 256 + c * 128:r * 256 + (c + 1) * 128],
                            in_=pt)

            def t_to_ht(src, dst):
                for r in range(2):
                    for c in range(2):
                        pt = pst.tile([128, 128], F32, tag="pst")
                        nc.tensor.transpose(
                            pt, src[:, r * 256 + c * 128:r * 256 + (c + 1) * 128],
                            i_t)
                        nc.scalar.copy(
                            out=dst[:, c * 256 + r * 128:c * 256 + (r + 1) * 128],
                            in_=pt)

            def tap(nm, t):
                if nm in dbg:
                    nc.gpsimd.dma_start(out=dbg[nm][:], in_=t)

            def load_state(dram_y, dram_q, row0, npart, C):
                sty = hb.tile([npart, C * 256], F32, tag="sty_f")
                stq = hb.tile([npart, C * 576], BF, tag="stq")
                sgq = wk.tile([npart, C * 576], F32, tag="stg_q")
                nc.gpsimd.dma_start(out=sty,
                                    in_=dram_y[row0:row0 + npart, :])
                nc.gpsimd.dma_start(out=sgq, in_=dram_q[row0:row0 + npart, :])
                nc.vector.tensor_copy(stq[:, :], sgq[:, :])
                return sty, stq

            h_sp = None
            for i in range(L):
                # ---- B1. token SRWM ----
                sty, stq = load_state(tkSy, tkSq, i * 128, 128, 2)
                ys_tk = hb.tile([128, S * 2 * 16], F32, tag="ys")
                ht_cur = hT

                def x_tk(s, ht_cur=ht_cur):
                    return ht_cur[:, :].rearrange("p (c sp) -> p c sp", c=2)[
                        :, :, s * 16:(s + 1) * 16]

                _srwm_scan(nc, wk, 128, 2, sty, stq, x_tk, ys_tk)

                hT2 = hb.tile([128, 512], F32, tag="hT2")
                ys_v = ys_tk[:, :].rearrange("p (s c j) -> p c s j", s=S, c=2)
                out_v = hT2[:, :].rearrange("p (c s j) -> p c s j", c=2, j=16)
                g_v = tkg_t[:, i * 16:(i + 1) * 16].unsqueeze(1).unsqueeze(1) \
                    .broadcast_to([128, 2, 16, 16])
                b_v = tkb_t[:, i * 16:(i + 1) * 16].unsqueeze(1).unsqueeze(1) \
                    .broadcast_to([128, 2, 16, 16])
                _ln(nc, wk, "a", 128, 2, 16, 16, out_v, ys_v, g_v, b_v)
                tap(f"d_hT2_{i}", hT2)

                # ---- B2. token mixer ----
                h2 = hb.tile([128, 512], F32, tag="h2")
                t_to_sp(hT2, h2)
                hn = hb.tile([128, 512], F32, tag="hn")
                hn_v = hn[:, :].rearrange("p (a r d) -> p a r d", a=1, r=2)
                h2_v = h2[:, :].rearrange("p (a r d) -> p a r d", a=1, r=2)
                g_v = tkmg_t[:, i * 256:(i + 1) * 256].unsqueeze(1).unsqueeze(1) \
                    .broadcast_to([128, 1, 2, 256])
                b_v = tkmb_t[:, i * 256:(i + 1) * 256].unsqueeze(1).unsqueeze(1) \
                    .broadcast_to([128, 1, 2, 256])
                _ln(nc, wk, "b", 128, 1, 2, 256, hn_v, h2_v, g_v, b_v)
                drn = dr.tile([S, P, D], F32, tag="drn")
                for r in range(2):
                    nc.gpsimd.dma_start(
                        out=drn[r * 8:(r + 1) * 8, :, :],
                        in_=hn[:, r * 256:(r + 1) * 256])
                pmj = hb.tile([16, S * D], F32, tag="pmj")
                nc.gpsimd.dma_start(
                    out=pmj[:, :].rearrange("p (s d) -> p s d", s=S),
                    in_=drn[:, :, :].rearrange("s p d -> p s d"))
                gl = hb.tile([64, S * D], F32, tag="gl")
                for k in range(8):
                    pt1 = psm.tile([64, 512], F32, tag="psm")
                    nc.tensor.matmul(pt1, tkmW1_t[:, i * 64:(i + 1) * 64],
                                     pmj[:, k * 512:(k + 1) * 512],
                                     start=True, stop=True)
                    _gelu(nc, wk, gl[:, k * 512:(k + 1) * 512], pt1[:, :],
                          tkmB1_t[:, i:i + 1], 64, 512)
                mxo = hb.tile([16, S * D], F32, tag="mxo")
                for k in range(8):
                    pt2 = psm.tile([16, 512], F32, tag="psm")
                    nc.tensor.matmul(pt2, tkmW2_t[:, i * 16:(i + 1) * 16],
                                     gl[:, k * 512:(k + 1) * 512],
                                     start=True, stop=True)
                    nc.scalar.activation(out=mxo[:, k * 512:(k + 1) * 512],
                                         in_=pt2, func=AF.Identity,
                                         bias=tkmB2_t[:, i:i + 1], scale=1.0)
                dr3 = dr.tile([S, P, D], F32, tag="dr3")
                nc.gpsimd.dma_start(
                    out=dr3[:, :, :].rearrange("s p d -> p s d"),
                    in_=mxo[:, :].rearrange("p (s d) -> p s d", s=S))
                mxsp = hb.tile([128, 512], F32, tag="mxsp")
                for r in range(2):
                    nc.gpsimd.dma_start(
                        out=mxsp[:, r * 256:(r + 1) * 256],
                        in_=dr3[r * 8:(r + 1) * 8, :, :])
                h3 = hb.tile([128, 512], F32, tag="h3")
                nc.vector.tensor_add(h3[:, :], h2[:, :], mxsp[:, :])
                tap(f"d_h3_{i}", h3)

                # ---- B3. channel SRWM ----
                dr1 = dr.tile([S, P, H, DH], F32, tag="dr1")
                for r in range(2):
                    nc.gpsimd.dma_start(
                        out=dr1[r * 8:(r + 1) * 8, :, :, :].rearrange(
                            "s p hh j -> s p (hh j)"),
                        in_=h3[:, r * 256:(r + 1) * 256])
                xc = hb.tile([128, 512], F32, tag="xc")
                for c in range(2):
                    nc.gpsimd.dma_start(
                        out=xc[:, :].rearrange("q (c s j) -> q c s j",
                                               c=2, s=S)[:, c],
                        in_=dr1[:, c * 8:(c + 1) * 8, :, :].rearrange(
                            "s ph hh j -> (ph hh) s j"))
                xcb = hb.tile([128, 512], BF, tag="xcb")
                nc.vector.tensor_copy(xcb[:, :], xc[:, :])
                sty, stq = load_state(chSy, chSq, i * 128, 128, 2)
                ys_ch = hb.tile([128, S * 2 * 16], F32, tag="ys")

                def x_ch(s, xcb=xcb):
                    return xcb[:, :].rearrange("p (c s j) -> p c s j",
                                               c=2, s=S)[:, :, s, :]

                _srwm_scan(nc, wk, 128, 2, sty, stq, x_ch, ys_ch)

                dr2 = dr.tile([S, P, H, DH], F32, tag="dr2")
                for c in range(2):
                    nc.gpsimd.dma_start(
                        out=dr2[:, c * 8:(c + 1) * 8, :, :].rearrange(
                            "s ph hh i -> (ph hh) s i"),
                        in_=ys_ch[:, :].rearrange(
                            "q (s c j) -> q s c j", s=S, c=2)[:, :, c, :])
                ysp = hb.tile([128, 512], F32, tag="ysp")
                for r in range(2):
                    nc.gpsimd.dma_start(
                        out=ysp[:, r * 256:(r + 1) * 256],
                        in_=dr2[r * 8:(r + 1) * 8, :, :, :].rearrange(
                            "s p hh i -> s p (hh i)"))
                h4 = hb.tile([128, 512], F32, tag="h4")
                h4_v = h4[:, :].rearrange("p (a r d) -> p a r d", a=1, r=2)
                ysp_v = ysp[:, :].rearrange("p (a r d) -> p a r d", a=1, r=2)
                g_v = chg_t[:, i * 256:(i + 1) * 256].unsqueeze(1).unsqueeze(1) \
                    .broadcast_to([128, 1, 2, 256])
                b_v = chb_t[:, i * 256:(i + 1) * 256].unsqueeze(1).unsqueeze(1) \
                    .broadcast_to([128, 1, 2, 256])
                _ln(nc, wk, "c", 128, 1, 2, 256, h4_v, ysp_v, g_v, b_v)
                tap(f"d_h4_{i}", h4)

                # ---- B4. channel mixer ----
                hn2 = hb.tile([128, 512], F32, tag="hn2")
                hn2_v = hn2[:, :].rearrange("p (a r d) -> p a r d", a=1, r=2)
                g_v = chmg_t[:, i * 256:(i + 1) * 256].unsqueeze(1).unsqueeze(1) \
                    .broadcast_to([128, 1, 2, 256])
                b_v = chmb_t[:, i * 256:(i + 1) * 256].unsqueeze(1).unsqueeze(1) \
                    .broadcast_to([128, 1, 2, 256])
                _ln(nc, wk, "d", 128, 1, 2, 256, hn2_v, h4_v, g_v, b_v)
                hn2T = hb.tile([128, 512], F32, tag="hn2T")
                t_to_ht(hn2, hn2T)
                pt1 = psm.tile([128, 256], F32, tag="psm")
                for c in range(2):
                    nc.tensor.matmul(
                        pt1, chmW1_t[:, i * 256 + c * 128:i * 256 + (c + 1) * 128],
                        hn2T[:, c * 256:(c + 1) * 256],
                        start=(c == 0), stop=(c == 1))
                gl2 = hb.tile([128, 256], F32, tag="gl2")
                _gelu(nc, wk, gl2[:, :], pt1[:, :], chmB1_t[:, i:i + 1],
                      128, 256)
                moT = hb.tile([128, 512], F32, tag="moT")
                for c in range(2):
                    pt2 = psm.tile([128, 256], F32, tag="psm")
                    nc.tensor.matmul(
                        pt2, chmW2_t[:, i * 256 + c * 128:i * 256 + (c + 1) * 128],
                        gl2, start=True, stop=True)
                    nc.scalar.activation(out=moT[:, c * 256:(c + 1) * 256],
                                         in_=pt2, func=AF.Identity,
                                         bias=chmB2_t[:, i * 2 + c:i * 2 + c + 1],
                                         scale=1.0)
                mosp = hb.tile([128, 512], F32, tag="mosp")
                t_to_sp(moT, mosp)
                h5 = hb.tile([128, 512], F32, tag="h5")
                nc.vector.tensor_add(h5[:, :], h4[:, :], mosp[:, :])
                h_sp = h5
                tap(f"d_h5_{i}", h5)

                if i + 1 < L:
                    hT = hb.tile([128, 512], BF, tag="hT0")
                    t_to_ht(h_sp, hT)

            # ---- C. final ----
            hf = hb.tile([128, 512], F32, tag="hf")
            hf_v = hf[:, :].rearrange("p (a r d) -> p a r d", a=1, r=2)
            hsp_v = h_sp[:, :].rearrange("p (a r d) -> p a r d", a=1, r=2)
            g_v = flng_t[:, :].unsqueeze(1).unsqueeze(1) \
                .broadcast_to([128, 1, 2, 256])
            b_v = flnb_t[:, :].unsqueeze(1).unsqueeze(1) \
                .broadcast_to([128, 1, 2, 256])
            _ln(nc, wk, "f", 128, 1, 2, 256, hf_v, hsp_v, g_v, b_v)
            pm = psm.tile([16, 256], F32, tag="psm")
            for r in range(2):
                nc.tensor.matmul(pm, mean_t[:, r * 16:(r + 1) * 16],
                                 hf[:, r * 256:(r + 1) * 256],
                                 start=(r == 0), stop=(r == 1))
            ho = hb.tile([16, 256], F32, tag="ho")
            nc.scalar.copy(out=ho, in_=pm)
            tap("d_ho", ho)
            dr4 = dr.tile([S, H, DH], F32, tag="dr4")
            nc.gpsimd.dma_start(
                out=dr4[:, :, :],
                in_=ho[:, :].rearrange("s (hh j) -> s hh j", hh=H))
            xo = hb.tile([16, S * DH], F32, tag="xo")
            nc.gpsimd.dma_start(
                out=xo[:, :].rearrange("p (s j) -> p s j", s=S),
                in_=dr4[:, :, :].rearrange("s hh j -> hh s j"))
            xob = hb.tile([16, S * DH], BF, tag="xob")
            nc.vector.tensor_copy(xob[:, :], xo[:, :])
            sty, stq = load_state(oSy, oSq, 0, 16, 1)
            ys_o = hb.tile([16, S * 16], F32, tag="ys_o")

            def x_o(s, xob=xob):
                return xob[:, :].rearrange("p (c s j) -> p c s j", c=1, s=S)[
                    :, :, s, :]

            _srwm_scan(nc, wk, 16, 1, sty, stq, x_o, ys_o, use_gp=False)

            dr5 = dr.tile([H, S, DH], F32, tag="dr5")
            nc.gpsimd.dma_start(
                out=dr5[:, :, :],
                in_=ys_o[:, :].rearrange("p (s j) -> p s j", s=S))
            ho2 = hb.tile([16, 256], F32, tag="ho2")
            nc.gpsimd.dma_start(
                out=ho2[:, :].rearrange("s (hh i) -> s hh i", hh=H),
                in_=dr5[:, :, :].rearrange("hh s i -> s hh i"))
            hon = hb.tile([16, 256], F32, tag="hon")
            _ln(nc, wk, "o", 16, 1, 1, 256,
                hon[:, :].unsqueeze(1).unsqueeze(1),
                ho2[:, :].unsqueeze(1).unsqueeze(1),
                og_t[:, :].unsqueeze(1).unsqueeze(1),
                ob_t[:, :].unsqueeze(1).unsqueeze(1))
            hoT = hb.tile([128, 32], F32, tag="hoT")
            for c in range(2):
                pt = pst.tile([128, 128], F32, tag="pst")
                nc.tensor.transpose(pt[:, 0:16], hon[:, c * 128:(c + 1) * 128],
                                    i_t[0:16, 0:16])
                nc.scalar.copy(out=hoT[:, c * 16:(c + 1) * 16], in_=pt[:, 0:16])
            po = psm.tile([16, 5], F32, tag="psm")
            for c in range(2):
                nc.tensor.matmul(po, hoT[:, c * 16:(c + 1) * 16],
                                 outW_t[:, c * 5:(c + 1) * 5],
                                 start=(c == 0), stop=(c == 1))
            fin = hb.tile([16, 5], F32, tag="fin")
            nc.vector.tensor_add(fin[:, :], po[:, :], outB_t[:, :])
            nc.gpsimd.dma_start(out=out[:], in_=fin)

    nc.finalize()
    return nc


# ------------------------- host marshaling -------------------------

def _patchify(x):
    s, bb, c, hh, ww = x.shape
    h, w = hh // PS, ww // PS
    x = x.reshape(s, bb, c, h, PS, w, PS)
    x = x.transpose(0, 1, 3, 5, 4, 6, 2)
    return x.reshape(s, bb, h * w, PS * PS * c)


def _state_init(Wy, Wq, Wk, wb, pair_of, npart=128, C=2):
    sy = np.zeros((npart, C, 16, 16), np.float32)
    sq = np.zeros((npart, C, 36, 16), np.float32)
    for c in range(C):
        for q in range(npart):
            h = pair_of(c, q)
            sy[q, c] = Wy[h]
            sq[q, c, 0:16] = Wq[h]
            sq[q, c, 16:32] = Wk[h]
            sq[q, c, 32:36] = wb[h]
    return sy.reshape(npart, C * 256), sq.reshape(npart, C * 576)


def marshal(ins):
    """Returns (xin (S,B,P,QIN), shared input dict)."""
    x, fb = ins["x"].astype(np.float32), ins["fb"]
    xp = _patchify(x)
    emb = np.zeros((S, B, NCLS), np.float32)
    emb[np.arange(S)[:, None], np.arange(B)[None, :], fb] = 1.0
    emb = np.broadcast_to(emb[:, :, None, :], (S, B, P, NCLS))
    xin = np.concatenate([xp, emb], -1)

    f32 = lambda k: np.asarray(ins[k], np.float32)
    sh = {}
    sh["inW"] = np.ascontiguousarray(f32("in_W"))
    sh["inb"] = f32("in_b").reshape(2, 128).T.copy()
    sh["I128"] = np.eye(128, dtype=np.float32)
    mean = np.zeros((128, 32), np.float32)
    for r in range(2):
        for sp in range(128):
            s = r * 8 + sp // 16
            mean[sp, r * 16 + s] = 1.0 / 16.0
    sh["MEAN"] = mean
    tky, tkq = zip(*[
        _state_init(f32("tk_Wy")[i], f32("tk_Wq")[i], f32("tk_Wk")[i],
                    f32("tk_wb")[i], lambda c, q: 0) for i in range(L)])
    sh["tkSy"] = np.concatenate(tky, 0)
    sh["tkSq"] = np.concatenate(tkq, 0)
    chy, chq = zip(*[
        _state_init(f32("ch_Wy")[i], f32("ch_Wq")[i], f32("ch_Wk")[i],
                    f32("ch_wb")[i], lambda c, q: q % 16) for i in range(L)])
    sh["chSy"] = np.concatenate(chy, 0)
    sh["chSq"] = np.concatenate(chq, 0)
    oy, oq = _state_init(f32("o_Wy"), f32("o_Wq"), f32("o_Wk"), f32("o_wb"),
                         lambda c, q: q, npart=16, C=1)
    sh["oSy"] = oy
    sh["oSq"] = oq
    rep = lambda a, n=128: np.broadcast_to(
        np.asarray(a, np.float32).reshape(1, -1),
        (n, np.asarray(a).size)).copy()
    sh["tkg"] = rep(f32("tk_lng"))
    sh["tkb"] = rep(f32("tk_lnb"))
    sh["tkmg"] = rep(f32("tkm_g"))
    sh["tkmb"] = rep(f32("tkm_b"))
    sh["chg"] = rep(f32("ch_lng"))
    sh["chb"] = rep(f32("ch_lnb"))
    sh["chmg"] = rep(f32("chm_g"))
    sh["chmb"] = rep(f32("chm_b"))
    sh["flng"] = rep(f32("fln_g"))
    sh["flnb"] = rep(f32("fln_b"))
    sh["og"] = rep(f32("o_lng"), 16)
    sh["ob"] = rep(f32("o_lnb"), 16)
    sh["tkmW1"] = np.concatenate([f32("tkm_W1")[i] for i in range(L)], 1)
    sh["tkmB1"] = np.stack([f32("tkm_b1")[i] for i in range(L)], 1)
    sh["tkmW2"] = np.concatenate([f32("tkm_W2")[i] for i in range(L)], 1)
    sh["tkmB2"] = np.stack([f32("tkm_b2")[i] for i in range(L)], 1)
    # chmW1[i] is (D=256, DFT=128); lhsT chunk c = chm_W1[i][c*128:(c+1)*128, :]
    sh["chmW1"] = np.concatenate(
        [f32("chm_W1")[i][c * 128:(c + 1) * 128, :]
         for i in range(L) for c in range(2)], 1)
    sh["chmB1"] = np.stack([f32("chm_b1")[i] for i in range(L)], 1)
    # chmW2[i] is (DFT=128, D=256); lhsT chunk c = chm_W2[i][:, c*128:(c+1)*128]
    sh["chmW2"] = np.concatenate(
        [f32("chm_W2")[i][:, c * 128:(c + 1) * 128]
         for i in range(L) for c in range(2)], 1)
    # chmB2: bias per d; chunk c column holds b2[c*128:(c+1)*128]
    sh["chmB2"] = np.stack(
        [f32("chm_b2")[i][c * 128:(c + 1) * 128]
         for i in range(L) for c in range(2)], 1)
    sh["outW"] = np.concatenate(
        [f32("out_W")[c * 128:(c + 1) * 128, :] for c in range(2)], 1)
    sh["outB"] = rep(f32("out_b"), 16)
    return xin, sh


def in_maps_for(xin, sh):
    maps = []
    for b in range(B):
        m = dict(sh)
        m["xinT"] = np.ascontiguousarray(
            xin[:, b].reshape(S * P, QIN).T)
        maps.append(m)
    return maps


from concourse.bass_utils import run_bass_kernel_spmd

_CACHE = {}


def kernel(**inputs):
    ins = {k: np.ascontiguousarray(np.asarray(v)) for k, v in inputs.items()}
    if "nc" not in _CACHE:
        _CACHE["nc"] = build_nc()
    nc = _CACHE["nc"]
    xin, sh = marshal(ins)
    maps = in_maps_for(xin, sh)
    res = run_bass_kernel_spmd(nc, maps, core_ids=list(range(8)))
    out = np.stack([res.results[c]["out"] for c in range(B)], axis=1)
    return out.astype(np.float32)


# revision 13
# speedup vs baseline: 1.3547x; 1.0372x over previous
"""Trainium2 Bass kernel for nn_CompatStatefulSelfModMixerModel.

Fully on-device: input projection, 2x (token SRWM scan + token mixer +
channel SRWM scan + channel mixer), final LN + patch-mean, output SRWM,
linear head - one Bass program per core. Data-parallel over batch B=8
across 8 NeuronCores (1 sample/core, weights replicated, no collectives).

Scan fast-math: fast-weight state kept in bf16 (DVE 2x_1p mode for all
big tensor_tensor ops), state split into y-rows (gpsimd-updated) and
q/k/beta-rows (vector-updated), softmax without max-subtraction, single
activation table set (rsqrt via exp(-0.5*ln(v)), sigmoid via exp).
"""
import sys

sys.path.insert(0, "/opt/trn_rl_repo")

import numpy as np

import concourse.bacc as bacc
import concourse.tile as tile
from concourse import mybir
from concourse import hw_specs as _hw

# Route every activation (Exp/Ln/Identity/Square/Copy) to the one table
# set containing them all, so the program needs a single ACT_TABLE_LOAD.
# Sets earlier in act_info.json order are emptied (indices preserved) so
# first-match lands on natural_log_exp_and_others.
_orig_gat = _hw.get_activation_tables


def _patched_gat(arch):
    t = _orig_gat(arch)
    shadow = ("exp_and_others", "softplus_and_others", "sigmoid_and_others",
              "sqrt_and_others", "small", "natural_log")
    return {k: (set() if k in shadow else v) for k, v in t.items()}


bacc.get_activation_tables = _patched_gat

F32 = mybir.dt.float32
BF = mybir.dt.bfloat16
AF = mybir.ActivationFunctionType
ALU = mybir.AluOpType
AX = mybir.AxisListType

S, B, NCLS = 16, 8, 5
D, H, DH = 256, 16, 16
PS, IMG = 7, 28
P = 16
L = 2
PD = 49
QIN = PD + NCLS  # 54
DFT = 128
EPS = 1e-5


def _srwm_scan(nc, wk, npart, C, sty, stq, x_of_step, ys_all,
               use_gp=True):
    """S steps of the SRWM recurrence.

    sty: f32 [npart, C*256] - Wy rows (muls on gpsimd, add on vector)
    stq: bf16 [npart, C*576] - Wq/Wk/wb rows, viewed [p, c, g, 16]
         (0:16 Wq, 16:32 Wk, 32:36 wb)
    x_of_step(s) -> bf16 AP [npart, C, 16]
    ys_all: fp32 [npart, S*C*16]; y_t lands at [:, s, c, :].
    """
    styf_v = sty[:, :].rearrange("p (c i j) -> p c i j", c=C, j=16)
    sty_v = styf_v
    stq_v = stq[:, :].rearrange("p (c g j) -> p c g j", c=C, j=16)
    GE = nc.gpsimd if use_gp else nc.vector

    for s in range(S):
        xt = x_of_step(s)  # [p, C, 16] bf16
        # y path: y_t = Wy . x in f32 (mul on gpsimd; reduce on vector,
        # scheduled into the exp-wait gap below)
        zy0 = wk.tile([npart, C * 256], F32, tag="sc_zy0")
        zy0_v = zy0[:, :].rearrange("p (c i j) -> p c i j", c=C, j=16)
        GE.tensor_mul(zy0_v, styf_v,
                      xt.unsqueeze(2).broadcast_to([npart, C, 16, 16]))

        # seg = [q; k; -b-logits] = stq . x (wb rows stored negated)
        zq = wk.tile([npart, C * 576], BF, tag="sc_zq")
        zq_v = zq[:, :].rearrange("p (c g j) -> p c g j", c=C, j=16)
        nc.vector.tensor_mul(zq_v, stq_v,
                             xt.unsqueeze(2).broadcast_to([npart, C, 36, 16]))
        zqh = wk.tile([npart, C * 288], BF, tag="sc_zqh")
        zqh_v = zqh[:, :].rearrange("p (c g h) -> p c g h", c=C, h=8)
        nc.vector.tensor_add(zqh_v, zq_v[:, :, :, 0:8], zq_v[:, :, :, 8:16])
        seg = wk.tile([npart, C * 36], F32, tag="sc_seg")
        seg_v = seg[:, :].rearrange("p (c g) -> p c g", c=C)
        nc.vector.tensor_reduce(seg_v, zqh_v, axis=AX.X, op=ALU.add)

        # one exp for q, k and (negated) beta logits
        es = wk.tile([npart, C * 36], F32, tag="sc_es")
        es_v = es[:, :].rearrange("p (c g) -> p c g", c=C)
        nc.scalar.activation(out=es_v, in_=seg_v, func=AF.Exp)
        eqk_v = es[:, :].rearrange("p (c g) -> p c g", c=C)[:, :, 0:32] \
            .rearrange("p c (t j) -> p c t j", t=2)

        # fill the exp wait: y reduce (gpsimd mul finished by now)
        y_out = ys_all[:, :].rearrange("p (s c j) -> p s c j", s=S, c=C)[:, s]
        nc.vector.tensor_reduce(y_out, zy0_v, axis=AX.X, op=ALU.add)

        sums = wk.tile([npart, C * 2], F32, tag="sc_sums")
        sums_v = sums[:, :].rearrange("p (c t) -> p c t", c=C)
        nc.vector.tensor_reduce(sums_v, eqk_v, axis=AX.X, op=ALU.add)
        rec = wk.tile([npart, C * 2], F32, tag="sc_rec")
        nc.vector.reciprocal_approx_fast(rec[:, :], sums[:, :])
        rec_v = rec[:, :].rearrange("p (c t) -> p c t", c=C)
        kq = wk.tile([npart, C * 32], BF, tag="sc_kq")
        kq_v = kq[:, :].rearrange("p (c t j) -> p c t j", c=C, t=2)
        nc.vector.tensor_mul(kq_v, eqk_v,
                             rec_v.unsqueeze(3).broadcast_to([npart, C, 2, 16]))
        qs = kq_v[:, :, 0]
        ks = kq_v[:, :, 1]
        e = wk.tile([npart, C * 16], BF, tag="sc_e")
        e_v = e[:, :].rearrange("p (c j) -> p c j", c=C)
        nc.vector.tensor_sub(e_v, qs, ks)

        # beta = 1 / (1 + exp(-logit))  (exp already in es rows 32:36)
        bta = wk.tile([npart, C * 4], F32, tag="sc_beta")
        bta_v = bta[:, :].rearrange("p (c w) -> p c w", c=C)
        nc.vector.tensor_scalar(bta_v, es_v[:, :, 32:36], 1.0, None, ALU.add)
        nc.vector.reciprocal_approx_fast(bta[:, :], bta[:, :])

        # d rows 16:52 = stq . (qs - ks)
        z2 = wk.tile([npart, C * 576], BF, tag="sc_z2")
        z2_v = z2[:, :].rearrange("p (c g j) -> p c g j", c=C, j=16)
        nc.vector.tensor_mul(z2_v, stq_v,
                             e_v.unsqueeze(2).broadcast_to([npart, C, 36, 16]))
        z2h = wk.tile([npart, C * 288], BF, tag="sc_z2h")
        z2h_v = z2h[:, :].rearrange("p (c g h) -> p c g h", c=C, h=8)
        nc.vector.tensor_add(z2h_v, z2_v[:, :, :, 0:8], z2_v[:, :, :, 8:16])
        d = wk.tile([npart, C * 52], F32, tag="sc_d")
        d_v = d[:, :].rearrange("p (c g) -> p c g", c=C)
        nc.vector.tensor_reduce(d_v[:, :, 16:52], z2h_v, axis=AX.X, op=ALU.add)

        # vy over both qs and ks: vykq[c,t,i] = sum_j Wy[c,i,j]*kq[c,t,j]
        zy = wk.tile([npart, C * 512], BF, tag="sc_zy")
        zy_v = zy[:, :].rearrange("p (c t i j) -> p c t i j", c=C, t=2, j=16)
        for t in range(2):
            nc.vector.tensor_mul(
                zy_v[:, :, t], sty_v,
                kq_v[:, :, t].unsqueeze(2).broadcast_to([npart, C, 16, 16]))
        zyh = wk.tile([npart, C * 256], BF, tag="sc_zyh")
        zyh_v = zyh[:, :].rearrange("p (a h) -> p a h", h=8)
        zy_f = zy[:, :].rearrange("p (a j) -> p a j", j=16)
        nc.vector.tensor_add(zyh_v, zy_f[:, :, 0:8], zy_f[:, :, 8:16])
        vykq = wk.tile([npart, C * 32], F32, tag="sc_vykq")
        vykq_v = vykq[:, :].rearrange("p (c t i) -> p c t i", c=C, t=2)
        nc.vector.tensor_reduce(vykq[:, :], zyh_v, axis=AX.X, op=ALU.add)

        # v-softmax on vy_q; d rows 0:16 = softmax(vy_q) - vy_k
        ev = wk.tile([npart, C * 16], F32, tag="sc_ev")
        ev_v = ev[:, :].rearrange("p (c i) -> p c i", c=C)
        nc.scalar.activation(out=ev_v, in_=vykq_v[:, :, 0], func=AF.Exp)

        # fill the exp wait: expand beta to per-row b52 (needs only bta)
        b52 = wk.tile([npart, C * 52], F32, tag="sc_b52")
        b52_v = b52[:, :].rearrange("p (c g) -> p c g", c=C)
        nc.vector.tensor_scalar(
            b52_v[:, :, 0:48].rearrange("p c (w g) -> p c w g", g=16),
            bta_v[:, :, 0:3].unsqueeze(3).broadcast_to([npart, C, 3, 16]),
            1.0, None, ALU.mult)
        nc.vector.tensor_scalar(
            b52_v[:, :, 48:52],
            bta_v[:, :, 3:4].broadcast_to([npart, C, 4]),
            1.0, None, ALU.mult)

        vs = wk.tile([npart, C], F32, tag="sc_vs")
        nc.vector.tensor_reduce(vs[:, :], ev_v, axis=AX.X, op=ALU.add)
        nc.vector.reciprocal_approx_fast(vs[:, :], vs[:, :])
        for c in range(C):
            nc.vector.scalar_tensor_tensor(
                out=d_v[:, c, 0:16], in0=ev_v[:, c], scalar=vs[:, c:c + 1],
                in1=vykq_v[:, c, 1], op0=ALU.mult, op1=ALU.subtract)

        d2x = wk.tile([npart, C * 104], BF, tag="sc_d2x")
        d2x_v = d2x[:, :].rearrange("p (c g t) -> p c g t", c=C, t=2)
        nc.vector.tensor_mul(
            d2x_v,
            d_v.unsqueeze(3).broadcast_to([npart, C, 52, 2]),
            b52_v.unsqueeze(3).broadcast_to([npart, C, 52, 2]))

        # state update: W += d (x) ks  (paired views keep 2x mode)
        kspq = ks.rearrange("p c (j2 t) -> p c j2 t", t=2)  # [p, C, 8, 2]
        zu = wk.tile([npart, C * 576], BF, tag="sc_zu")
        zu_p = zu[:, :].rearrange("p (c g j2 t) -> p c g j2 t", c=C, j2=8, t=2)
        for c in range(C):
            nc.vector.tensor_mul(
                zu_p[:, c],
                d2x_v[:, c, 16:52].unsqueeze(2)
                .broadcast_to([npart, 36, 8, 2]),
                kspq[:, c].unsqueeze(1).broadcast_to([npart, 36, 8, 2]))
        nc.vector.tensor_add(stq[:, :], stq[:, :], zu[:, :])
        zuy = wk.tile([npart, C * 256], F32, tag="sc_zuy")
        zuy_p = zuy[:, :].rearrange("p (c g j2 t) -> p c g j2 t",
                                    c=C, j2=8, t=2)
        for c in range(C):
            GE.tensor_mul(
                zuy_p[:, c],
                d2x_v[:, c, 0:16].unsqueeze(2)
                .broadcast_to([npart, 16, 8, 2]),
                kspq[:, c].unsqueeze(1).broadcast_to([npart, 16, 8, 2]))
        nc.vector.tensor_add(sty[:, :], sty[:, :], zuy[:, :])


def _ln(nc, wk, tag, npart, A1, A2, J, out_v, in_v, g_v, b_v):
    """LayerNorm over innermost J of 4-dim views [npart, A1, A2, J].

    rstd computed as exp(-0.5*ln(v+eps)) to stay in one act table set.
    """
    A = A1 * A2
    m = wk.tile([npart, A], F32, tag=f"ln_m_{tag}")
    v = wk.tile([npart, A], F32, tag=f"ln_v_{tag}")
    sq = wk.tile([npart, A * J], F32, tag="ln_sq")
    m4 = m[:, :].rearrange("p (a b) -> p a b", a=A1)
    v4 = v[:, :].rearrange("p (a b) -> p a b", a=A1)
    nc.vector.tensor_reduce(m4, in_v, axis=AX.X, op=ALU.add)
    nc.vector.tensor_scalar(m[:, :], m[:, :], -1.0 / J, None, ALU.mult)
    m_b = m4.unsqueeze(3).broadcast_to([npart, A1, A2, J])
    nc.vector.tensor_add(out_v, in_v, m_b)
    sq_v = sq[:, :].rearrange("p (a b j) -> p a b j", a=A1, b=A2)
    nc.vector.tensor_mul(sq_v, out_v, out_v)
    nc.vector.tensor_reduce(v4, sq_v, axis=AX.X, op=ALU.add)
    nc.vector.tensor_scalar(v[:, :], v[:, :], 1.0 / J, EPS, ALU.mult, ALU.add)
    nc.scalar.activation(out=v[:, :], in_=v[:, :], func=AF.Ln)
    nc.scalar.activation(out=v[:, :], in_=v[:, :], func=AF.Exp, scale=-0.5)
    v_b = v4.unsqueeze(3).broadcast_to([npart, A1, A2, J])
    nc.vector.tensor_mul(out_v, out_v, v_b)
    nc.vector.tensor_mul(out_v, out_v, g_v)
    nc.vector.tensor_add(out_v, out_v, b_v)


GELU_C = 1.5957691216057308  # 2*sqrt(2/pi)
GELU_A = 0.044715


def _gelu(nc, wk, out_v, pt_v, bias_ap, npart, F):
    """out = gelu_tanh(pt + bias); pt may be PSUM. [npart, F] views.

    sigmoid computed via exp to stay in one act table set.
    """
    xb = wk.tile([npart, F], F32, tag="gelu_xb")
    x2 = wk.tile([npart, F], F32, tag="gelu_x2")
    nc.scalar.activation(out=xb[:, :], in_=pt_v, func=AF.Identity,
                         bias=bias_ap, scale=1.0)
    nc.scalar.activation(out=x2[:, :], in_=pt_v, func=AF.Square,
                         bias=bias_ap, scale=1.0)
    nc.vector.tensor_scalar(x2[:, :], x2[:, :], GELU_A, 1.0, ALU.mult, ALU.add)
    nc.vector.tensor_mul(x2[:, :], x2[:, :], xb[:, :])
    nc.scalar.activation(out=x2[:, :], in_=x2[:, :], func=AF.Exp,
                         scale=-GELU_C)
    nc.vector.tensor_scalar(x2[:, :], x2[:, :], 1.0, None, ALU.add)
    nc.vector.reciprocal_approx_fast(x2[:, :], x2[:, :])
    nc.vector.tensor_mul(out_v, xb[:, :], x2[:, :])


def build_nc(debug=()):
    nc = bacc.Bacc(None, target_bir_lowering=False)
    dp = lambda nm, shp: nc.declare_dram_parameter(nm, shp, F32, isOutput=False)

    xinT = dp("xinT", [QIN, S * P])
    inW = dp("inW", [QIN, D])
    inb = dp("inb", [128, 2])
    I128 = dp("I128", [128, 128])
    MEAN = dp("MEAN", [128, 32])
    tkSy = dp("tkSy", [L * 128, 512])
    tkSq = dp("tkSq", [L * 128, 1152])
    chSy = dp("chSy", [L * 128, 512])
    chSq = dp("chSq", [L * 128, 1152])
    oSy = dp("oSy", [16, 256])
    oSq = dp("oSq", [16, 576])
    tkg = dp("tkg", [128, L * 16])
    tkb = dp("tkb", [128, L * 16])
    tkmg = dp("tkmg", [128, L * 256])
    tkmb = dp("tkmb", [128, L * 256])
    chg = dp("chg", [128, L * 256])
    chb = dp("chb", [128, L * 256])
    chmg = dp("chmg", [128, L * 256])
    chmb = dp("chmb", [128, L * 256])
    flng = dp("flng", [128, 256])
    flnb = dp("flnb", [128, 256])
    og = dp("og", [16, 256])
    ob = dp("ob", [16, 256])
    tkmW1 = dp("tkmW1", [16, L * 64])
    tkmB1 = dp("tkmB1", [64, L])
    tkmW2 = dp("tkmW2", [64, L * 16])
    tkmB2 = dp("tkmB2", [16, L])
    chmW1 = dp("chmW1", [128, L * 256])
    chmB1 = dp("chmB1", [128, L])
    chmW2 = dp("chmW2", [128, L * 256])
    chmB2 = dp("chmB2", [128, L * 2])
    outW = dp("outW", [128, 10])
    outB = dp("outB", [16, 5])
    out = nc.declare_dram_parameter("out", [S, NCLS], F32, isOutput=True)
    dbg = {}
    for nm, shp in debug:
        dbg[nm] = nc.declare_dram_parameter(nm, shp, F32, isOutput=True)

    with tile.TileContext(nc) as tc:
        with tc.tile_pool(name="cst", bufs=1) as cst, \
             tc.tile_pool(name="hb", bufs=1) as hb, \
             tc.tile_pool(name="wk", bufs=2) as wk, \
             tc.tile_pool(name="pst", bufs=4, space="PSUM") as pst, \
             tc.tile_pool(name="psm", bufs=2, space="PSUM") as psm, \
             tc.tile_pool(name="dr", bufs=1, space="DRAM") as dr:

            def load(tensor, shape, tag, sl=None):
                t = cst.tile(shape, F32, tag=tag)
                nc.gpsimd.dma_start(out=t, in_=tensor[:] if sl is None else sl)
                return t

            i_t = load(I128, [128, 128], "I128")
            mean_t = load(MEAN, [128, 32], "MEAN")
            tkg_t = load(tkg, [128, L * 16], "tkg")
            tkb_t = load(tkb, [128, L * 16], "tkb")
            tkmg_t = load(tkmg, [128, L * 256], "tkmg")
            tkmb_t = load(tkmb, [128, L * 256], "tkmb")
            chg_t = load(chg, [128, L * 256], "chg")
            chb_t = load(chb, [128, L * 256], "chb")
            chmg_t = load(chmg, [128, L * 256], "chmg")
            chmb_t = load(chmb, [128, L * 256], "chmb")
            flng_t = load(flng, [128, 256], "flng")
            flnb_t = load(flnb, [128, 256], "flnb")
            og_t = load(og, [16, 256], "og")
            ob_t = load(ob, [16, 256], "ob")
            tkmW1_t = load(tkmW1, [16, L * 64], "tkmW1")
            tkmB1_t = load(tkmB1, [64, L], "tkmB1")
            tkmW2_t = load(tkmW2, [64, L * 16], "tkmW2")
            tkmB2_t = load(tkmB2, [16, L], "tkmB2")
            chmW1_t = load(chmW1, [128, L * 256], "chmW1")
            chmB1_t = load(chmB1, [128, L], "chmB1")
            chmW2_t = load(chmW2, [128, L * 256], "chmW2")
            chmB2_t = load(chmB2, [128, L * 2], "chmB2")
            outW_t = load(outW, [128, 10], "outW")
            outB_t = load(outB, [16, 5], "outB")

            # ---- A. input projection -> hT (bf16) [128, (c, s*16+p)] ----
            xin_t = cst.tile([QIN, S * P], F32, tag="xin")
            w_t = cst.tile([QIN, D], F32, tag="inW")
            b_t = cst.tile([128, 2], F32, tag="inb")
            nc.gpsimd.dma_start(out=xin_t, in_=xinT[:])
            nc.gpsimd.dma_start(out=w_t, in_=inW[:])
            nc.gpsimd.dma_start(out=b_t, in_=inb[:])
            hT = hb.tile([128, 512], BF, tag="hT0")
            for c in range(2):
                pt = psm.tile([128, S * P], F32, tag="psm")
                nc.tensor.matmul(pt, w_t[:, c * 128:(c + 1) * 128], xin_t,
                                 start=True, stop=True)
                nc.scalar.activation(out=hT[:, c * 256:(c + 1) * 256], in_=pt,
                                     func=AF.Identity, bias=b_t[:, c:c + 1],
                                     scale=1.0)

            def t_to_sp(src, dst):
                """src [128, (c,sp)] hT-layout -> dst [128, (r,d)] SP-layout."""
                for c in range(2):
                    for r in range(2):
                        pt = pst.tile([128, 128], F32, tag="pst")
                        nc.tensor.transpose(
                            pt, src[:, c * 256 + r * 128:c * 256 + (r + 1) * 128],
                            i_t)
                        nc.scalar.copy(
                            out=dst[:, r * 256 + c * 128:r * 256 + (c + 1) * 128],
                            in_=pt)

            def t_to_ht(src, dst):
                for r in range(2):
                    for c in range(2):
                        pt = pst.tile([128, 128], F32, tag="pst")
                        nc.tensor.transpose(
                            pt, src[:, r * 256 + c * 128:r * 256 + (c + 1) * 128],
                            i_t)
                        nc.scalar.copy(
                            out=dst[:, c * 256 + r * 128:c * 256 + (r + 1) * 128],
                            in_=pt)

            def tap(nm, t):
                if nm in dbg:
                    nc.gpsimd.dma_start(out=dbg[nm][:], in_=t)

            def load_state(dram_y, dram_q, row0, npart, C):
                sty = hb.tile([npart, C * 256], F32, tag="sty_f")
                stq = hb.tile([npart, C * 576], BF, tag="stq")
                sgq = wk.tile([npart, C * 576], F32, tag="stg_q")
                nc.gpsimd.dma_start(out=sty,
                                    in_=dram_y[row0:row0 + npart, :])
                nc.gpsimd.dma_start(out=sgq, in_=dram_q[row0:row0 + npart, :])
                nc.vector.tensor_copy(stq[:, :], sgq[:, :])
                return sty, stq

            h_sp = None
            for i in range(L):
                # ---- B1. token SRWM ----
                sty, stq = load_state(tkSy, tkSq, i * 128, 128, 2)
                ys_tk = hb.tile([128, S * 2 * 16], F32, tag="ys")
                ht_cur = hT

                def x_tk(s, ht_cur=ht_cur):
                    return ht_cur[:, :].rearrange("p (c sp) -> p c sp", c=2)[
                        :, :, s * 16:(s + 1) * 16]

                _srwm_scan(nc, wk, 128, 2, sty, stq, x_tk, ys_tk)

                hT2 = hb.tile([128, 512], F32, tag="hT2")
                ys_v = ys_tk[:, :].rearrange("p (s c j) -> p c s j", s=S, c=2)
                out_v = hT2[:, :].rearrange("p (c s j) -> p c s j", c=2, j=16)
                g_v = tkg_t[:, i * 16:(i + 1) * 16].unsqueeze(1).unsqueeze(1) \
                    .broadcast_to([128, 2, 16, 16])
                b_v = tkb_t[:, i * 16:(i + 1) * 16].unsqueeze(1).unsqueeze(1) \
                    .broadcast_to([128, 2, 16, 16])
                _ln(nc, wk, "a", 128, 2, 16, 16, out_v, ys_v, g_v, b_v)
                tap(f"d_hT2_{i}", hT2)

                # ---- B2. token mixer ----
                h2 = hb.tile([128, 512], F32, tag="h2")
                t_to_sp(hT2, h2)
                hn = hb.tile([128, 512], F32, tag="hn")
                hn_v = hn[:, :].rearrange("p (a r d) -> p a r d", a=1, r=2)
                h2_v = h2[:, :].rearrange("p (a r d) -> p a r d", a=1, r=2)
                g_v = tkmg_t[:, i * 256:(i + 1) * 256].unsqueeze(1).unsqueeze(1) \
                    .broadcast_to([128, 1, 2, 256])
                b_v = tkmb_t[:, i * 256:(i + 1) * 256].unsqueeze(1).unsqueeze(1) \
                    .broadcast_to([128, 1, 2, 256])
                _ln(nc, wk, "b", 128, 1, 2, 256, hn_v, h2_v, g_v, b_v)
                drn = dr.tile([S, P, D], F32, tag="drn")
                for r in range(2):
                    nc.gpsimd.dma_start(
                        out=drn[r * 8:(r + 1) * 8, :, :],
                        in_=hn[:, r * 256:(r + 1) * 256])
                pmj = hb.tile([16, S * D], F32, tag="pmj")
                nc.gpsimd.dma_start(
                    out=pmj[:, :].rearrange("p (s d) -> p s d", s=S),
                    in_=drn[:, :, :].rearrange("s p d -> p s d"))
                gl = hb.tile([64, S * D], F32, tag="gl")
                for k in range(8):
                    pt1 = psm.tile([64, 512], F32, tag="psm")
                    nc.tensor.matmul(pt1, tkmW1_t[:, i * 64:(i + 1) * 64],
                                     pmj[:, k * 512:(k + 1) * 512],
                                     start=True, stop=True)
                    _gelu(nc, wk, gl[:, k * 512:(k + 1) * 512], pt1[:, :],
                          tkmB1_t[:, i:i + 1], 64, 512)
                mxo = hb.tile([16, S * D], F32, tag="mxo")
                for k in range(8):
                    pt2 = psm.tile([16, 512], F32, tag="psm")
                    nc.tensor.matmul(pt2, tkmW2_t[:, i * 16:(i + 1) * 16],
                                     gl[:, k * 512:(k + 1) * 512],
                                     start=True, stop=True)
                    nc.scalar.activation(out=mxo[:, k * 512:(k + 1) * 512],
                                         in_=pt2, func=AF.Identity,
                                         bias=tkmB2_t[:, i:i + 1], scale=1.0)
                dr3 = dr.tile([S, P, D], F32, tag="dr3")
                nc.gpsimd.dma_start(
                    out=dr3[:, :, :].rearrange("s p d -> p s d"),
                    in_=mxo[:, :].rearrange("p (s d) -> p s d", s=S))
                mxsp = hb.tile([128, 512], F32, tag="mxsp")
                for r in range(2):
                    nc.gpsimd.dma_start(
                        out=mxsp[:, r * 256:(r + 1) * 256],
                        in_=dr3[r * 8:(r + 1) * 8, :, :])
                h3 = hb.tile([128, 512], F32, tag="h3")
                nc.vector.tensor_add(h3[:, :], h2[:, :], mxsp[:, :])
                tap(f"d_h3_{i}", h3)

                # ---- B3. channel SRWM ----
                dr1 = dr.tile([S, P, H, DH], F32, tag="dr1")
                for r in range(2):
                    nc.gpsimd.dma_start(
                        out=dr1[r * 8:(r + 1) * 8, :, :, :].rearrange(
                            "s p hh j -> s p (hh j)"),
                        in_=h3[:, r * 256:(r + 1) * 256])
                xc = hb.tile([128, 512], F32, tag="xc")
                for c in range(2):
                    nc.gpsimd.dma_start(
                        out=xc[:, :].rearrange("q (c s j) -> q c s j",
                                               c=2, s=S)[:, c],
                        in_=dr1[:, c * 8:(c + 1) * 8, :, :].rearrange(
                            "s ph hh j -> (ph hh) s j"))
                xcb = hb.tile([128, 512], BF, tag="xcb")
                nc.vector.tensor_copy(xcb[:, :], xc[:, :])
                sty, stq = load_state(chSy, chSq, i * 128, 128, 2)
                ys_ch = hb.tile([128, S * 2 * 16], F32, tag="ys")

                def x_ch(s, xcb=xcb):
                    return xcb[:, :].rearrange("p (c s j) -> p c s j",
                                               c=2, s=S)[:, :, s, :]

                _srwm_scan(nc, wk, 128, 2, sty, stq, x_ch, ys_ch)

                dr2 = dr.tile([S, P, H, DH], F32, tag="dr2")
                for c in range(2):
                    nc.gpsimd.dma_start(
                        out=dr2[:, c * 8:(c + 1) * 8, :, :].rearrange(
                            "s ph hh i -> (ph hh) s i"),
                        in_=ys_ch[:, :].rearrange(
                            "q (s c j) -> q s c j", s=S, c=2)[:, :, c, :])
                ysp = hb.tile([128, 512], F32, tag="ysp")
                for r in range(2):
                    nc.gpsimd.dma_start(
                        out=ysp[:, r * 256:(r + 1) * 256],
                        in_=dr2[r * 8:(r + 1) * 8, :, :, :].rearrange(
                            "s p hh i -> s p (hh i)"))
                h4 = hb.tile([128, 512], F32, tag="h4")
                h4_v = h4[:, :].rearrange("p (a r d) -> p a r d", a=1, r=2)
                ysp_v = ysp[:, :].rearrange("p (a r d) -> p a r d", a=1, r=2)
                g_v = chg_t[:, i * 256:(i + 1) * 256].unsqueeze(1).unsqueeze(1) \
                    .broadcast_to([128, 1, 2, 256])
                b_v = chb_t[:, i * 256:(i + 1) * 256].unsqueeze(1).unsqueeze(1) \
                    .broadcast_to([128, 1, 2, 256])
                _ln(nc, wk, "c", 128, 1, 2, 256, h4_v, ysp_v, g_v, b_v)
                tap(f"d_h4_{i}", h4)

                # ---- B4. channel mixer ----
                hn2 = hb.tile([128, 512], F32, tag="hn2")
                hn2_v = hn2[:, :].rearrange("p (a r d) -> p a r d", a=1, r=2)
                g_v = chmg_t[:, i * 256:(i + 1) * 256].unsqueeze(1).unsqueeze(1) \
                    .broadcast_to([128, 1, 2, 256])
                b_v = chmb_t[:, i * 256:(i + 1) * 256].unsqueeze(1).unsqueeze(1) \
                    .broadcast_to([128, 1, 2, 256])
                _ln(nc, wk, "d", 128, 1, 2, 256, hn2_v, h4_v, g_v, b_v)
                hn2T = hb.tile([128, 512], F32, tag="hn2T")
                t_to_ht(hn2, hn2T)
                pt1 = psm.tile([128, 256], F32, tag="psm")
                for c in range(2):
                    nc.tensor.matmul(
                        pt1, chmW1_t[:, i * 256 + c * 128:i * 256 + (c + 1) * 128],
                        hn2T[:, c * 256:(c + 1) * 256],
                        start=(c == 0), stop=(c == 1))
                gl2 = hb.tile([128, 256], F32, tag="gl2")
                _gelu(nc, wk, gl2[:, :], pt1[:, :], chmB1_t[:, i:i + 1],
                      128, 256)
                moT = hb.tile([128, 512], F32, tag="moT")
                for c in range(2):
                    pt2 = psm.tile([128, 256], F32, tag="psm")
                    nc.tensor.matmul(
                        pt2, chmW2_t[:, i * 256 + c * 128:i * 256 + (c + 1) * 128],
                        gl2, start=True, stop=True)
                    nc.scalar.activation(out=moT[:, c * 256:(c + 1) * 256],
                                         in_=pt2, func=AF.Identity,
                                         bias=chmB2_t[:, i * 2 + c:i * 2 + c + 1],
                                         scale=1.0)
                mosp = hb.tile([128, 512], F32, tag="mosp")
                t_to_sp(moT, mosp)
                h5 = hb.tile([128, 512], F32, tag="h5")
                nc.vector.tensor_add(h5[:, :], h4[:, :], mosp[:, :])
                h_sp = h5
                tap(f"d_h5_{i}", h5)

                if i + 1 < L:
                    hT = hb.tile([128, 512], BF, tag="hT0")
                    t_to_ht(h_sp, hT)

            # ---- C. final ----
            hf = hb.tile([128, 512], F32, tag="hf")
            hf_v = hf[:, :].rearrange("p (a r d) -> p a r d", a=1, r=2)
            hsp_v = h_sp[:, :].rearrange("p (a r d) -> p a r d", a=1, r=2)
            g_v = flng_t[:, :].unsqueeze(1).unsqueeze(1) \
                .broadcast_to([128, 1, 2, 256])
            b_v = flnb_t[:, :].unsqueeze(1).unsqueeze(1) \
                .broadcast_to([128, 1, 2, 256])
            _ln(nc, wk, "f", 128, 1, 2, 256, hf_v, hsp_v, g_v, b_v)
            pm = psm.tile([16, 256], F32, tag="psm")
            for r in range(2):
                nc.tensor.matmul(pm, mean_t[:, r * 16:(r + 1) * 16],
                                 hf[:, r * 256:(r + 1) * 256],
                                 start=(r == 0), stop=(r == 1))
            ho = hb.tile([16, 256], F32, tag="ho")
            nc.scalar.copy(out=ho, in_=pm)
            tap("d_ho", ho)
            dr4 = dr.tile([S, H, DH], F32, tag="dr4")
            nc.gpsimd.dma_start(
                out=dr4[:, :, :],
                in_=ho[:, :].rearrange("s (hh j) -> s hh j", hh=H))
            xo = hb.tile([16, S * DH], F32, tag="xo")
            nc.gpsimd.dma_start(
                out=xo[:, :].rearrange("p (s j) -> p s j", s=S),
                in_=dr4[:, :, :].rearrange("s hh j -> hh s j"))
            xob = hb.tile([16, S * DH], BF, tag="xob")
            nc.vector.tensor_copy(xob[:, :], xo[:, :])
            sty, stq = load_state(oSy, oSq, 0, 16, 1)
            ys_o = hb.tile([16, S * 16], F32, tag="ys_o")

            def x_o(s, xob=xob):
                return xob[:, :].rearrange("p (c s j) -> p c s j", c=1, s=S)[
                    :, :, s, :]

            _srwm_scan(nc, wk, 16, 1, sty, stq, x_o, ys_o, use_gp=False)

            dr5 = dr.tile([H, S, DH], F32, tag="dr5")
            nc.gpsimd.dma_start(
                out=dr5[:, :, :],
                in_=ys_o[:, :].rearrange("p (s j) -> p s j", s=S))
            ho2 = hb.tile([16, 256], F32, tag="ho2")
            nc.gpsimd.dma_start(
                out=ho2[:, :].rearrange("s (hh i) -> s hh i", hh=H),
                in_=dr5[:, :, :].rearrange("hh s i -> s hh i"))
            hon = hb.tile([16, 256], F32, tag="hon")
            _ln(nc, wk, "o", 16, 1, 1, 256,
                hon[:, :].unsqueeze(1).unsqueeze(1),
                ho2[:, :].unsqueeze(1).unsqueeze(1),
                og_t[:, :].unsqueeze(1).unsqueeze(1),
                ob_t[:, :].unsqueeze(1).unsqueeze(1))
            hoT = hb.tile([128, 32], F32, tag="hoT")
            for c in range(2):
                pt = pst.tile([128, 128], F32, tag="pst")
                nc.tensor.transpose(pt[:, 0:16], hon[:, c * 128:(c + 1) * 128],
                                    i_t[0:16, 0:16])
                nc.scalar.copy(out=hoT[:, c * 16:(c + 1) * 16], in_=pt[:, 0:16])
            po = psm.tile([16, 5], F32, tag="psm")
            for c in range(2):
                nc.tensor.matmul(po, hoT[:, c * 16:(c + 1) * 16],
                                 outW_t[:, c * 5:(c + 1) * 5],
                                 start=(c == 0), stop=(c == 1))
            fin = hb.tile([16, 5], F32, tag="fin")
            nc.vector.tensor_add(fin[:, :], po[:, :], outB_t[:, :])
            nc.gpsimd.dma_start(out=out[:], in_=fin)

    nc.finalize()
    return nc


# ------------------------- host marshaling -------------------------

def _patchify(x):
    s, bb, c, hh, ww = x.shape
    h, w = hh // PS, ww // PS
    x = x.reshape(s, bb, c, h, PS, w, PS)
    x = x.transpose(0, 1, 3, 5, 4, 6, 2)
    return x.reshape(s, bb, h * w, PS * PS * c)


def _state_init(Wy, Wq, Wk, wb, pair_of, npart=128, C=2):
    sy = np.zeros((npart, C, 16, 16), np.float32)
    sq = np.zeros((npart, C, 36, 16), np.float32)
    for c in range(C):
        for q in range(npart):
            h = pair_of(c, q)
            sy[q, c] = Wy[h]
            sq[q, c, 0:16] = Wq[h]
            sq[q, c, 16:32] = Wk[h]
            # wb rows stored negated: seg_b = -logit, so sigmoid(logit)
            # = 1/(1+exp(seg_b)) shares the single per-step exp, and the
            # negated rows are self-consistent under the delta update.
            sq[q, c, 32:36] = -wb[h]
    return sy.reshape(npart, C * 256), sq.reshape(npart, C * 576)


def marshal(ins):
    """Returns (xin (S,B,P,QIN), shared input dict)."""
    x, fb = ins["x"].astype(np.float32), ins["fb"]
    xp = _patchify(x)
    emb = np.zeros((S, B, NCLS), np.float32)
    emb[np.arange(S)[:, None], np.arange(B)[None, :], fb] = 1.0
    emb = np.broadcast_to(emb[:, :, None, :], (S, B, P, NCLS))
    xin = np.concatenate([xp, emb], -1)

    f32 = lambda k: np.asarray(ins[k], np.float32)
    sh = {}
    sh["inW"] = np.ascontiguousarray(f32("in_W"))
    sh["inb"] = f32("in_b").reshape(2, 128).T.copy()
    sh["I128"] = np.eye(128, dtype=np.float32)
    mean = np.zeros((128, 32), np.float32)
    for r in range(2):
        for sp in range(128):
            s = r * 8 + sp // 16
            mean[sp, r * 16 + s] = 1.0 / 16.0
    sh["MEAN"] = mean
    tky, tkq = zip(*[
        _state_init(f32("tk_Wy")[i], f32("tk_Wq")[i], f32("tk_Wk")[i],
                    f32("tk_wb")[i], lambda c, q: 0) for i in range(L)])
    sh["tkSy"] = np.concatenate(tky, 0)
    sh["tkSq"] = np.concatenate(tkq, 0)
    chy, chq = zip(*[
        _state_init(f32("ch_Wy")[i], f32("ch_Wq")[i], f32("ch_Wk")[i],
                    f32("ch_wb")[i], lambda c, q: q % 16) for i in range(L)])
    sh["chSy"] = np.concatenate(chy, 0)
    sh["chSq"] = np.concatenate(chq, 0)
    oy, oq = _state_init(f32("o_Wy"), f32("o_Wq"), f32("o_Wk"), f32("o_wb"),
                         lambda c, q: q, npart=16, C=1)
    sh["oSy"] = oy
    sh["oSq"] = oq
    rep = lambda a, n=128: np.broadcast_to(
        np.asarray(a, np.float32).reshape(1, -1),
        (n, np.asarray(a).size)).copy()
    sh["tkg"] = rep(f32("tk_lng"))
    sh["tkb"] = rep(f32("tk_lnb"))
    sh["tkmg"] = rep(f32("tkm_g"))
    sh["tkmb"] = rep(f32("tkm_b"))
    sh["chg"] = rep(f32("ch_lng"))
    sh["chb"] = rep(f32("ch_lnb"))
    sh["chmg"] = rep(f32("chm_g"))
    sh["chmb"] = rep(f32("chm_b"))
    sh["flng"] = rep(f32("fln_g"))
    sh["flnb"] = rep(f32("fln_b"))
    sh["og"] = rep(f32("o_lng"), 16)
    sh["ob"] = rep(f32("o_lnb"), 16)
    sh["tkmW1"] = np.concatenate([f32("tkm_W1")[i] for i in range(L)], 1)
    sh["tkmB1"] = np.stack([f32("tkm_b1")[i] for i in range(L)], 1)
    sh["tkmW2"] = np.concatenate([f32("tkm_W2")[i] for i in range(L)], 1)
    sh["tkmB2"] = np.stack([f32("tkm_b2")[i] for i in range(L)], 1)
    # chmW1[i] is (D=256, DFT=128); lhsT chunk c = chm_W1[i][c*128:(c+1)*128, :]
    sh["chmW1"] = np.concatenate(
        [f32("chm_W1")[i][c * 128:(c + 1) * 128, :]
         for i in range(L) for c in range(2)], 1)
    sh["chmB1"] = np.stack([f32("chm_b1")[i] for i in range(L)], 1)
    # chmW2[i] is (DFT=128, D=256); lhsT chunk c = chm_W2[i][:, c*128:(c+1)*128]
    sh["chmW2"] = np.concatenate(
        [f32("chm_W2")[i][:, c * 128:(c + 1) * 128]
         for i in range(L) for c in range(2)], 1)
    # chmB2: bias per d; chunk c column holds b2[c*128:(c+1)*128]
    sh["chmB2"] = np.stack(
        [f32("chm_b2")[i][c * 128:(c + 1) * 128]
         for i in range(L) for c in range(2)], 1)
    sh["outW"] = np.concatenate(
        [f32("out_W")[c * 128:(c + 1) * 128, :] for c in range(2)], 1)
    sh["outB"] = rep(f32("out_b"), 16)
    return xin, sh


def in_maps_for(xin, sh):
    maps = []
    for b in range(B):
        m = dict(sh)
        m["xinT"] = np.ascontiguousarray(
            xin[:, b].reshape(S * P, QIN).T)
        maps.append(m)
    return maps


from concourse.bass_utils import run_bass_kernel_spmd

_CACHE = {}


def kernel(**inputs):
    ins = {k: np.ascontiguousarray(np.asarray(v)) for k, v in inputs.items()}
    if "nc" not in _CACHE:
        _CACHE["nc"] = build_nc()
    nc = _CACHE["nc"]
    xin, sh = marshal(ins)
    maps = in_maps_for(xin, sh)
    res = run_bass_kernel_spmd(nc, maps, core_ids=list(range(8)))
    out = np.stack([res.results[c]["out"] for c in range(B)], axis=1)
    return out.astype(np.float32)


# revision 16
# speedup vs baseline: 1.5096x; 1.1143x over previous
"""Trainium2 Bass kernel for nn_CompatStatefulSelfModMixerModel.

Fully on-device: input projection, 2x (token SRWM scan + token mixer +
channel SRWM scan + channel mixer), final LN + patch-mean, output SRWM,
linear head - one Bass program per core. Data-parallel over batch B=8
across 8 NeuronCores (1 sample/core, weights replicated, no collectives).

Scan fast-math: fast-weight state kept in bf16 (DVE 2x_1p mode for all
big tensor_tensor ops), state split into y-rows (gpsimd-updated) and
q/k/beta-rows (vector-updated), softmax without max-subtraction, single
activation table set (rsqrt via exp(-0.5*ln(v)), sigmoid via exp).
"""
import sys

sys.path.insert(0, "/opt/trn_rl_repo")

import numpy as np

import concourse.bacc as bacc
import concourse.tile as tile
from concourse import mybir
from concourse import hw_specs as _hw

# Route every activation (Exp/Ln/Identity/Square/Copy) to the one table
# set containing them all, so the program needs a single ACT_TABLE_LOAD.
# Sets earlier in act_info.json order are emptied (indices preserved) so
# first-match lands on natural_log_exp_and_others.
_orig_gat = _hw.get_activation_tables


def _patched_gat(arch):
    t = _orig_gat(arch)
    shadow = ("exp_and_others", "softplus_and_others", "sigmoid_and_others",
              "sqrt_and_others", "small", "natural_log")
    return {k: (set() if k in shadow else v) for k, v in t.items()}


bacc.get_activation_tables = _patched_gat

F32 = mybir.dt.float32
BF = mybir.dt.bfloat16
AF = mybir.ActivationFunctionType
ALU = mybir.AluOpType
AX = mybir.AxisListType

S, B, NCLS = 16, 8, 5
D, H, DH = 256, 16, 16
PS, IMG = 7, 28
P = 16
L = 2
PD = 49
QIN = PD + NCLS  # 54
DFT = 128
EPS = 1e-5


def _srwm_scan(nc, wk, npart, C, sty, stq, x_of_step, ys_all,
               use_gp=True):
    """S steps of the SRWM recurrence.

    sty: f32 [npart, C*256] - Wy rows (muls on gpsimd, add on vector)
    stq: bf16 [npart, C*576] - Wq/Wk/wb rows, viewed [p, c, g, 16]
         (0:16 Wq, 16:32 Wk, 32:36 wb)
    x_of_step(s) -> bf16 AP [npart, C, 16]
    ys_all: fp32 [npart, S*C*16]; y_t lands at [:, s, c, :].
    """
    styf_v = sty[:, :].rearrange("p (c i j) -> p c i j", c=C, j=16)
    stq_v = stq[:, :].rearrange("p (c g j) -> p c g j", c=C, j=16)
    GE = nc.gpsimd if use_gp else nc.vector
    prev_zy0 = None

    for s in range(S):
        xt = x_of_step(s)  # [p, C, 16] bf16
        # y path: y_t = Wy . x in f32 on gpsimd; the reduce is deferred
        # to the next step so gpsimd has a full step of slack.
        zy0 = wk.tile([npart, C * 256], F32, tag="sc_zy0")
        zy0_v = zy0[:, :].rearrange("p (c i j) -> p c i j", c=C, j=16)
        GE.tensor_mul(zy0_v, styf_v,
                      xt.unsqueeze(2).broadcast_to([npart, C, 16, 16]))
        if prev_zy0 is not None:
            y_prev = ys_all[:, :].rearrange("p (s c j) -> p s c j",
                                            s=S, c=C)[:, s - 1]
            nc.vector.tensor_reduce(y_prev, prev_zy0, axis=AX.X, op=ALU.add)
        prev_zy0 = zy0_v

        # bf16 mirror of Wy for the vy-path muls
        styb = wk.tile([npart, C * 256], BF, tag="sc_styb")
        nc.vector.tensor_copy(styb[:, :], sty[:, :])
        sty_v = styb[:, :].rearrange("p (c i j) -> p c i j", c=C, j=16)

        # seg = [q; k; -b-logits] = stq . x (wb rows stored negated)
        zq = wk.tile([npart, C * 576], BF, tag="sc_zq")
        zq_v = zq[:, :].rearrange("p (c g j) -> p c g j", c=C, j=16)
        nc.vector.tensor_mul(zq_v, stq_v,
                             xt.unsqueeze(2).broadcast_to([npart, C, 36, 16]))
        zqh = wk.tile([npart, C * 288], BF, tag="sc_zqh")
        zqh_v = zqh[:, :].rearrange("p (c g h) -> p c g h", c=C, h=8)
        nc.vector.tensor_add(zqh_v, zq_v[:, :, :, 0:8], zq_v[:, :, :, 8:16])
        seg = wk.tile([npart, C * 36], F32, tag="sc_seg")
        seg_v = seg[:, :].rearrange("p (c g) -> p c g", c=C)
        nc.vector.tensor_reduce(seg_v, zqh_v, axis=AX.X, op=ALU.add)

        # one exp for q, k and (negated) beta logits
        es = wk.tile([npart, C * 36], F32, tag="sc_es")
        es_v = es[:, :].rearrange("p (c g) -> p c g", c=C)
        nc.scalar.activation(out=es_v, in_=seg_v, func=AF.Exp)
        eqk_v = es[:, :].rearrange("p (c g) -> p c g", c=C)[:, :, 0:32] \
            .rearrange("p c (t j) -> p c t j", t=2)

        sums = wk.tile([npart, C * 2], F32, tag="sc_sums")
        sums_v = sums[:, :].rearrange("p (c t) -> p c t", c=C)
        nc.vector.tensor_reduce(sums_v, eqk_v, axis=AX.X, op=ALU.add)
        rec = wk.tile([npart, C * 2], F32, tag="sc_rec")
        nc.vector.reciprocal_approx_fast(rec[:, :], sums[:, :])
        rec_v = rec[:, :].rearrange("p (c t) -> p c t", c=C)
        kq = wk.tile([npart, C * 32], BF, tag="sc_kq")
        kq_v = kq[:, :].rearrange("p (c t j) -> p c t j", c=C, t=2)
        nc.vector.tensor_mul(kq_v, eqk_v,
                             rec_v.unsqueeze(3).broadcast_to([npart, C, 2, 16]))
        qs = kq_v[:, :, 0]
        ks = kq_v[:, :, 1]
        e = wk.tile([npart, C * 16], BF, tag="sc_e")
        e_v = e[:, :].rearrange("p (c j) -> p c j", c=C)
        nc.vector.tensor_sub(e_v, qs, ks)

        # beta = 1 / (1 + exp(-logit))  (exp already in es rows 32:36)
        bta = wk.tile([npart, C * 4], F32, tag="sc_beta")
        bta_v = bta[:, :].rearrange("p (c w) -> p c w", c=C)
        nc.vector.tensor_scalar(bta_v, es_v[:, :, 32:36], 1.0, None, ALU.add)
        nc.vector.reciprocal_approx_fast(bta[:, :], bta[:, :])

        # d rows 16:52 = stq . (qs - ks)
        z2 = wk.tile([npart, C * 576], BF, tag="sc_z2")
        z2_v = z2[:, :].rearrange("p (c g j) -> p c g j", c=C, j=16)
        nc.vector.tensor_mul(z2_v, stq_v,
                             e_v.unsqueeze(2).broadcast_to([npart, C, 36, 16]))
        z2h = wk.tile([npart, C * 288], BF, tag="sc_z2h")
        z2h_v = z2h[:, :].rearrange("p (c g h) -> p c g h", c=C, h=8)
        nc.vector.tensor_add(z2h_v, z2_v[:, :, :, 0:8], z2_v[:, :, :, 8:16])
        d = wk.tile([npart, C * 52], F32, tag="sc_d")
        d_v = d[:, :].rearrange("p (c g) -> p c g", c=C)
        nc.vector.tensor_reduce(d_v[:, :, 16:52], z2h_v, axis=AX.X, op=ALU.add)

        # vy over both qs and ks: vykq[c,t,i] = sum_j Wy[c,i,j]*kq[c,t,j]
        zy = wk.tile([npart, C * 512], BF, tag="sc_zy")
        zy_v = zy[:, :].rearrange("p (c t i j) -> p c t i j", c=C, t=2, j=16)
        for t in range(2):
            nc.vector.tensor_mul(
                zy_v[:, :, t], sty_v,
                kq_v[:, :, t].unsqueeze(2).broadcast_to([npart, C, 16, 16]))
        zyh = wk.tile([npart, C * 256], BF, tag="sc_zyh")
        zyh_v = zyh[:, :].rearrange("p (a h) -> p a h", h=8)
        zy_f = zy[:, :].rearrange("p (a j) -> p a j", j=16)
        nc.vector.tensor_add(zyh_v, zy_f[:, :, 0:8], zy_f[:, :, 8:16])
        vykq = wk.tile([npart, C * 32], F32, tag="sc_vykq")
        vykq_v = vykq[:, :].rearrange("p (c t i) -> p c t i", c=C, t=2)
        nc.vector.tensor_reduce(vykq[:, :], zyh_v, axis=AX.X, op=ALU.add)

        # v-softmax on vy_q; d rows 0:16 = softmax(vy_q) - vy_k
        ev = wk.tile([npart, C * 16], F32, tag="sc_ev")
        ev_v = ev[:, :].rearrange("p (c i) -> p c i", c=C)
        nc.scalar.activation(out=ev_v, in_=vykq_v[:, :, 0], func=AF.Exp)

        # fill the exp wait: expand beta to per-row b52 (needs only bta)
        b52 = wk.tile([npart, C * 52], F32, tag="sc_b52")
        b52_v = b52[:, :].rearrange("p (c g) -> p c g", c=C)
        nc.vector.tensor_scalar(
            b52_v[:, :, 0:48].rearrange("p c (w g) -> p c w g", g=16),
            bta_v[:, :, 0:3].unsqueeze(3).broadcast_to([npart, C, 3, 16]),
            1.0, None, ALU.mult)
        nc.vector.tensor_scalar(
            b52_v[:, :, 48:52],
            bta_v[:, :, 3:4].broadcast_to([npart, C, 4]),
            1.0, None, ALU.mult)

        vs = wk.tile([npart, C], F32, tag="sc_vs")
        nc.vector.tensor_reduce(vs[:, :], ev_v, axis=AX.X, op=ALU.add)
        nc.vector.reciprocal_approx_fast(vs[:, :], vs[:, :])
        for c in range(C):
            nc.vector.scalar_tensor_tensor(
                out=d_v[:, c, 0:16], in0=ev_v[:, c], scalar=vs[:, c:c + 1],
                in1=vykq_v[:, c, 1], op0=ALU.mult, op1=ALU.subtract)

        d2x = wk.tile([npart, C * 104], BF, tag="sc_d2x")
        d2x_v = d2x[:, :].rearrange("p (c g t) -> p c g t", c=C, t=2)
        nc.vector.tensor_mul(
            d2x_v,
            d_v.unsqueeze(3).broadcast_to([npart, C, 52, 2]),
            b52_v.unsqueeze(3).broadcast_to([npart, C, 52, 2]))

        # state update: W += d (x) ks  (paired views keep 2x mode)
        kspq = ks.rearrange("p c (j2 t) -> p c j2 t", t=2)  # [p, C, 8, 2]
        zu = wk.tile([npart, C * 576], BF, tag="sc_zu")
        zu_p = zu[:, :].rearrange("p (c g j2 t) -> p c g j2 t", c=C, j2=8, t=2)
        for c in range(C):
            nc.vector.tensor_mul(
                zu_p[:, c],
                d2x_v[:, c, 16:52].unsqueeze(2)
                .broadcast_to([npart, 36, 8, 2]),
                kspq[:, c].unsqueeze(1).broadcast_to([npart, 36, 8, 2]))
        nc.vector.tensor_add(stq[:, :], stq[:, :], zu[:, :])
        zuy = wk.tile([npart, C * 256], BF, tag="sc_zuy")
        zuy_p = zuy[:, :].rearrange("p (c g j2 t) -> p c g j2 t",
                                    c=C, j2=8, t=2)
        for c in range(C):
            nc.vector.tensor_mul(
                zuy_p[:, c],
                d2x_v[:, c, 0:16].unsqueeze(2)
                .broadcast_to([npart, 16, 8, 2]),
                kspq[:, c].unsqueeze(1).broadcast_to([npart, 16, 8, 2]))
        nc.vector.tensor_add(sty[:, :], sty[:, :], zuy[:, :])

    y_last = ys_all[:, :].rearrange("p (s c j) -> p s c j", s=S, c=C)[:, S - 1]
    nc.vector.tensor_reduce(y_last, prev_zy0, axis=AX.X, op=ALU.add)


def _ln(nc, wk, tag, npart, A1, A2, J, out_v, in_v, g_v, b_v):
    """LayerNorm over innermost J of 4-dim views [npart, A1, A2, J].

    rstd computed as exp(-0.5*ln(v+eps)) to stay in one act table set.
    """
    A = A1 * A2
    m = wk.tile([npart, A], F32, tag=f"ln_m_{tag}")
    v = wk.tile([npart, A], F32, tag=f"ln_v_{tag}")
    sq = wk.tile([npart, A * J], F32, tag="ln_sq")
    m4 = m[:, :].rearrange("p (a b) -> p a b", a=A1)
    v4 = v[:, :].rearrange("p (a b) -> p a b", a=A1)
    nc.vector.tensor_reduce(m4, in_v, axis=AX.X, op=ALU.add)
    nc.vector.tensor_scalar(m[:, :], m[:, :], -1.0 / J, None, ALU.mult)
    m_b = m4.unsqueeze(3).broadcast_to([npart, A1, A2, J])
    nc.vector.tensor_add(out_v, in_v, m_b)
    sq_v = sq[:, :].rearrange("p (a b j) -> p a b j", a=A1, b=A2)
    nc.vector.tensor_mul(sq_v, out_v, out_v)
    nc.vector.tensor_reduce(v4, sq_v, axis=AX.X, op=ALU.add)
    nc.vector.tensor_scalar(v[:, :], v[:, :], 1.0 / J, EPS, ALU.mult, ALU.add)
    nc.scalar.activation(out=v[:, :], in_=v[:, :], func=AF.Ln)
    nc.scalar.activation(out=v[:, :], in_=v[:, :], func=AF.Exp, scale=-0.5)
    v_b = v4.unsqueeze(3).broadcast_to([npart, A1, A2, J])
    nc.vector.tensor_mul(out_v, out_v, v_b)
    nc.vector.tensor_mul(out_v, out_v, g_v)
    nc.vector.tensor_add(out_v, out_v, b_v)


GELU_C = 1.5957691216057308  # 2*sqrt(2/pi)
GELU_A = 0.044715


def _gelu(nc, wk, out_v, pt_v, bias_ap, npart, F):
    """out = gelu_tanh(pt + bias); pt may be PSUM. [npart, F] views.

    sigmoid computed via exp to stay in one act table set.
    """
    xb = wk.tile([npart, F], F32, tag="gelu_xb")
    x2 = wk.tile([npart, F], F32, tag="gelu_x2")
    nc.scalar.activation(out=xb[:, :], in_=pt_v, func=AF.Identity,
                         bias=bias_ap, scale=1.0)
    nc.scalar.activation(out=x2[:, :], in_=pt_v, func=AF.Square,
                         bias=bias_ap, scale=1.0)
    nc.vector.tensor_scalar(x2[:, :], x2[:, :], GELU_A, 1.0, ALU.mult, ALU.add)
    nc.vector.tensor_mul(x2[:, :], x2[:, :], xb[:, :])
    nc.scalar.activation(out=x2[:, :], in_=x2[:, :], func=AF.Exp,
                         scale=-GELU_C)
    nc.vector.tensor_scalar(x2[:, :], x2[:, :], 1.0, None, ALU.add)
    nc.vector.reciprocal_approx_fast(x2[:, :], x2[:, :])
    nc.vector.tensor_mul(out_v, xb[:, :], x2[:, :])


def build_nc(debug=()):
    nc = bacc.Bacc(None, target_bir_lowering=False)
    dp = lambda nm, shp: nc.declare_dram_parameter(nm, shp, F32, isOutput=False)

    xinT = dp("xinT", [QIN, S * P])
    inW = dp("inW", [QIN, D])
    inb = dp("inb", [128, 2])
    I128 = dp("I128", [128, 128])
    MEAN = dp("MEAN", [128, 32])
    tkSy = dp("tkSy", [L * 128, 512])
    tkSq = dp("tkSq", [L * 128, 1152])
    chSy = dp("chSy", [L * 128, 512])
    chSq = dp("chSq", [L * 128, 1152])
    oSy = dp("oSy", [16, 256])
    oSq = dp("oSq", [16, 576])
    tkg = dp("tkg", [128, L * 16])
    tkb = dp("tkb", [128, L * 16])
    tkmg = dp("tkmg", [128, L * 256])
    tkmb = dp("tkmb", [128, L * 256])
    chg = dp("chg", [128, L * 256])
    chb = dp("chb", [128, L * 256])
    chmg = dp("chmg", [128, L * 256])
    chmb = dp("chmb", [128, L * 256])
    flng = dp("flng", [128, 256])
    flnb = dp("flnb", [128, 256])
    og = dp("og", [16, 256])
    ob = dp("ob", [16, 256])
    tkmW1 = dp("tkmW1", [16, L * 64])
    tkmB1 = dp("tkmB1", [64, L])
    tkmW2 = dp("tkmW2", [64, L * 16])
    tkmB2 = dp("tkmB2", [16, L])
    chmW1 = dp("chmW1", [128, L * 256])
    chmB1 = dp("chmB1", [128, L])
    chmW2 = dp("chmW2", [128, L * 256])
    chmB2 = dp("chmB2", [128, L * 2])
    outW = dp("outW", [128, 10])
    outB = dp("outB", [16, 5])
    out = nc.declare_dram_parameter("out", [S, NCLS], F32, isOutput=True)
    dbg = {}
    for nm, shp in debug:
        dbg[nm] = nc.declare_dram_parameter(nm, shp, F32, isOutput=True)

    with tile.TileContext(nc) as tc:
        with tc.tile_pool(name="cst", bufs=1) as cst, \
             tc.tile_pool(name="hb", bufs=1) as hb, \
             tc.tile_pool(name="wk", bufs=2) as wk, \
             tc.tile_pool(name="pst", bufs=4, space="PSUM") as pst, \
             tc.tile_pool(name="psm", bufs=2, space="PSUM") as psm, \
             tc.tile_pool(name="dr", bufs=1, space="DRAM") as dr:

            def load(tensor, shape, tag, sl=None):
                t = cst.tile(shape, F32, tag=tag)
                nc.gpsimd.dma_start(out=t, in_=tensor[:] if sl is None else sl)
                return t

            i_t = load(I128, [128, 128], "I128")
            mean_t = load(MEAN, [128, 32], "MEAN")
            tkg_t = load(tkg, [128, L * 16], "tkg")
            tkb_t = load(tkb, [128, L * 16], "tkb")
            tkmg_t = load(tkmg, [128, L * 256], "tkmg")
            tkmb_t = load(tkmb, [128, L * 256], "tkmb")
            chg_t = load(chg, [128, L * 256], "chg")
            chb_t = load(chb, [128, L * 256], "chb")
            chmg_t = load(chmg, [128, L * 256], "chmg")
            chmb_t = load(chmb, [128, L * 256], "chmb")
            flng_t = load(flng, [128, 256], "flng")
            flnb_t = load(flnb, [128, 256], "flnb")
            og_t = load(og, [16, 256], "og")
            ob_t = load(ob, [16, 256], "ob")
            tkmW1_t = load(tkmW1, [16, L * 64], "tkmW1")
            tkmB1_t = load(tkmB1, [64, L], "tkmB1")
            tkmW2_t = load(tkmW2, [64, L * 16], "tkmW2")
            tkmB2_t = load(tkmB2, [16, L], "tkmB2")
            chmW1_t = load(chmW1, [128, L * 256], "chmW1")
            chmB1_t = load(chmB1, [128, L], "chmB1")
            chmW2_t = load(chmW2, [128, L * 256], "chmW2")
            chmB2_t = load(chmB2, [128, L * 2], "chmB2")
            outW_t = load(outW, [128, 10], "outW")
            outB_t = load(outB, [16, 5], "outB")

            # ---- A. input projection -> hT (bf16) [128, (c, s*16+p)] ----
            xin_t = cst.tile([QIN, S * P], F32, tag="xin")
            w_t = cst.tile([QIN, D], F32, tag="inW")
            b_t = cst.tile([128, 2], F32, tag="inb")
            nc.gpsimd.dma_start(out=xin_t, in_=xinT[:])
            nc.gpsimd.dma_start(out=w_t, in_=inW[:])
            nc.gpsimd.dma_start(out=b_t, in_=inb[:])
            hT = hb.tile([128, 512], BF, tag="hT0")
            for c in range(2):
                pt = psm.tile([128, S * P], F32, tag="psm")
                nc.tensor.matmul(pt, w_t[:, c * 128:(c + 1) * 128], xin_t,
                                 start=True, stop=True)
                nc.scalar.activation(out=hT[:, c * 256:(c + 1) * 256], in_=pt,
                                     func=AF.Identity, bias=b_t[:, c:c + 1],
                                     scale=1.0)

            def t_to_sp(src, dst):
                """src [128, (c,sp)] hT-layout -> dst [128, (r,d)] SP-layout."""
                for c in range(2):
                    for r in range(2):
                        pt = pst.tile([128, 128], F32, tag="pst")
                        nc.tensor.transpose(
                            pt, src[:, c * 256 + r * 128:c * 256 + (r + 1) * 128],
                            i_t)
                        nc.scalar.copy(
                            out=dst[:, r * 256 + c * 128:r * 256 + (c + 1) * 128],
                            in_=pt)

            def t_to_ht(src, dst):
                for r in range(2):
                    for c in range(2):
                        pt = pst.tile([128, 128], F32, tag="pst")
                        nc.tensor.transpose(
                            pt, src[:, r * 256 + c * 128:r * 256 + (c + 1) * 128],
                            i_t)
                        nc.scalar.copy(
                            out=dst[:, c * 256 + r * 128:c * 256 + (r + 1) * 128],
                            in_=pt)

            def tap(nm, t):
                if nm in dbg:
                    nc.gpsimd.dma_start(out=dbg[nm][:], in_=t)

            def load_state(dram_y, dram_q, row0, npart, C):
                sty = hb.tile([npart, C * 256], F32, tag="sty_f")
                stq = hb.tile([npart, C * 576], BF, tag="stq")
                sgq = wk.tile([npart, C * 576], F32, tag="stg_q")
                nc.gpsimd.dma_start(out=sty,
                                    in_=dram_y[row0:row0 + npart, :])
                nc.gpsimd.dma_start(out=sgq, in_=dram_q[row0:row0 + npart, :])
                nc.vector.tensor_copy(stq[:, :], sgq[:, :])
                return sty, stq

            h_sp = None
            for i in range(L):
                # ---- B1. token SRWM ----
                sty, stq = load_state(tkSy, tkSq, i * 128, 128, 2)
                ys_tk = hb.tile([128, S * 2 * 16], F32, tag="ys")
                ht_cur = hT

                def x_tk(s, ht_cur=ht_cur):
                    return ht_cur[:, :].rearrange("p (c sp) -> p c sp", c=2)[
                        :, :, s * 16:(s + 1) * 16]

                _srwm_scan(nc, wk, 128, 2, sty, stq, x_tk, ys_tk)

                hT2 = hb.tile([128, 512], F32, tag="hT2")
                ys_v = ys_tk[:, :].rearrange("p (s c j) -> p c s j", s=S, c=2)
                out_v = hT2[:, :].rearrange("p (c s j) -> p c s j", c=2, j=16)
                g_v = tkg_t[:, i * 16:(i + 1) * 16].unsqueeze(1).unsqueeze(1) \
                    .broadcast_to([128, 2, 16, 16])
                b_v = tkb_t[:, i * 16:(i + 1) * 16].unsqueeze(1).unsqueeze(1) \
                    .broadcast_to([128, 2, 16, 16])
                _ln(nc, wk, "a", 128, 2, 16, 16, out_v, ys_v, g_v, b_v)
                tap(f"d_hT2_{i}", hT2)

                # ---- B2. token mixer ----
                h2 = hb.tile([128, 512], F32, tag="h2")
                t_to_sp(hT2, h2)
                hn = hb.tile([128, 512], F32, tag="hn")
                hn_v = hn[:, :].rearrange("p (a r d) -> p a r d", a=1, r=2)
                h2_v = h2[:, :].rearrange("p (a r d) -> p a r d", a=1, r=2)
                g_v = tkmg_t[:, i * 256:(i + 1) * 256].unsqueeze(1).unsqueeze(1) \
                    .broadcast_to([128, 1, 2, 256])
                b_v = tkmb_t[:, i * 256:(i + 1) * 256].unsqueeze(1).unsqueeze(1) \
                    .broadcast_to([128, 1, 2, 256])
                _ln(nc, wk, "b", 128, 1, 2, 256, hn_v, h2_v, g_v, b_v)
                drn = dr.tile([S, P, D], F32, tag="drn")
                for r in range(2):
                    nc.gpsimd.dma_start(
                        out=drn[r * 8:(r + 1) * 8, :, :],
                        in_=hn[:, r * 256:(r + 1) * 256])
                pmj = hb.tile([16, S * D], F32, tag="pmj")
                nc.gpsimd.dma_start(
                    out=pmj[:, :].rearrange("p (s d) -> p s d", s=S),
                    in_=drn[:, :, :].rearrange("s p d -> p s d"))
                gl = hb.tile([64, S * D], F32, tag="gl")
                for k in range(8):
                    pt1 = psm.tile([64, 512], F32, tag="psm")
                    nc.tensor.matmul(pt1, tkmW1_t[:, i * 64:(i + 1) * 64],
                                     pmj[:, k * 512:(k + 1) * 512],
                                     start=True, stop=True)
                    _gelu(nc, wk, gl[:, k * 512:(k + 1) * 512], pt1[:, :],
                          tkmB1_t[:, i:i + 1], 64, 512)
                mxo = hb.tile([16, S * D], F32, tag="mxo")
                for k in range(8):
                    pt2 = psm.tile([16, 512], F32, tag="psm")
                    nc.tensor.matmul(pt2, tkmW2_t[:, i * 16:(i + 1) * 16],
                                     gl[:, k * 512:(k + 1) * 512],
                                     start=True, stop=True)
                    nc.scalar.activation(out=mxo[:, k * 512:(k + 1) * 512],
                                         in_=pt2, func=AF.Identity,
                                         bias=tkmB2_t[:, i:i + 1], scale=1.0)
                dr3 = dr.tile([S, P, D], F32, tag="dr3")
                nc.gpsimd.dma_start(
                    out=dr3[:, :, :].rearrange("s p d -> p s d"),
                    in_=mxo[:, :].rearrange("p (s d) -> p s d", s=S))
                mxsp = hb.tile([128, 512], F32, tag="mxsp")
                for r in range(2):
                    nc.gpsimd.dma_start(
                        out=mxsp[:, r * 256:(r + 1) * 256],
                        in_=dr3[r * 8:(r + 1) * 8, :, :])
                h3 = hb.tile([128, 512], F32, tag="h3")
                nc.vector.tensor_add(h3[:, :], h2[:, :], mxsp[:, :])
                tap(f"d_h3_{i}", h3)

                # ---- B3. channel SRWM ----
                dr1 = dr.tile([S, P, H, DH], F32, tag="dr1")
                for r in range(2):
                    nc.gpsimd.dma_start(
                        out=dr1[r * 8:(r + 1) * 8, :, :, :].rearrange(
                            "s p hh j -> s p (hh j)"),
                        in_=h3[:, r * 256:(r + 1) * 256])
                xc = hb.tile([128, 512], F32, tag="xc")
                for c in range(2):
                    nc.gpsimd.dma_start(
                        out=xc[:, :].rearrange("q (c s j) -> q c s j",
                                               c=2, s=S)[:, c],
                        in_=dr1[:, c * 8:(c + 1) * 8, :, :].rearrange(
                            "s ph hh j -> (ph hh) s j"))
                xcb = hb.tile([128, 512], BF, tag="xcb")
                nc.vector.tensor_copy(xcb[:, :], xc[:, :])
                sty, stq = load_state(chSy, chSq, i * 128, 128, 2)
                ys_ch = hb.tile([128, S * 2 * 16], F32, tag="ys")

                def x_ch(s, xcb=xcb):
                    return xcb[:, :].rearrange("p (c s j) -> p c s j",
                                               c=2, s=S)[:, :, s, :]

                _srwm_scan(nc, wk, 128, 2, sty, stq, x_ch, ys_ch)

                dr2 = dr.tile([S, P, H, DH], F32, tag="dr2")
                for c in range(2):
                    nc.gpsimd.dma_start(
                        out=dr2[:, c * 8:(c + 1) * 8, :, :].rearrange(
                            "s ph hh i -> (ph hh) s i"),
                        in_=ys_ch[:, :].rearrange(
                            "q (s c j) -> q s c j", s=S, c=2)[:, :, c, :])
                ysp = hb.tile([128, 512], F32, tag="ysp")
                for r in range(2):
                    nc.gpsimd.dma_start(
                        out=ysp[:, r * 256:(r + 1) * 256],
                        in_=dr2[r * 8:(r + 1) * 8, :, :, :].rearrange(
                            "s p hh i -> s p (hh i)"))
                h4 = hb.tile([128, 512], F32, tag="h4")
                h4_v = h4[:, :].rearrange("p (a r d) -> p a r d", a=1, r=2)
                ysp_v = ysp[:, :].rearrange("p (a r d) -> p a r d", a=1, r=2)
                g_v = chg_t[:, i * 256:(i + 1) * 256].unsqueeze(1).unsqueeze(1) \
                    .broadcast_to([128, 1, 2, 256])
                b_v = chb_t[:, i * 256:(i + 1) * 256].unsqueeze(1).unsqueeze(1) \
                    .broadcast_to([128, 1, 2, 256])
                _ln(nc, wk, "c", 128, 1, 2, 256, h4_v, ysp_v, g_v, b_v)
                tap(f"d_h4_{i}", h4)

                # ---- B4. channel mixer ----
                hn2 = hb.tile([128, 512], F32, tag="hn2")
                hn2_v = hn2[:, :].rearrange("p (a r d) -> p a r d", a=1, r=2)
                g_v = chmg_t[:, i * 256:(i + 1) * 256].unsqueeze(1).unsqueeze(1) \
                    .broadcast_to([128, 1, 2, 256])
                b_v = chmb_t[:, i * 256:(i + 1) * 256].unsqueeze(1).unsqueeze(1) \
                    .broadcast_to([128, 1, 2, 256])
                _ln(nc, wk, "d", 128, 1, 2, 256, hn2_v, h4_v, g_v, b_v)
                hn2T = hb.tile([128, 512], F32, tag="hn2T")
                t_to_ht(hn2, hn2T)
                pt1 = psm.tile([128, 256], F32, tag="psm")
                for c in range(2):
                    nc.tensor.matmul(
                        pt1, chmW1_t[:, i * 256 + c * 128:i * 256 + (c + 1) * 128],
                        hn2T[:, c * 256:(c + 1) * 256],
                        start=(c == 0), stop=(c == 1))
                gl2 = hb.tile([128, 256], F32, tag="gl2")
                _gelu(nc, wk, gl2[:, :], pt1[:, :], chmB1_t[:, i:i + 1],
                      128, 256)
                moT = hb.tile([128, 512], F32, tag="moT")
                for c in range(2):
                    pt2 = psm.tile([128, 256], F32, tag="psm")
                    nc.tensor.matmul(
                        pt2, chmW2_t[:, i * 256 + c * 128:i * 256 + (c + 1) * 128],
                        gl2, start=True, stop=True)
                    nc.scalar.activation(out=moT[:, c * 256:(c + 1) * 256],
                                         in_=pt2, func=AF.Identity,
                                         bias=chmB2_t[:, i * 2 + c:i * 2 + c + 1],
                                         scale=1.0)
                mosp = hb.tile([128, 512], F32, tag="mosp")
                t_to_sp(moT, mosp)
                h5 = hb.tile([128, 512], F32, tag="h5")
                nc.vector.tensor_add(h5[:, :], h4[:, :], mosp[:, :])
                h_sp = h5
                tap(f"d_h5_{i}", h5)

                if i + 1 < L:
                    hT = hb.tile([128, 512], BF, tag="hT0")
                    t_to_ht(h_sp, hT)

            # ---- C. final ----
            hf = hb.tile([128, 512], F32, tag="hf")
            hf_v = hf[:, :].rearrange("p (a r d) -> p a r d", a=1, r=2)
            hsp_v = h_sp[:, :].rearrange("p (a r d) -> p a r d", a=1, r=2)
            g_v = flng_t[:, :].unsqueeze(1).unsqueeze(1) \
                .broadcast_to([128, 1, 2, 256])
            b_v = flnb_t[:, :].unsqueeze(1).unsqueeze(1) \
                .broadcast_to([128, 1, 2, 256])
            _ln(nc, wk, "f", 128, 1, 2, 256, hf_v, hsp_v, g_v, b_v)
            pm = psm.tile([16, 256], F32, tag="psm")
            for r in range(2):
                nc.tensor.matmul(pm, mean_t[:, r * 16:(r + 1) * 16],
                                 hf[:, r * 256:(r + 1) * 256],
                                 start=(r == 0), stop=(r == 1))
            ho = hb.tile([16, 256], F32, tag="ho")
            nc.scalar.copy(out=ho, in_=pm)
            tap("d_ho", ho)
            dr4 = dr.tile([S, H, DH], F32, tag="dr4")
            nc.gpsimd.dma_start(
                out=dr4[:, :, :],
                in_=ho[:, :].rearrange("s (hh j) -> s hh j", hh=H))
            xo = hb.tile([16, S * DH], F32, tag="xo")
            nc.gpsimd.dma_start(
                out=xo[:, :].rearrange("p (s j) -> p s j", s=S),
                in_=dr4[:, :, :].rearrange("s hh j -> hh s j"))
            xob = hb.tile([16, S * DH], BF, tag="xob")
            nc.vector.tensor_copy(xob[:, :], xo[:, :])
            sty, stq = load_state(oSy, oSq, 0, 16, 1)
            ys_o = hb.tile([16, S * 16], F32, tag="ys_o")

            def x_o(s, xob=xob):
                return xob[:, :].rearrange("p (c s j) -> p c s j", c=1, s=S)[
                    :, :, s, :]

            _srwm_scan(nc, wk, 16, 1, sty, stq, x_o, ys_o, use_gp=False)

            dr5 = dr.tile([H, S, DH], F32, tag="dr5")
            nc.gpsimd.dma_start(
                out=dr5[:, :, :],
                in_=ys_o[:, :].rearrange("p (s j) -> p s j", s=S))
            ho2 = hb.tile([16, 256], F32, tag="ho2")
            nc.gpsimd.dma_start(
                out=ho2[:, :].rearrange("s (hh i) -> s hh i", hh=H),
                in_=dr5[:, :, :].rearrange("hh s i -> s hh i"))
            hon = hb.tile([16, 256], F32, tag="hon")
            _ln(nc, wk, "o", 16, 1, 1, 256,
                hon[:, :].unsqueeze(1).unsqueeze(1),
                ho2[:, :].unsqueeze(1).unsqueeze(1),
                og_t[:, :].unsqueeze(1).unsqueeze(1),
                ob_t[:, :].unsqueeze(1).unsqueeze(1))
            hoT = hb.tile([128, 32], F32, tag="hoT")
            for c in range(2):
                pt = pst.tile([128, 128], F32, tag="pst")
                nc.tensor.transpose(pt[:, 0:16], hon[:, c * 128:(c + 1) * 128],
                                    i_t[0:16, 0:16])
                nc.scalar.copy(out=hoT[:, c * 16:(c + 1) * 16], in_=pt[:, 0:16])
            po = psm.tile([16, 5], F32, tag="psm")
            for c in range(2):
                nc.tensor.matmul(po, hoT[:, c * 16:(c + 1) * 16],
                                 outW_t[:, c * 5:(c + 1) * 5],
                                 start=(c == 0), stop=(c == 1))
            fin = hb.tile([16, 5], F32, tag="fin")
            nc.vector.tensor_add(fin[:, :], po[:, :], outB_t[:, :])
            nc.gpsimd.dma_start(out=out[:], in_=fin)

    nc.finalize()
    return nc


# ------------------------- host marshaling -------------------------

def _patchify(x):
    s, bb, c, hh, ww = x.shape
    h, w = hh // PS, ww // PS
    x = x.reshape(s, bb, c, h, PS, w, PS)
    x = x.transpose(0, 1, 3, 5, 4, 6, 2)
    return x.reshape(s, bb, h * w, PS * PS * c)


def _state_init(Wy, Wq, Wk, wb, pair_of, npart=128, C=2):
    sy = np.zeros((npart, C, 16, 16), np.float32)
    sq = np.zeros((npart, C, 36, 16), np.float32)
    for c in range(C):
        for q in range(npart):
            h = pair_of(c, q)
            sy[q, c] = Wy[h]
            sq[q, c, 0:16] = Wq[h]
            sq[q, c, 16:32] = Wk[h]
            # wb rows stored negated: seg_b = -logit, so sigmoid(logit)
            # = 1/(1+exp(seg_b)) shares the single per-step exp, and the
            # negated rows are self-consistent under the delta update.
            sq[q, c, 32:36] = -wb[h]
    return sy.reshape(npart, C * 256), sq.reshape(npart, C * 576)


def marshal(ins):
    """Returns (xin (S,B,P,QIN), shared input dict)."""
    x, fb = ins["x"].astype(np.float32), ins["fb"]
    xp = _patchify(x)
    emb = np.zeros((S, B, NCLS), np.float32)
    emb[np.arange(S)[:, None], np.arange(B)[None, :], fb] = 1.0
    emb = np.broadcast_to(emb[:, :, None, :], (S, B, P, NCLS))
    xin = np.concatenate([xp, emb], -1)

    f32 = lambda k: np.asarray(ins[k], np.float32)
    sh = {}
    sh["inW"] = np.ascontiguousarray(f32("in_W"))
    sh["inb"] = f32("in_b").reshape(2, 128).T.copy()
    sh["I128"] = np.eye(128, dtype=np.float32)
    mean = np.zeros((128, 32), np.float32)
    for r in range(2):
        for sp in range(128):
            s = r * 8 + sp // 16
            mean[sp, r * 16 + s] = 1.0 / 16.0
    sh["MEAN"] = mean
    tky, tkq = zip(*[
        _state_init(f32("tk_Wy")[i], f32("tk_Wq")[i], f32("tk_Wk")[i],
                    f32("tk_wb")[i], lambda c, q: 0) for i in range(L)])
    sh["tkSy"] = np.concatenate(tky, 0)
    sh["tkSq"] = np.concatenate(tkq, 0)
    chy, chq = zip(*[
        _state_init(f32("ch_Wy")[i], f32("ch_Wq")[i], f32("ch_Wk")[i],
                    f32("ch_wb")[i], lambda c, q: q % 16) for i in range(L)])
    sh["chSy"] = np.concatenate(chy, 0)
    sh["chSq"] = np.concatenate(chq, 0)
    oy, oq = _state_init(f32("o_Wy"), f32("o_Wq"), f32("o_Wk"), f32("o_wb"),
                         lambda c, q: q, npart=16, C=1)
    sh["oSy"] = oy
    sh["oSq"] = oq
    rep = lambda a, n=128: np.broadcast_to(
        np.asarray(a, np.float32).reshape(1, -1),
        (n, np.asarray(a).size)).copy()
    sh["tkg"] = rep(f32("tk_lng"))
    sh["tkb"] = rep(f32("tk_lnb"))
    sh["tkmg"] = rep(f32("tkm_g"))
    sh["tkmb"] = rep(f32("tkm_b"))
    sh["chg"] = rep(f32("ch_lng"))
    sh["chb"] = rep(f32("ch_lnb"))
    sh["chmg"] = rep(f32("chm_g"))
    sh["chmb"] = rep(f32("chm_b"))
    sh["flng"] = rep(f32("fln_g"))
    sh["flnb"] = rep(f32("fln_b"))
    sh["og"] = rep(f32("o_lng"), 16)
    sh["ob"] = rep(f32("o_lnb"), 16)
    sh["tkmW1"] = np.concatenate([f32("tkm_W1")[i] for i in range(L)], 1)
    sh["tkmB1"] = np.stack([f32("tkm_b1")[i] for i in range(L)], 1)
    sh["tkmW2"] = np.concatenate([f32("tkm_W2")[i] for i in range(L)], 1)
    sh["tkmB2"] = np.stack([f32("tkm_b2")[i] for i in range(L)], 1)
    # chmW1[i] is (D=256, DFT=128); lhsT chunk c = chm_W1[i][c*128:(c+1)*128, :]
    sh["chmW1"] = np.concatenate(
        [f32("chm_W1")[i][c * 128:(c + 1) * 128, :]
         for i in range(L) for c in range(2)], 1)
    sh["chmB1"] = np.stack([f32("chm_b1")[i] for i in range(L)], 1)
    # chmW2[i] is (DFT=128, D=256); lhsT chunk c = chm_W2[i][:, c*128:(c+1)*128]
    sh["chmW2"] = np.concatenate(
        [f32("chm_W2")[i][:, c * 128:(c + 1) * 128]
         for i in range(L) for c in range(2)], 1)
    # chmB2: bias per d; chunk c column holds b2[c*128:(c+1)*128]
    sh["chmB2"] = np.stack(
        [f32("chm_b2")[i][c * 128:(c + 1) * 128]
         for i in range(L) for c in range(2)], 1)
    sh["outW"] = np.concatenate(
        [f32("out_W")[c * 128:(c + 1) * 128, :] for c in range(2)], 1)
    sh["outB"] = rep(f32("out_b"), 16)
    return xin, sh


def in_maps_for(xin, sh):
    maps = []
    for b in range(B):
        m = dict(sh)
        m["xinT"] = np.ascontiguousarray(
            xin[:, b].reshape(S * P, QIN).T)
        maps.append(m)
    return maps


from concourse.bass_utils import run_bass_kernel_spmd

_CACHE = {}


def kernel(**inputs):
    ins = {k: np.ascontiguousarray(np.asarray(v)) for k, v in inputs.items()}
    if "nc" not in _CACHE:
        _CACHE["nc"] = build_nc()
    nc = _CACHE["nc"]
    xin, sh = marshal(ins)
    maps = in_maps_for(xin, sh)
    res = run_bass_kernel_spmd(nc, maps, core_ids=list(range(8)))
    out = np.stack([res.results[c]["out"] for c in range(B)], axis=1)
    return out.astype(np.float32)


# revision 20
# speedup vs baseline: 1.6444x; 1.0893x over previous
"""Trainium2 Bass kernel for nn_CompatStatefulSelfModMixerModel.

Fully on-device: input projection, 2x (token SRWM scan + token mixer +
channel SRWM scan + channel mixer), final LN + patch-mean, output SRWM,
linear head - one Bass program per core. Data-parallel over batch B=8
across 8 NeuronCores (1 sample/core, weights replicated, no collectives).

Scan fast-math: fast-weight state kept in bf16 (DVE 2x_1p mode for all
big tensor_tensor ops), state split into y-rows (gpsimd-updated) and
q/k/beta-rows (vector-updated), softmax without max-subtraction, single
activation table set (rsqrt via exp(-0.5*ln(v)), sigmoid via exp).
"""
import sys

sys.path.insert(0, "/opt/trn_rl_repo")

import numpy as np

import concourse.bacc as bacc
import concourse.tile as tile
from concourse import mybir
from concourse import hw_specs as _hw

# Route every activation (Exp/Ln/Identity/Square/Copy) to the one table
# set containing them all, so the program needs a single ACT_TABLE_LOAD.
# Sets earlier in act_info.json order are emptied (indices preserved) so
# first-match lands on natural_log_exp_and_others.
_orig_gat = _hw.get_activation_tables


def _patched_gat(arch):
    t = _orig_gat(arch)
    shadow = ("exp_and_others", "softplus_and_others", "sigmoid_and_others",
              "sqrt_and_others", "small", "natural_log")
    return {k: (set() if k in shadow else v) for k, v in t.items()}


bacc.get_activation_tables = _patched_gat

F32 = mybir.dt.float32
BF = mybir.dt.bfloat16
AF = mybir.ActivationFunctionType
ALU = mybir.AluOpType
AX = mybir.AxisListType

S, B, NCLS = 16, 8, 5
D, H, DH = 256, 16, 16
PS, IMG = 7, 28
P = 16
L = 2
PD = 49
QIN = PD + NCLS  # 54
DFT = 128
EPS = 1e-5


def _srwm_scan(nc, wk, npart, C, sty, stq, x_of_step, ys_all,
               use_gp=True):
    """S steps of the SRWM recurrence.

    sty: f32 [npart, C*256] - Wy rows (muls on gpsimd, add on vector)
    stq: bf16 [npart, C*576] - Wq/Wk/wb rows, viewed [p, c, g, 16]
         (0:16 Wq, 16:32 Wk, 32:36 wb)
    x_of_step(s) -> bf16 AP [npart, C, 16]
    ys_all: fp32 [npart, S*C*16]; y_t lands at [:, s, c, :].
    """
    styf_v = sty[:, :].rearrange("p (c i j) -> p c i j", c=C, j=16)
    stq_v = stq[:, :].rearrange("p (c g j) -> p c g j", c=C, j=16)
    GE = nc.gpsimd if use_gp else nc.vector
    wide = npart == 128  # pairwise presums / bf16 mirror only pay off wide
    prev_zy0 = None

    for s in range(S):
        xt = x_of_step(s)  # [p, C, 16] bf16
        # y path: y_t = Wy . x in f32 on gpsimd; the reduce is deferred
        # to the next step so gpsimd has a full step of slack.
        zy0 = wk.tile([npart, C * 256], F32, tag="sc_zy0")
        zy0_v = zy0[:, :].rearrange("p (c i j) -> p c i j", c=C, j=16)
        GE.tensor_mul(zy0_v, styf_v,
                      xt.unsqueeze(2).broadcast_to([npart, C, 16, 16]))
        if prev_zy0 is not None:
            y_prev = ys_all[:, :].rearrange("p (s c j) -> p s c j",
                                            s=S, c=C)[:, s - 1]
            nc.vector.tensor_reduce(y_prev, prev_zy0, axis=AX.X, op=ALU.add)
        prev_zy0 = zy0_v

        # bf16 mirror of Wy for the vy-path muls
        if wide:
            styb = wk.tile([npart, C * 256], BF, tag="sc_styb")
            nc.vector.tensor_copy(styb[:, :], sty[:, :])
            sty_v = styb[:, :].rearrange("p (c i j) -> p c i j", c=C, j=16)
        else:
            sty_v = styf_v

        # seg = [q; k; -b-logits] = stq . x (wb rows stored negated)
        zq = wk.tile([npart, C * 576], BF, tag="sc_zq")
        zq_v = zq[:, :].rearrange("p (c g j) -> p c g j", c=C, j=16)
        nc.vector.tensor_mul(zq_v, stq_v,
                             xt.unsqueeze(2).broadcast_to([npart, C, 36, 16]))
        seg = wk.tile([npart, C * 36], F32, tag="sc_seg")
        seg_v = seg[:, :].rearrange("p (c g) -> p c g", c=C)
        if wide:
            zqh = wk.tile([npart, C * 288], BF, tag="sc_zqh")
            zqh_v = zqh[:, :].rearrange("p (c g h) -> p c g h", c=C, h=8)
            nc.vector.tensor_add(zqh_v, zq_v[:, :, :, 0:8],
                                 zq_v[:, :, :, 8:16])
            nc.vector.tensor_reduce(seg_v, zqh_v, axis=AX.X, op=ALU.add)
        else:
            nc.vector.tensor_reduce(seg_v, zq_v, axis=AX.X, op=ALU.add)

        # one exp for q, k and (negated) beta logits
        es = wk.tile([npart, C * 36], F32, tag="sc_es")
        es_v = es[:, :].rearrange("p (c g) -> p c g", c=C)
        nc.scalar.activation(out=es_v, in_=seg_v, func=AF.Exp)
        eqk_v = es[:, :].rearrange("p (c g) -> p c g", c=C)[:, :, 0:32] \
            .rearrange("p c (t j) -> p c t j", t=2)

        sums = wk.tile([npart, C * 2], F32, tag="sc_sums")
        sums_v = sums[:, :].rearrange("p (c t) -> p c t", c=C)
        nc.vector.tensor_reduce(sums_v, eqk_v, axis=AX.X, op=ALU.add)
        rec = wk.tile([npart, C * 2], F32, tag="sc_rec")
        nc.vector.reciprocal_approx_fast(rec[:, :], sums[:, :])
        rec_v = rec[:, :].rearrange("p (c t) -> p c t", c=C)
        kq = wk.tile([npart, C * 32], BF, tag="sc_kq")
        kq_v = kq[:, :].rearrange("p (c t j) -> p c t j", c=C, t=2)
        nc.vector.tensor_mul(kq_v, eqk_v,
                             rec_v.unsqueeze(3).broadcast_to([npart, C, 2, 16]))
        qs = kq_v[:, :, 0]
        ks = kq_v[:, :, 1]
        e = wk.tile([npart, C * 16], BF, tag="sc_e")
        e_v = e[:, :].rearrange("p (c j) -> p c j", c=C)
        nc.vector.tensor_sub(e_v, qs, ks)

        # beta = 1 / (1 + exp(-logit))  (exp already in es rows 32:36)
        bta = wk.tile([npart, C * 4], F32, tag="sc_beta")
        bta_v = bta[:, :].rearrange("p (c w) -> p c w", c=C)
        nc.vector.tensor_scalar(bta_v, es_v[:, :, 32:36], 1.0, None, ALU.add)
        nc.vector.reciprocal_approx_fast(bta[:, :], bta[:, :])

        # d rows 16:52 = stq . (qs - ks)
        z2 = wk.tile([npart, C * 576], BF, tag="sc_z2")
        z2_v = z2[:, :].rearrange("p (c g j) -> p c g j", c=C, j=16)
        nc.vector.tensor_mul(z2_v, stq_v,
                             e_v.unsqueeze(2).broadcast_to([npart, C, 36, 16]))
        d = wk.tile([npart, C * 52], F32, tag="sc_d")
        d_v = d[:, :].rearrange("p (c g) -> p c g", c=C)
        if wide:
            z2h = wk.tile([npart, C * 288], BF, tag="sc_z2h")
            z2h_v = z2h[:, :].rearrange("p (c g h) -> p c g h", c=C, h=8)
            nc.vector.tensor_add(z2h_v, z2_v[:, :, :, 0:8],
                                 z2_v[:, :, :, 8:16])
            nc.vector.tensor_reduce(d_v[:, :, 16:52], z2h_v,
                                    axis=AX.X, op=ALU.add)
        else:
            nc.vector.tensor_reduce(d_v[:, :, 16:52], z2_v,
                                    axis=AX.X, op=ALU.add)

        # vy over both qs and ks: vykq[c,t,i] = sum_j Wy[c,i,j]*kq[c,t,j]
        zy = wk.tile([npart, C * 512], BF, tag="sc_zy")
        zy_v = zy[:, :].rearrange("p (c t i j) -> p c t i j", c=C, t=2, j=16)
        for t in range(2):
            nc.vector.tensor_mul(
                zy_v[:, :, t], sty_v,
                kq_v[:, :, t].unsqueeze(2).broadcast_to([npart, C, 16, 16]))
        vykq = wk.tile([npart, C * 32], F32, tag="sc_vykq")
        vykq_v = vykq[:, :].rearrange("p (c t i) -> p c t i", c=C, t=2)
        zy_f = zy[:, :].rearrange("p (a j) -> p a j", j=16)
        if wide:
            zyh = wk.tile([npart, C * 256], BF, tag="sc_zyh")
            zyh_v = zyh[:, :].rearrange("p (a h) -> p a h", h=8)
            nc.vector.tensor_add(zyh_v, zy_f[:, :, 0:8], zy_f[:, :, 8:16])
            nc.vector.tensor_reduce(vykq[:, :], zyh_v, axis=AX.X, op=ALU.add)
        else:
            nc.vector.tensor_reduce(vykq[:, :], zy_f, axis=AX.X, op=ALU.add)

        # v-softmax on vy_q; d rows 0:16 = softmax(vy_q) - vy_k
        ev = wk.tile([npart, C * 16], F32, tag="sc_ev")
        ev_v = ev[:, :].rearrange("p (c i) -> p c i", c=C)
        nc.scalar.activation(out=ev_v, in_=vykq_v[:, :, 0], func=AF.Exp)

        # fill the exp wait: expand beta to per-row b52 (needs only bta)
        b52 = wk.tile([npart, C * 52], F32, tag="sc_b52")
        b52_v = b52[:, :].rearrange("p (c g) -> p c g", c=C)
        nc.vector.tensor_scalar(
            b52_v[:, :, 0:48].rearrange("p c (w g) -> p c w g", g=16),
            bta_v[:, :, 0:3].unsqueeze(3).broadcast_to([npart, C, 3, 16]),
            1.0, None, ALU.mult)
        nc.vector.tensor_scalar(
            b52_v[:, :, 48:52],
            bta_v[:, :, 3:4].broadcast_to([npart, C, 4]),
            1.0, None, ALU.mult)

        vs = wk.tile([npart, C], F32, tag="sc_vs")
        nc.vector.tensor_reduce(vs[:, :], ev_v, axis=AX.X, op=ALU.add)
        nc.vector.reciprocal_approx_fast(vs[:, :], vs[:, :])
        for c in range(C):
            nc.vector.scalar_tensor_tensor(
                out=d_v[:, c, 0:16], in0=ev_v[:, c], scalar=vs[:, c:c + 1],
                in1=vykq_v[:, c, 1], op0=ALU.mult, op1=ALU.subtract)

        d2x = wk.tile([npart, C * 104], BF, tag="sc_d2x")
        d2x_v = d2x[:, :].rearrange("p (c g t) -> p c g t", c=C, t=2)
        if C > 1:
            nc.vector.tensor_mul(
                d2x_v,
                d_v.unsqueeze(3).broadcast_to([npart, C, 52, 2]),
                b52_v.unsqueeze(3).broadcast_to([npart, C, 52, 2]))
        else:
            nc.vector.tensor_mul(
                d2x_v[:, 0, 0:48].rearrange("p (w g) t -> p w g t", g=16),
                d_v[:, 0, 0:48].rearrange("p (w g) -> p w g", g=16)
                .unsqueeze(3).broadcast_to([npart, 3, 16, 2]),
                bta_v[:, 0, 0:3].unsqueeze(2).unsqueeze(3)
                .broadcast_to([npart, 3, 16, 2]))
            nc.vector.tensor_mul(
                d2x_v[:, 0, 48:52],
                d_v[:, 0, 48:52].unsqueeze(2).broadcast_to([npart, 4, 2]),
                bta_v[:, 0, 3:4].unsqueeze(2).broadcast_to([npart, 4, 2]))

        # state update: W += d (x) ks  (paired views keep 2x mode)
        kspq = ks.rearrange("p c (j2 t) -> p c j2 t", t=2)  # [p, C, 8, 2]
        zu = wk.tile([npart, C * 576], BF, tag="sc_zu")
        zu_p = zu[:, :].rearrange("p (c g j2 t) -> p c g j2 t", c=C, j2=8, t=2)
        for c in range(C):
            nc.vector.tensor_mul(
                zu_p[:, c],
                d2x_v[:, c, 16:52].unsqueeze(2)
                .broadcast_to([npart, 36, 8, 2]),
                kspq[:, c].unsqueeze(1).broadcast_to([npart, 36, 8, 2]))
        nc.vector.tensor_add(stq[:, :], stq[:, :], zu[:, :])
        zuy = wk.tile([npart, C * 256], BF, tag="sc_zuy")
        zuy_p = zuy[:, :].rearrange("p (c g j2 t) -> p c g j2 t",
                                    c=C, j2=8, t=2)
        for c in range(C):
            nc.vector.tensor_mul(
                zuy_p[:, c],
                d2x_v[:, c, 0:16].unsqueeze(2)
                .broadcast_to([npart, 16, 8, 2]),
                kspq[:, c].unsqueeze(1).broadcast_to([npart, 16, 8, 2]))
        nc.vector.tensor_add(sty[:, :], sty[:, :], zuy[:, :])

    y_last = ys_all[:, :].rearrange("p (s c j) -> p s c j", s=S, c=C)[:, S - 1]
    nc.vector.tensor_reduce(y_last, prev_zy0, axis=AX.X, op=ALU.add)


def _ln(nc, wk, tag, npart, A1, A2, J, out_v, in_v, g_v, b_v):
    """LayerNorm over innermost J of 4-dim views [npart, A1, A2, J].

    rstd computed as exp(-0.5*ln(v+eps)) to stay in one act table set.
    """
    A = A1 * A2
    m = wk.tile([npart, A], F32, tag=f"ln_m_{tag}")
    v = wk.tile([npart, A], F32, tag=f"ln_v_{tag}")
    sq = wk.tile([npart, A * J], F32, tag="ln_sq")
    m4 = m[:, :].rearrange("p (a b) -> p a b", a=A1)
    v4 = v[:, :].rearrange("p (a b) -> p a b", a=A1)
    nc.vector.tensor_reduce(m4, in_v, axis=AX.X, op=ALU.add)
    nc.vector.tensor_scalar(m[:, :], m[:, :], -1.0 / J, None, ALU.mult)
    m_b = m4.unsqueeze(3).broadcast_to([npart, A1, A2, J])
    nc.vector.tensor_add(out_v, in_v, m_b)
    sq_v = sq[:, :].rearrange("p (a b j) -> p a b j", a=A1, b=A2)
    nc.vector.tensor_mul(sq_v, out_v, out_v)
    nc.vector.tensor_reduce(v4, sq_v, axis=AX.X, op=ALU.add)
    nc.vector.tensor_scalar(v[:, :], v[:, :], 1.0 / J, EPS, ALU.mult, ALU.add)
    nc.scalar.activation(out=v[:, :], in_=v[:, :], func=AF.Ln)
    nc.scalar.activation(out=v[:, :], in_=v[:, :], func=AF.Exp, scale=-0.5)
    v_b = v4.unsqueeze(3).broadcast_to([npart, A1, A2, J])
    nc.vector.tensor_mul(out_v, out_v, v_b)
    nc.vector.tensor_mul(out_v, out_v, g_v)
    nc.vector.tensor_add(out_v, out_v, b_v)


GELU_C = 1.5957691216057308  # 2*sqrt(2/pi)
GELU_A = 0.044715


def _gelu(nc, wk, out_v, pt_v, bias_ap, npart, F):
    """out = gelu_tanh(pt + bias); pt may be PSUM. [npart, F] views.

    sigmoid computed via exp to stay in one act table set.
    """
    xb = wk.tile([npart, F], F32, tag="gelu_xb")
    x2 = wk.tile([npart, F], F32, tag="gelu_x2")
    nc.scalar.activation(out=xb[:, :], in_=pt_v, func=AF.Identity,
                         bias=bias_ap, scale=1.0)
    nc.scalar.activation(out=x2[:, :], in_=pt_v, func=AF.Square,
                         bias=bias_ap, scale=1.0)
    nc.vector.tensor_scalar(x2[:, :], x2[:, :], GELU_A, 1.0, ALU.mult, ALU.add)
    nc.vector.tensor_mul(x2[:, :], x2[:, :], xb[:, :])
    nc.scalar.activation(out=x2[:, :], in_=x2[:, :], func=AF.Exp,
                         scale=-GELU_C)
    nc.vector.tensor_scalar(x2[:, :], x2[:, :], 1.0, None, ALU.add)
    nc.vector.reciprocal_approx_fast(x2[:, :], x2[:, :])
    nc.vector.tensor_mul(out_v, xb[:, :], x2[:, :])


def build_nc(debug=()):
    nc = bacc.Bacc(None, target_bir_lowering=False)
    dp = lambda nm, shp: nc.declare_dram_parameter(nm, shp, F32, isOutput=False)

    xinT = dp("xinT", [QIN, S * P])
    inW = dp("inW", [QIN, D])
    inb = dp("inb", [128, 2])
    I128 = dp("I128", [128, 128])
    MEAN = dp("MEAN", [128, 32])
    tkSy = dp("tkSy", [L * 128, 512])
    tkSq = dp("tkSq", [L * 128, 1152])
    chSy = dp("chSy", [L * 128, 512])
    chSq = dp("chSq", [L * 128, 1152])
    oSy = dp("oSy", [16, 256])
    oSq = dp("oSq", [16, 576])
    tkg = dp("tkg", [128, L * 16])
    tkb = dp("tkb", [128, L * 16])
    tkmg = dp("tkmg", [128, L * 256])
    tkmb = dp("tkmb", [128, L * 256])
    chg = dp("chg", [128, L * 256])
    chb = dp("chb", [128, L * 256])
    chmg = dp("chmg", [128, L * 256])
    chmb = dp("chmb", [128, L * 256])
    flng = dp("flng", [128, 256])
    flnb = dp("flnb", [128, 256])
    og = dp("og", [16, 256])
    ob = dp("ob", [16, 256])
    tkmW1B = dp("tkmW1B", [128, L * 512])
    tkmW2B = dp("tkmW2B", [128, L * 512])
    tkmB1c = dp("tkmB1c", [128, L])
    tkmB2c = dp("tkmB2c", [128, L])
    chmW1 = dp("chmW1", [128, L * 256])
    chmB1 = dp("chmB1", [128, L])
    chmW2 = dp("chmW2", [128, L * 256])
    chmB2 = dp("chmB2", [128, L * 2])
    outW = dp("outW", [128, 10])
    outB = dp("outB", [16, 5])
    out = nc.declare_dram_parameter("out", [S, NCLS], F32, isOutput=True)
    dbg = {}
    for nm, shp in debug:
        dbg[nm] = nc.declare_dram_parameter(nm, shp, F32, isOutput=True)

    with tile.TileContext(nc) as tc:
        with tc.tile_pool(name="cst", bufs=1) as cst, \
             tc.tile_pool(name="hb", bufs=1) as hb, \
             tc.tile_pool(name="wk", bufs=2) as wk, \
             tc.tile_pool(name="pst", bufs=4, space="PSUM") as pst, \
             tc.tile_pool(name="psm", bufs=2, space="PSUM") as psm, \
             tc.tile_pool(name="dr", bufs=1, space="DRAM") as dr:

            def load(tensor, shape, tag, sl=None):
                t = cst.tile(shape, F32, tag=tag)
                nc.sync.dma_start(out=t, in_=tensor[:] if sl is None else sl)
                return t

            i_t = load(I128, [128, 128], "I128")
            mean_t = load(MEAN, [128, 32], "MEAN")
            tkg_t = load(tkg, [128, L * 16], "tkg")
            tkb_t = load(tkb, [128, L * 16], "tkb")
            tkmg_t = load(tkmg, [128, L * 256], "tkmg")
            tkmb_t = load(tkmb, [128, L * 256], "tkmb")
            chg_t = load(chg, [128, L * 256], "chg")
            chb_t = load(chb, [128, L * 256], "chb")
            chmg_t = load(chmg, [128, L * 256], "chmg")
            chmb_t = load(chmb, [128, L * 256], "chmb")
            flng_t = load(flng, [128, 256], "flng")
            flnb_t = load(flnb, [128, 256], "flnb")
            og_t = load(og, [16, 256], "og")
            ob_t = load(ob, [16, 256], "ob")
            tkmW1B_t = load(tkmW1B, [128, L * 512], "tkmW1B")
            tkmW2B_t = load(tkmW2B, [128, L * 512], "tkmW2B")
            tkmB1c_t = load(tkmB1c, [128, L], "tkmB1c")
            tkmB2c_t = load(tkmB2c, [128, L], "tkmB2c")
            chmW1_t = load(chmW1, [128, L * 256], "chmW1")
            chmB1_t = load(chmB1, [128, L], "chmB1")
            chmW2_t = load(chmW2, [128, L * 256], "chmW2")
            chmB2_t = load(chmB2, [128, L * 2], "chmB2")
            outW_t = load(outW, [128, 10], "outW")
            outB_t = load(outB, [16, 5], "outB")

            # ---- A. input projection -> hT (bf16) [128, (c, s*16+p)] ----
            xin_t = cst.tile([QIN, S * P], F32, tag="xin")
            w_t = cst.tile([QIN, D], F32, tag="inW")
            b_t = cst.tile([128, 2], F32, tag="inb")
            nc.gpsimd.dma_start(out=xin_t, in_=xinT[:])
            nc.gpsimd.dma_start(out=w_t, in_=inW[:])
            nc.gpsimd.dma_start(out=b_t, in_=inb[:])
            hT = hb.tile([128, 512], BF, tag="hT0")
            for c in range(2):
                pt = psm.tile([128, S * P], F32, tag="psm")
                nc.tensor.matmul(pt, w_t[:, c * 128:(c + 1) * 128], xin_t,
                                 start=True, stop=True)
                nc.scalar.activation(out=hT[:, c * 256:(c + 1) * 256], in_=pt,
                                     func=AF.Identity, bias=b_t[:, c:c + 1],
                                     scale=1.0)

            def t_to_sp(src, dst):
                """src [128, (c,sp)] hT-layout -> dst [128, (r,d)] SP-layout."""
                for c in range(2):
                    for r in range(2):
                        pt = pst.tile([128, 128], F32, tag="pst")
                        nc.tensor.transpose(
                            pt, src[:, c * 256 + r * 128:c * 256 + (r + 1) * 128],
                            i_t)
                        nc.scalar.copy(
                            out=dst[:, r * 256 + c * 128:r * 256 + (c + 1) * 128],
                            in_=pt)

            def t_to_ht(src, dst):
                for r in range(2):
                    for c in range(2):
                        pt = pst.tile([128, 128], F32, tag="pst")
                        nc.tensor.transpose(
                            pt, src[:, r * 256 + c * 128:r * 256 + (c + 1) * 128],
                            i_t)
                        nc.scalar.copy(
                            out=dst[:, c * 256 + r * 128:c * 256 + (r + 1) * 128],
                            in_=pt)

            def tap(nm, t):
                if nm in dbg:
                    nc.gpsimd.dma_start(out=dbg[nm][:], in_=t)

            def load_state(dram_y, dram_q, row0, npart, C):
                sty = hb.tile([npart, C * 256], F32, tag="sty_f")
                stq = hb.tile([npart, C * 576], BF, tag="stq")
                sgq = wk.tile([npart, C * 576], F32, tag="stg_q")
                nc.gpsimd.dma_start(out=sty,
                                    in_=dram_y[row0:row0 + npart, :])
                nc.gpsimd.dma_start(out=sgq, in_=dram_q[row0:row0 + npart, :])
                nc.vector.tensor_copy(stq[:, :], sgq[:, :])
                return sty, stq

            h_sp = None
            for i in range(L):
                # ---- B1. token SRWM ----
                sty, stq = load_state(tkSy, tkSq, i * 128, 128, 2)
                ys_tk = hb.tile([128, S * 2 * 16], F32, tag="ys")
                ht_cur = hT

                def x_tk(s, ht_cur=ht_cur):
                    return ht_cur[:, :].rearrange("p (c sp) -> p c sp", c=2)[
                        :, :, s * 16:(s + 1) * 16]

                _srwm_scan(nc, wk, 128, 2, sty, stq, x_tk, ys_tk)

                hT2 = hb.tile([128, 512], F32, tag="hT2")
                ys_v = ys_tk[:, :].rearrange("p (s c j) -> p c s j", s=S, c=2)
                out_v = hT2[:, :].rearrange("p (c s j) -> p c s j", c=2, j=16)
                g_v = tkg_t[:, i * 16:(i + 1) * 16].unsqueeze(1).unsqueeze(1) \
                    .broadcast_to([128, 2, 16, 16])
                b_v = tkb_t[:, i * 16:(i + 1) * 16].unsqueeze(1).unsqueeze(1) \
                    .broadcast_to([128, 2, 16, 16])
                _ln(nc, wk, "a", 128, 2, 16, 16, out_v, ys_v, g_v, b_v)
                tap(f"d_hT2_{i}", hT2)

                # ---- B2. token mixer ----
                h2 = hb.tile([128, 512], F32, tag="h2")
                t_to_sp(hT2, h2)
                hn = hb.tile([128, 512], F32, tag="hn")
                hn_v = hn[:, :].rearrange("p (a r d) -> p a r d", a=1, r=2)
                h2_v = h2[:, :].rearrange("p (a r d) -> p a r d", a=1, r=2)
                g_v = tkmg_t[:, i * 256:(i + 1) * 256].unsqueeze(1).unsqueeze(1) \
                    .broadcast_to([128, 1, 2, 256])
                b_v = tkmb_t[:, i * 256:(i + 1) * 256].unsqueeze(1).unsqueeze(1) \
                    .broadcast_to([128, 1, 2, 256])
                _ln(nc, wk, "b", 128, 1, 2, 256, hn_v, h2_v, g_v, b_v)
                # patch-axis FFN as block-diagonal matmuls in SP layout:
                # block b covers s8 in {2b, 2b+1} (partition rows 32b:32b+32)
                mxsp = hb.tile([128, 512], F32, tag="mxsp")
                pt2 = psm.tile([128, 512], F32, tag="psmW2")
                for b in range(4):
                    pt1 = psm.tile([128, 512], F32, tag="psm")
                    nc.tensor.matmul(
                        pt1,
                        tkmW1B_t[:, i * 512 + b * 128:i * 512 + (b + 1) * 128],
                        hn, start=True, stop=True)
                    glb = wk.tile([128, 512], F32, tag="mx_gl")
                    _gelu(nc, wk, glb[:, :], pt1[:, :],
                          tkmB1c_t[:, i:i + 1], 128, 512)
                    nc.tensor.matmul(
                        pt2,
                        tkmW2B_t[:, i * 512 + b * 128:i * 512 + (b + 1) * 128],
                        glb, start=(b == 0), stop=(b == 3))
                nc.scalar.activation(out=mxsp[:, :], in_=pt2[:, :],
                                     func=AF.Identity,
                                     bias=tkmB2c_t[:, i:i + 1], scale=1.0)
                h3 = hb.tile([128, 512], F32, tag="h3")
                nc.vector.tensor_add(h3[:, :], h2[:, :], mxsp[:, :])
                tap(f"d_h3_{i}", h3)

                # ---- B3. channel SRWM ----
                dr1 = dr.tile([S, P, H, DH], F32, tag="dr1")
                for r in range(2):
                    nc.gpsimd.dma_start(
                        out=dr1[r * 8:(r + 1) * 8, :, :, :].rearrange(
                            "s p hh j -> s p (hh j)"),
                        in_=h3[:, r * 256:(r + 1) * 256])
                xc = hb.tile([128, 512], F32, tag="xc")
                for c in range(2):
                    nc.gpsimd.dma_start(
                        out=xc[:, :].rearrange("q (c s j) -> q c s j",
                                               c=2, s=S)[:, c],
                        in_=dr1[:, c * 8:(c + 1) * 8, :, :].rearrange(
                            "s ph hh j -> (ph hh) s j"))
                xcb = hb.tile([128, 512], BF, tag="xcb")
                nc.vector.tensor_copy(xcb[:, :], xc[:, :])
                sty, stq = load_state(chSy, chSq, i * 128, 128, 2)
                ys_ch = hb.tile([128, S * 2 * 16], F32, tag="ys")

                def x_ch(s, xcb=xcb):
                    return xcb[:, :].rearrange("p (c s j) -> p c s j",
                                               c=2, s=S)[:, :, s, :]

                _srwm_scan(nc, wk, 128, 2, sty, stq, x_ch, ys_ch)

                dr2 = dr.tile([S, P, H, DH], F32, tag="dr2")
                for c in range(2):
                    nc.gpsimd.dma_start(
                        out=dr2[:, c * 8:(c + 1) * 8, :, :].rearrange(
                            "s ph hh i -> (ph hh) s i"),
                        in_=ys_ch[:, :].rearrange(
                            "q (s c j) -> q s c j", s=S, c=2)[:, :, c, :])
                ysp = hb.tile([128, 512], F32, tag="ysp")
                for r in range(2):
                    nc.gpsimd.dma_start(
                        out=ysp[:, r * 256:(r + 1) * 256],
                        in_=dr2[r * 8:(r + 1) * 8, :, :, :].rearrange(
                            "s p hh i -> s p (hh i)"))
                h4 = hb.tile([128, 512], F32, tag="h4")
                h4_v = h4[:, :].rearrange("p (a r d) -> p a r d", a=1, r=2)
                ysp_v = ysp[:, :].rearrange("p (a r d) -> p a r d", a=1, r=2)
                g_v = chg_t[:, i * 256:(i + 1) * 256].unsqueeze(1).unsqueeze(1) \
                    .broadcast_to([128, 1, 2, 256])
                b_v = chb_t[:, i * 256:(i + 1) * 256].unsqueeze(1).unsqueeze(1) \
                    .broadcast_to([128, 1, 2, 256])
                _ln(nc, wk, "c", 128, 1, 2, 256, h4_v, ysp_v, g_v, b_v)
                tap(f"d_h4_{i}", h4)

                # ---- B4. channel mixer ----
                hn2 = hb.tile([128, 512], F32, tag="hn2")
                hn2_v = hn2[:, :].rearrange("p (a r d) -> p a r d", a=1, r=2)
                g_v = chmg_t[:, i * 256:(i + 1) * 256].unsqueeze(1).unsqueeze(1) \
                    .broadcast_to([128, 1, 2, 256])
                b_v = chmb_t[:, i * 256:(i + 1) * 256].unsqueeze(1).unsqueeze(1) \
                    .broadcast_to([128, 1, 2, 256])
                _ln(nc, wk, "d", 128, 1, 2, 256, hn2_v, h4_v, g_v, b_v)
                hn2T = hb.tile([128, 512], F32, tag="hn2T")
                t_to_ht(hn2, hn2T)
                pt1 = psm.tile([128, 256], F32, tag="psm")
                for c in range(2):
                    nc.tensor.matmul(
                        pt1, chmW1_t[:, i * 256 + c * 128:i * 256 + (c + 1) * 128],
                        hn2T[:, c * 256:(c + 1) * 256],
                        start=(c == 0), stop=(c == 1))
                gl2 = hb.tile([128, 256], F32, tag="gl2")
                _gelu(nc, wk, gl2[:, :], pt1[:, :], chmB1_t[:, i:i + 1],
                      128, 256)
                moT = hb.tile([128, 512], F32, tag="moT")
                for c in range(2):
                    pt2 = psm.tile([128, 256], F32, tag="psm")
                    nc.tensor.matmul(
                        pt2, chmW2_t[:, i * 256 + c * 128:i * 256 + (c + 1) * 128],
                        gl2, start=True, stop=True)
                    nc.scalar.activation(out=moT[:, c * 256:(c + 1) * 256],
                                         in_=pt2, func=AF.Identity,
                                         bias=chmB2_t[:, i * 2 + c:i * 2 + c + 1],
                                         scale=1.0)
                mosp = hb.tile([128, 512], F32, tag="mosp")
                t_to_sp(moT, mosp)
                h5 = hb.tile([128, 512], F32, tag="h5")
                nc.vector.tensor_add(h5[:, :], h4[:, :], mosp[:, :])
                h_sp = h5
                tap(f"d_h5_{i}", h5)

                if i + 1 < L:
                    hT = hb.tile([128, 512], BF, tag="hT0")
                    t_to_ht(h_sp, hT)

            # ---- C. final ----
            hf = hb.tile([128, 512], F32, tag="hf")
            hf_v = hf[:, :].rearrange("p (a r d) -> p a r d", a=1, r=2)
            hsp_v = h_sp[:, :].rearrange("p (a r d) -> p a r d", a=1, r=2)
            g_v = flng_t[:, :].unsqueeze(1).unsqueeze(1) \
                .broadcast_to([128, 1, 2, 256])
            b_v = flnb_t[:, :].unsqueeze(1).unsqueeze(1) \
                .broadcast_to([128, 1, 2, 256])
            _ln(nc, wk, "f", 128, 1, 2, 256, hf_v, hsp_v, g_v, b_v)
            pm = psm.tile([16, 256], F32, tag="psm")
            for r in range(2):
                nc.tensor.matmul(pm, mean_t[:, r * 16:(r + 1) * 16],
                                 hf[:, r * 256:(r + 1) * 256],
                                 start=(r == 0), stop=(r == 1))
            ho = hb.tile([16, 256], F32, tag="ho")
            nc.scalar.copy(out=ho, in_=pm)
            tap("d_ho", ho)
            dr4 = dr.tile([S, H, DH], F32, tag="dr4")
            nc.gpsimd.dma_start(
                out=dr4[:, :, :],
                in_=ho[:, :].rearrange("s (hh j) -> s hh j", hh=H))
            xo = hb.tile([16, S * DH], F32, tag="xo")
            nc.gpsimd.dma_start(
                out=xo[:, :].rearrange("p (s j) -> p s j", s=S),
                in_=dr4[:, :, :].rearrange("s hh j -> hh s j"))
            xob = hb.tile([16, S * DH], BF, tag="xob")
            nc.vector.tensor_copy(xob[:, :], xo[:, :])
            sty, stq = load_state(oSy, oSq, 0, 16, 1)
            ys_o = hb.tile([16, S * 16], F32, tag="ys_o")

            def x_o(s, xob=xob):
                return xob[:, :].rearrange("p (c s j) -> p c s j", c=1, s=S)[
                    :, :, s, :]

            _srwm_scan(nc, wk, 16, 1, sty, stq, x_o, ys_o, use_gp=False)

            dr5 = dr.tile([H, S, DH], F32, tag="dr5")
            nc.gpsimd.dma_start(
                out=dr5[:, :, :],
                in_=ys_o[:, :].rearrange("p (s j) -> p s j", s=S))
            ho2 = hb.tile([16, 256], F32, tag="ho2")
            nc.gpsimd.dma_start(
                out=ho2[:, :].rearrange("s (hh i) -> s hh i", hh=H),
                in_=dr5[:, :, :].rearrange("hh s i -> s hh i"))
            hon = hb.tile([16, 256], F32, tag="hon")
            _ln(nc, wk, "o", 16, 1, 1, 256,
                hon[:, :].unsqueeze(1).unsqueeze(1),
                ho2[:, :].unsqueeze(1).unsqueeze(1),
                og_t[:, :].unsqueeze(1).unsqueeze(1),
                ob_t[:, :].unsqueeze(1).unsqueeze(1))
            hoT = hb.tile([128, 32], F32, tag="hoT")
            for c in range(2):
                pt = pst.tile([128, 128], F32, tag="pst")
                nc.tensor.transpose(pt[:, 0:16], hon[:, c * 128:(c + 1) * 128],
                                    i_t[0:16, 0:16])
                nc.scalar.copy(out=hoT[:, c * 16:(c + 1) * 16], in_=pt[:, 0:16])
            po = psm.tile([16, 5], F32, tag="psm")
            for c in range(2):
                nc.tensor.matmul(po, hoT[:, c * 16:(c + 1) * 16],
                                 outW_t[:, c * 5:(c + 1) * 5],
                                 start=(c == 0), stop=(c == 1))
            fin = hb.tile([16, 5], F32, tag="fin")
            nc.vector.tensor_add(fin[:, :], po[:, :], outB_t[:, :])
            nc.gpsimd.dma_start(out=out[:], in_=fin)

    nc.finalize()
    return nc


# ------------------------- host marshaling -------------------------

def _patchify(x):
    s, bb, c, hh, ww = x.shape
    h, w = hh // PS, ww // PS
    x = x.reshape(s, bb, c, h, PS, w, PS)
    x = x.transpose(0, 1, 3, 5, 4, 6, 2)
    return x.reshape(s, bb, h * w, PS * PS * c)


def _state_init(Wy, Wq, Wk, wb, pair_of, npart=128, C=2):
    sy = np.zeros((npart, C, 16, 16), np.float32)
    sq = np.zeros((npart, C, 36, 16), np.float32)
    for c in range(C):
        for q in range(npart):
            h = pair_of(c, q)
            sy[q, c] = Wy[h]
            sq[q, c, 0:16] = Wq[h]
            sq[q, c, 16:32] = Wk[h]
            # wb rows stored negated: seg_b = -logit, so sigmoid(logit)
            # = 1/(1+exp(seg_b)) shares the single per-step exp, and the
            # negated rows are self-consistent under the delta update.
            sq[q, c, 32:36] = -wb[h]
    return sy.reshape(npart, C * 256), sq.reshape(npart, C * 576)


def marshal(ins):
    """Returns (xin (S,B,P,QIN), shared input dict)."""
    x, fb = ins["x"].astype(np.float32), ins["fb"]
    xp = _patchify(x)
    emb = np.zeros((S, B, NCLS), np.float32)
    emb[np.arange(S)[:, None], np.arange(B)[None, :], fb] = 1.0
    emb = np.broadcast_to(emb[:, :, None, :], (S, B, P, NCLS))
    xin = np.concatenate([xp, emb], -1)

    f32 = lambda k: np.asarray(ins[k], np.float32)
    sh = {}
    sh["inW"] = np.ascontiguousarray(f32("in_W"))
    sh["inb"] = f32("in_b").reshape(2, 128).T.copy()
    sh["I128"] = np.eye(128, dtype=np.float32)
    mean = np.zeros((128, 32), np.float32)
    for r in range(2):
        for sp in range(128):
            s = r * 8 + sp // 16
            mean[sp, r * 16 + s] = 1.0 / 16.0
    sh["MEAN"] = mean
    tky, tkq = zip(*[
        _state_init(f32("tk_Wy")[i], f32("tk_Wq")[i], f32("tk_Wk")[i],
                    f32("tk_wb")[i], lambda c, q: 0) for i in range(L)])
    sh["tkSy"] = np.concatenate(tky, 0)
    sh["tkSq"] = np.concatenate(tkq, 0)
    chy, chq = zip(*[
        _state_init(f32("ch_Wy")[i], f32("ch_Wq")[i], f32("ch_Wk")[i],
                    f32("ch_wb")[i], lambda c, q: q % 16) for i in range(L)])
    sh["chSy"] = np.concatenate(chy, 0)
    sh["chSq"] = np.concatenate(chq, 0)
    oy, oq = _state_init(f32("o_Wy"), f32("o_Wq"), f32("o_Wk"), f32("o_wb"),
                         lambda c, q: q, npart=16, C=1)
    sh["oSy"] = oy
    sh["oSq"] = oq
    rep = lambda a, n=128: np.broadcast_to(
        np.asarray(a, np.float32).reshape(1, -1),
        (n, np.asarray(a).size)).copy()
    sh["tkg"] = rep(f32("tk_lng"))
    sh["tkb"] = rep(f32("tk_lnb"))
    sh["tkmg"] = rep(f32("tkm_g"))
    sh["tkmb"] = rep(f32("tkm_b"))
    sh["chg"] = rep(f32("ch_lng"))
    sh["chb"] = rep(f32("ch_lnb"))
    sh["chmg"] = rep(f32("chm_g"))
    sh["chmb"] = rep(f32("chm_b"))
    sh["flng"] = rep(f32("fln_g"))
    sh["flnb"] = rep(f32("fln_b"))
    sh["og"] = rep(f32("o_lng"), 16)
    sh["ob"] = rep(f32("o_lnb"), 16)
    w1blk = np.zeros((128, L * 512), np.float32)
    w2blk = np.zeros((128, L * 512), np.float32)
    b1c = np.zeros((128, L), np.float32)
    b2c = np.zeros((128, L), np.float32)
    for i in range(L):
        W1, W2 = f32("tkm_W1")[i], f32("tkm_W2")[i]
        for b in range(4):
            for sb in range(2):
                s8 = 2 * b + sb
                w1blk[s8 * 16:(s8 + 1) * 16,
                      i * 512 + b * 128 + sb * 64:
                      i * 512 + b * 128 + (sb + 1) * 64] = W1
        for b in range(4):
            for sb in range(2):
                s8 = 2 * b + sb
                w2blk[sb * 64:(sb + 1) * 64,
                      i * 512 + b * 128 + s8 * 16:
                      i * 512 + b * 128 + (s8 + 1) * 16] = W2
        for sb in range(2):
            b1c[sb * 64:(sb + 1) * 64, i] = f32("tkm_b1")[i]
        b2c[:, i] = np.tile(f32("tkm_b2")[i], 8)
    sh["tkmW1B"] = w1blk
    sh["tkmW2B"] = w2blk
    sh["tkmB1c"] = b1c
    sh["tkmB2c"] = b2c
    # chmW1[i] is (D=256, DFT=128); lhsT chunk c = chm_W1[i][c*128:(c+1)*128, :]
    sh["chmW1"] = np.concatenate(
        [f32("chm_W1")[i][c * 128:(c + 1) * 128, :]
         for i in range(L) for c in range(2)], 1)
    sh["chmB1"] = np.stack([f32("chm_b1")[i] for i in range(L)], 1)
    # chmW2[i] is (DFT=128, D=256); lhsT chunk c = chm_W2[i][:, c*128:(c+1)*128]
    sh["chmW2"] = np.concatenate(
        [f32("chm_W2")[i][:, c * 128:(c + 1) * 128]
         for i in range(L) for c in range(2)], 1)
    # chmB2: bias per d; chunk c column holds b2[c*128:(c+1)*128]
    sh["chmB2"] = np.stack(
        [f32("chm_b2")[i][c * 128:(c + 1) * 128]
         for i in range(L) for c in range(2)], 1)
    sh["outW"] = np.concatenate(
        [f32("out_W")[c * 128:(c + 1) * 128, :] for c in range(2)], 1)
    sh["outB"] = rep(f32("out_b"), 16)
    return xin, sh


def in_maps_for(xin, sh):
    maps = []
    for b in range(B):
        m = dict(sh)
        m["xinT"] = np.ascontiguousarray(
            xin[:, b].reshape(S * P, QIN).T)
        maps.append(m)
    return maps


from concourse.bass_utils import run_bass_kernel_spmd

_CACHE = {}


def kernel(**inputs):
    ins = {k: np.ascontiguousarray(np.asarray(v)) for k, v in inputs.items()}
    if "nc" not in _CACHE:
        _CACHE["nc"] = build_nc()
    nc = _CACHE["nc"]
    xin, sh = marshal(ins)
    maps = in_maps_for(xin, sh)
    res = run_bass_kernel_spmd(nc, maps, core_ids=list(range(8)))
    out = np.stack([res.results[c]["out"] for c in range(B)], axis=1)
    return out.astype(np.float32)
